# revision 25
# baseline (speedup 1.0000x reference)
"""Trainium2 Bass kernel for nn_ARTLayer (gnn_message_passing).

Math (reference):
    j(i,t) = t + (t>=i)                                    # [K, K-1] neighbor index
    alpha  = sigmoid(x@wa [i] + x@wb [j] + pf@wc + b_att)  # [K, K-1]
    msgs   = mean_t alpha * ((x@WobjT + b_obj)[j] + pf@WpairT + b_pair)
    out    = LN(x + msgs); out = LN(out + FFN(out))

Key algebraic rewrite (removes the 34-GFLOP [P,PD]x[PD,H] einsum):
    sum_t a*(pf@WpT)  = (sum_t a*pf) @ WpT              -> U[i,:] @ WpT
    sum_t a*oj[j]     = (A @ x) @ WobjT                  -> Gx[i,:] @ WoT
      with A[i,j] decomposed via lo/hi shifted views of x and a t>=i mask
    sum_t a*(b_obj+b_pair) = s_alpha[i] * bop

Sharding: rows i split across 8 cores (64 each); small tensors replicated;
host concatenates the per-core [64, 512] outputs.

Per-edge work (pf pass) runs in fp16 (DVE 2x mode + half DMA traffic);
sigmoid argument assembly, PSUM accumulation and the tail are fp32.

v2 optimizations (from NTFF profile of v1 @148us):
  - wc is folded into pf on the host (pf_sent[:,j] = pf[:,j]*scale_j with
    scale_j = sign(wc_j)*max(|wc_j|, 6e-5)); sc becomes a pure add-reduce and
    U is recovered exactly via W_pairT rows pre-divided by scale_j.
  - the sc reduce runs as an fp16 halving tree (2x DVE mode) instead of a
    1x-mode tensor_reduce with fp32 output.
  - U accumulation is alpha-stationary: lhsT = alpha column (1-wide weight
    load), rhs = pf tile streaming at N=128; U rows land packed in 4 PSUM
    banks at 32-aligned partition slots and are reassembled via a small
    DRAM bounce + one PE transpose.
"""
import numpy as np

import concourse.bass as bass
import concourse.tile as tile
from concourse import bacc, mybir

F32, F16 = mybir.dt.float32, mybir.dt.float16
AX = mybir.AxisListType
OP = mybir.AluOpType
AF = mybir.ActivationFunctionType

K, D, H, PD = 512, 512, 512, 128
T = K - 1                      # 511 neighbors per row
NCORES, IPC = 8, 64            # rows per core
NCH = 4                        # t-chunks of 128 (last chunk row 127 is t=511 pad)
IB, NIB = 32, 2                # i-block within a core
EPS = 1e-5


def build_program() -> bacc.Bacc:
    nc = bacc.Bacc("TRN2", target_bir_lowering=False, debug=False)

    def inp(name, shape, dt):
        return nc.dram_tensor(name, shape, dt, kind="ExternalInput").ap()

    pf = inp("pf", [IPC * T + 1, PD], F16)      # per-core edge shard, +1 zero pad row
    xf = inp("xf", [K + 1, D], F16)             # x with one zero pad row
    xi = inp("xi", [IPC, D], F32)               # this core's rows of x
    xi16 = inp("xi16", [IPC, D], F16)
    mask_lt = inp("mask_lt", [128, NCH, IPC], F32)   # t <  i_global
    mask_ge = inp("mask_ge", [128, NCH, IPC], F16)   # t >= i_global and t <= 510
    vmask = inp("vmask", [128, NCH], F32)            # 1/511, 0 at t=511 slot
    ident = inp("ident", [128, 128], F32)
    ones = inp("ones", [128, 128], F32)
    shift1 = inp("shift1", [128, 128], F32)     # [q,p] = (q == p+1)
    shift2 = inp("shift2", [128, 128], F32)     # [q,p] = (q==0)&(p==127)
    ones16 = inp("ones16", [128, 8], F16)
    zeros = inp("zeros", [8, 8], F32)
    wa = inp("wa", [D], F16)
    wb = inp("wb", [D], F16)
    b_att = inp("b_att", [1], F32)
    bop = inp("bop", [H], F32)                  # b_obj + b_pair
    b1 = inp("b1", [H], F32)
    b2 = inp("b2", [H], F32)
    lng = inp("lng", [H], F32)
    lnb = inp("lnb", [H], F32)
    WpT = inp("WpT", [PD, H], F32)              # W_pair.T
    WoT = inp("WoT", [D, H], F32)               # W_obj.T
    W1T = inp("W1T", [H, H], F32)
    W2T = inp("W2T", [H, H], F32)

    out_d = nc.dram_tensor("out", [IPC, H], F32, kind="ExternalOutput").ap()
    u_dram = nc.dram_tensor("u_dram", [4 * 128 * 512], F32).ap()  # U bank dumps

    with tile.TileContext(nc) as tc:
        with (
            tc.tile_pool(name="const", bufs=1) as cpool,
            tc.tile_pool(name="pfp", bufs=3) as pfp,
            tc.tile_pool(name="scrp", bufs=2) as scrp,
            tc.tile_pool(name="smallp", bufs=4) as smallp,
            tc.tile_pool(name="postp", bufs=2) as postp,
            tc.tile_pool(name="pss", bufs=2, space="PSUM") as pss,
            tc.tile_pool(name="psflex", bufs=4, space="PSUM") as psflex,
            tc.tile_pool(name="psmp", bufs=1, space="PSUM") as psmp,
            tc.tile_pool(name="psgp", bufs=1, space="PSUM") as psgp,
        ):
            # ---- constants & weights to SBUF ----
            wa_b = cpool.tile([IPC, D], F16)
            nc.sync.dma_start(out=wa_b, in_=wa[None, :].to_broadcast([IPC, D]))
            wb_b = cpool.tile([128, D], F16)
            nc.sync.dma_start(out=wb_b, in_=wb[None, :].to_broadcast([128, D]))
            id_sb = cpool.tile([128, 128], F32)
            nc.sync.dma_start(out=id_sb, in_=ident)
            ones_sb = cpool.tile([128, 128], F32)
            nc.sync.dma_start(out=ones_sb, in_=ones)
            sh1_sb = cpool.tile([128, 128], F32)
            nc.sync.dma_start(out=sh1_sb, in_=shift1)
            sh2_sb = cpool.tile([128, 128], F32)
            nc.sync.dma_start(out=sh2_sb, in_=shift2)
            ones16_sb = cpool.tile([128, 8], F16)
            nc.sync.dma_start(out=ones16_sb, in_=ones16)
            mlt_sb = cpool.tile([128, NCH, IPC], F32)
            nc.sync.dma_start(out=mlt_sb, in_=mask_lt)
            mge_sb = cpool.tile([128, NCH, IPC], F16)
            nc.sync.dma_start(out=mge_sb, in_=mask_ge)
            vm_sb = cpool.tile([128, NCH], F32)
            nc.sync.dma_start(out=vm_sb, in_=vmask)
            WpT_sb = cpool.tile([128, H], F32)
            nc.sync.dma_start(out=WpT_sb, in_=WpT)
            WoT_sb = cpool.tile([128, NCH, H], F32)
            nc.sync.dma_start(out=WoT_sb, in_=WoT.rearrange("(c p) h -> p c h", p=128))
            W1T_sb = cpool.tile([128, NCH, H], F32)
            nc.sync.dma_start(out=W1T_sb, in_=W1T.rearrange("(c p) h -> p c h", p=128))
            W2T_sb = cpool.tile([128, NCH, H], F32)
            nc.sync.dma_start(out=W2T_sb, in_=W2T.rearrange("(c p) h -> p c h", p=128))
            gb_sb = cpool.tile([IPC, H], F32)
            nc.sync.dma_start(out=gb_sb, in_=lng[None, :].to_broadcast([IPC, H]))
            bb_sb = cpool.tile([IPC, H], F32)
            nc.sync.dma_start(out=bb_sb, in_=lnb[None, :].to_broadcast([IPC, H]))
            b1_sb = cpool.tile([IPC, H], F32)
            nc.sync.dma_start(out=b1_sb, in_=b1[None, :].to_broadcast([IPC, H]))
            b2_sb = cpool.tile([IPC, H], F32)
            nc.sync.dma_start(out=b2_sb, in_=b2[None, :].to_broadcast([IPC, H]))
            bop_row = cpool.tile([1, H], F32)
            nc.sync.dma_start(out=bop_row, in_=bop[None, :])
            xi_sb = cpool.tile([IPC, D], F32)
            nc.sync.dma_start(out=xi_sb, in_=xi)
            xi16_sb = cpool.tile([IPC, D], F16)
            nc.sync.dma_start(out=xi16_sb, in_=xi16)
            xlo = cpool.tile([128, NCH, D], F16)
            nc.sync.dma_start(out=xlo, in_=xf[0:K].rearrange("(c p) d -> p c d", p=128))
            xhi = cpool.tile([128, NCH, D], F16)
            nc.sync.dma_start(out=xhi, in_=xf[1:K + 1].rearrange("(c p) d -> p c d", p=128))

            dx = cpool.tile([128, NCH, D], F16)
            nc.vector.tensor_tensor(dx, xhi, xlo, OP.subtract)

            eps_col = cpool.tile([IPC, 1], F32)
            nc.vector.memset(eps_col, EPS)

            # ---- sa (this core's rows) and sb (all rows) ----
            scr_sa = smallp.tile([IPC, D], F16)
            nc.vector.tensor_mul(scr_sa, xi16_sb, wa_b)
            sa_col = smallp.tile([IPC, 1], F32)
            nc.vector.tensor_reduce(sa_col, scr_sa, axis=AX.X, op=OP.add)
            b_att_col = smallp.tile([IPC, 1], F32)
            nc.sync.dma_start(out=b_att_col, in_=b_att[None, :].to_broadcast([IPC, 1]))
            nc.vector.tensor_add(sa_col, sa_col, b_att_col)
            sa_diag = smallp.tile([IPC, IPC], F32)
            nc.vector.tensor_mul(sa_diag, id_sb[0:IPC, 0:IPC], sa_col.to_broadcast([IPC, IPC]))

            sb_cols = smallp.tile([128, NCH], F32)
            for c in range(NCH):
                scr_sb = smallp.tile([128, D], F16)
                nc.vector.tensor_mul(scr_sb, xlo[:, c, :], wb_b)
                nc.vector.tensor_reduce(
                    sb_cols[:, c:c + 1], scr_sb, axis=AX.X, op=OP.add)

            # sb_hi[p, c] = sb[c*128+p+1] via shift matmuls (no DRAM bounce);
            # slot t=511 -> sb[512] stays 0, as required.
            sbhi_ps = pss.tile([128, NCH], F32, tag="ps_small")
            nc.tensor.matmul(sbhi_ps, sh1_sb, sb_cols, start=True, stop=False)
            nc.tensor.matmul(sbhi_ps[:, 0:NCH - 1], sh2_sb, sb_cols[:, 1:NCH],
                             start=False, stop=True)
            sbhi_cols = smallp.tile([128, NCH], F32)
            nc.vector.tensor_copy(sbhi_cols, sbhi_ps)

            # ---- SBJ[t, i] = sa[i] + b_att + sb_hi[t] + mask_lt*(sb_lo - sb_hi) ----
            sbj = cpool.tile([128, NCH, IPC], F32)
            for c in range(NCH):
                diffc = smallp.tile([128, 1], F32)
                nc.vector.tensor_tensor(
                    diffc, sb_cols[:, c:c + 1], sbhi_cols[:, c:c + 1], OP.subtract)
                diagc = smallp.tile([128, 128], F32)
                nc.vector.tensor_mul(diagc, id_sb, diffc.to_broadcast([128, 128]))
                diagb = smallp.tile([128, 128], F32)
                nc.vector.tensor_mul(
                    diagb, id_sb, sbhi_cols[:, c:c + 1].to_broadcast([128, 128]))
                ps_sbj = pss.tile([128, IPC], F32, tag="ps_small")
                nc.tensor.matmul(ps_sbj, ones_sb[0:IPC, :], sa_diag, start=True, stop=False)
                nc.tensor.matmul(ps_sbj, diagb, ones_sb[:, 0:IPC], start=False, stop=False)
                nc.tensor.matmul(ps_sbj, diagc, mlt_sb[:, c, :], start=False, stop=True)
                nc.vector.tensor_copy(sbj[:, c, :], ps_sbj)

            # ---- main edge pass ----
            alpha_full = cpool.tile([128, NCH, IPC], F16)   # alpha / 511
            age_full = cpool.tile([128, NCH, IPC], F16)     # masked (t>=i) alpha / 511
            gx_ps = psgp.tile([IPC, D], F32)                # Gx[i, d] = sum_t a*x[j]
            msg_ps = psmp.tile([IPC, H], F32)
            # U rows: 16 i per PSUM bank, at 32-aligned partition slots x 4
            # free offsets of 128. u_ps[b] row (s, f) holds U[i] for
            # i = 16 b + 4 (s/32) + f.
            u_ps = [psflex.tile([128, 512], F32, tag="flex", name=f"u_ps{b}")
                    for b in range(4)]
            for b in range(4):
                # init unused partitions so the post-loop full-bank copy is clean
                nc.vector.memset(u_ps[b], 0.0)

            for c in range(NCH):
                for ib in range(NIB):
                    i0 = ib * IB
                    pf_t = pfp.tile([128, IB, PD], F16)
                    src = bass.AP(
                        tensor=pf.tensor,
                        offset=(i0 * T + c * 128) * PD,
                        ap=[[PD, 128], [T * PD, IB], [1, PD]],
                    )
                    nc.sync.dma_start(out=pf_t, in_=src)
                    # sc = sum_pd pf_sent (wc pre-folded): fp16 halving tree
                    scr = scrp.tile([128, IB, 64], F16)
                    nc.vector.tensor_add(scr, pf_t[:, :, 0:64], pf_t[:, :, 64:128])
                    w = 32
                    while w >= 2:
                        nc.vector.tensor_add(
                            scr[:, :, 0:w], scr[:, :, 0:w], scr[:, :, w:2 * w])
                        w //= 2
                    sc_t = smallp.tile([128, IB], F32)
                    nc.vector.tensor_add(sc_t, scr[:, :, 0], scr[:, :, 1])
                    aarg = smallp.tile([128, IB], F32)
                    nc.vector.tensor_add(aarg, sc_t, sbj[:, c, i0:i0 + IB])
                    alph = smallp.tile([128, IB], F32)
                    nc.scalar.activation(alph, aarg, AF.Sigmoid)
                    nc.vector.tensor_mul(
                        alpha_full[:, c, i0:i0 + IB], alph,
                        vm_sb[:, c:c + 1].to_broadcast([128, IB]))
                    nc.vector.tensor_mul(
                        age_full[:, c, i0:i0 + IB], alpha_full[:, c, i0:i0 + IB],
                        mge_sb[:, c, i0:i0 + IB])
                    for k in range(IB):
                        i = i0 + k
                        b, rem = divmod(i, 16)
                        s, f = 32 * (rem // 4), rem % 4
                        nc.tensor.matmul(
                            u_ps[b][s:s + 1, f * 128:(f + 1) * 128],
                            alpha_full[:, c, i:i + 1], pf_t[:, k, :],
                            start=(c == 0 and f == 0),
                            stop=(c == NCH - 1 and f == 3),
                            tile_position=(0, s))
                nc.tensor.matmul(gx_ps, alpha_full[:, c, :], xlo[:, c, :],
                                 start=(c == 0), stop=False)
                nc.tensor.matmul(gx_ps, age_full[:, c, :], dx[:, c, :],
                                 start=False, stop=(c == NCH - 1))

            # ---- messages = U@WpT + Gx@WoT + s_alpha x bop  (all alpha-scaled 1/511) ----
            s_ps = pss.tile([IPC, 1], F32, tag="ps_small")
            for c in range(NCH):
                nc.tensor.matmul(s_ps, alpha_full[:, c, :], ones16_sb[:, 0:1],
                                 start=(c == 0), stop=(c == NCH - 1))
            s_col = smallp.tile([IPC, 1], F32)
            nc.vector.tensor_copy(s_col, s_ps)
            ps_sr = pss.tile([1, IPC], F32, tag="ps_small")
            nc.tensor.transpose(ps_sr, s_col, id_sb[0:IPC, 0:IPC])
            s_row = smallp.tile([1, IPC], F32)
            nc.vector.tensor_copy(s_row, ps_sr)

            # U reassembly: full-bank psum->sbuf copies, DRAM dump, then one
            # gather DMA picks row (b, s, f) -> i = 16b + 4(s/32) + f.
            for b in range(4):
                u_cp = postp.tile([128, 512], F32, tag="u_cp")
                nc.vector.tensor_copy(u_cp, u_ps[b])
                nc.sync.dma_start(
                    out=u_dram[b * 65536:(b + 1) * 65536].rearrange(
                        "(p f) -> p f", p=128),
                    in_=u_cp)
            u_sb64 = postp.tile([IPC, PD], F32)
            gather = bass.AP(tensor=u_dram.tensor, offset=0,
                             ap=[[65536, 4], [16384, 4], [128, 4], [1, 128]])
            nc.sync.dma_start(out=u_sb64, in_=gather)
            ps_ut = pss.tile([128, IPC], F32, tag="ps_small")
            nc.tensor.transpose(ps_ut, u_sb64, id_sb[0:IPC, 0:IPC])
            u_sb = postp.tile([128, IPC], F32)
            nc.vector.tensor_copy(u_sb, ps_ut)
            gx_sb = postp.tile([IPC, D], F32)
            nc.vector.tensor_copy(gx_sb, gx_ps)
            gxT = postp.tile([128, NCH, IPC], F32)
            for c in range(NCH):
                ptg = pss.tile([128, IPC], F32, tag="ps_small")
                nc.tensor.transpose(ptg, gx_sb[:, c * 128:(c + 1) * 128], id_sb[0:IPC, 0:IPC])
                nc.vector.tensor_copy(gxT[:, c, :], ptg)

            nc.tensor.matmul(msg_ps, u_sb, WpT_sb, start=True, stop=False)
            for c in range(NCH):
                nc.tensor.matmul(msg_ps, gxT[:, c, :], WoT_sb[:, c, :],
                                 start=False, stop=False)
            nc.tensor.matmul(msg_ps, s_row, bop_row, start=False, stop=True)

            # ---- residual + LN1 ----
            def layer_norm(v):
                stats = smallp.tile([IPC, 6], F32)
                nc.vector.bn_stats(out=stats, in_=v)
                mv = smallp.tile([IPC, 2], F32)
                nc.vector.bn_aggr(out=mv, in_=stats)
                std = smallp.tile([IPC, 1], F32)
                nc.scalar.activation(std, mv[:, 1:2], AF.Sqrt, bias=eps_col)
                rstd = smallp.tile([IPC, 1], F32)
                nc.vector.reciprocal(rstd, std)
                cen = postp.tile([IPC, H], F32)
                nc.vector.tensor_scalar(cen, v, mv[:, 0:1], rstd, OP.subtract, OP.mult)
                o = postp.tile([IPC, H], F32)
                nc.vector.tensor_mul(o, cen, gb_sb)
                nc.vector.tensor_add(o, o, bb_sb)
                return o

            h_sb = postp.tile([IPC, H], F32)
            nc.vector.tensor_add(h_sb, xi_sb, msg_ps)
            out1 = layer_norm(h_sb)

            # ---- FFN ----
            def transpose_rows(v):
                vT = postp.tile([128, NCH, IPC], F32, tag="vT")
                for c in range(NCH):
                    ptt = pss.tile([128, IPC], F32, tag="ps_small")
                    nc.tensor.transpose(ptt, v[:, c * 128:(c + 1) * 128], id_sb[0:IPC, 0:IPC])
                    nc.vector.tensor_copy(vT[:, c, :], ptt)
                return vT

            o1T = transpose_rows(out1)
            f1_ps = psflex.tile([IPC, H], F32, tag="flex")
            for c in range(NCH):
                nc.tensor.matmul(f1_ps, o1T[:, c, :], W1T_sb[:, c, :],
                                 start=(c == 0), stop=(c == NCH - 1))
            f1 = postp.tile([IPC, H], F32)
            nc.vector.tensor_add(f1, f1_ps, b1_sb)
            nc.vector.tensor_scalar_max(f1, f1, 0.0)

            f1T = transpose_rows(f1)
            f2_ps = psflex.tile([IPC, H], F32, tag="flex")
            for c in range(NCH):
                nc.tensor.matmul(f2_ps, f1T[:, c, :], W2T_sb[:, c, :],
                                 start=(c == 0), stop=(c == NCH - 1))
            h2 = postp.tile([IPC, H], F32)
            nc.vector.tensor_add(h2, f2_ps, b2_sb)
            nc.vector.tensor_add(h2, h2, out1)
            out2 = layer_norm(h2)

            nc.sync.dma_start(out=out_d, in_=out2)

    return nc


def _shift2() -> np.ndarray:
    s = np.zeros((128, 128), np.float32)
    s[0, 127] = 1.0
    return s


def prep_in_maps(inputs) -> list[dict]:
    x = np.asarray(inputs["x"], np.float32)
    pf = np.asarray(inputs["pair_feats"], np.float32)
    W_att = np.asarray(inputs["W_att"], np.float32)
    b_att = np.asarray(inputs["b_att"], np.float32)
    W_obj = np.asarray(inputs["W_obj"], np.float32)
    b_obj = np.asarray(inputs["b_obj"], np.float32)
    W_pair = np.asarray(inputs["W_pair"], np.float32)
    b_pair = np.asarray(inputs["b_pair"], np.float32)
    ln_g = np.asarray(inputs["ln_g"], np.float32)
    ln_b = np.asarray(inputs["ln_b"], np.float32)
    W1 = np.asarray(inputs["W1"], np.float32)
    b1 = np.asarray(inputs["b1"], np.float32)
    W2 = np.asarray(inputs["W2"], np.float32)
    b2 = np.asarray(inputs["b2"], np.float32)

    wa, wb, wc = W_att[0, :D], W_att[0, D:2 * D], W_att[0, 2 * D:]
    xpad = np.concatenate([x, np.zeros((1, D), np.float32)], axis=0)

    # fold wc into pf columns; recover U via pre-divided W_pair.T rows.
    # Floor |scale| at fp16 min-normal so the recovery never divides by ~0;
    # the sc error from flooring is <= 6e-5 * |pf| (negligible in a sigmoid).
    colscale = np.sign(wc) * np.maximum(np.abs(wc), 6e-5)
    colscale[colscale == 0] = 6e-5
    WpT2 = (W_pair.T / colscale[:, None]).astype(np.float32)

    base = dict(
        xf=xpad.astype(np.float16),
        ident=np.eye(128, dtype=np.float32),
        ones=np.ones((128, 128), np.float32),
        shift1=np.eye(128, k=-1, dtype=np.float32),
        shift2=_shift2(),
        ones16=np.ones((128, 8), np.float16),
        zeros=np.zeros((8, 8), np.float32),
        wa=wa.astype(np.float16),
        wb=wb.astype(np.float16),
        b_att=b_att.astype(np.float32),
        bop=(b_obj + b_pair).astype(np.float32),
        b1=b1.astype(np.float32),
        b2=b2.astype(np.float32),
        lng=ln_g.astype(np.float32),
        lnb=ln_b.astype(np.float32),
        WpT=np.ascontiguousarray(WpT2),
        WoT=np.ascontiguousarray(W_obj.T).astype(np.float32),
        W1T=np.ascontiguousarray(W1.T).astype(np.float32),
        W2T=np.ascontiguousarray(W2.T).astype(np.float32),
    )

    pfr = pf.reshape(K, T, PD)
    tgrid = np.arange(128)[:, None] + 128 * np.arange(NCH)[None, :]   # [128, NCH]
    vm = np.full((128, NCH), 1.0 / T, np.float32)
    vm[127, NCH - 1] = 0.0

    in_maps = []
    for core in range(NCORES):
        ig = np.arange(core * IPC, (core + 1) * IPC)
        mlt = (tgrid[:, :, None] < ig[None, None, :]).astype(np.float32)
        mge = ((tgrid[:, :, None] >= ig[None, None, :])
               & (tgrid[:, :, None] <= T - 1)).astype(np.float16)
        shard = pfr[ig].reshape(-1, PD) * colscale[None, :]
        shard = np.concatenate([shard, np.zeros((1, PD), np.float32)], axis=0)
        xi = x[ig]
        m = dict(base)
        m.update(
            pf=shard.astype(np.float16),
            xi=xi.astype(np.float32),
            xi16=xi.astype(np.float16),
            mask_lt=mlt,
            mask_ge=mge,
            vmask=vm,
        )
        in_maps.append(m)
    return in_maps


_COMPILED = None


def _get_program() -> bacc.Bacc:
    global _COMPILED
    if _COMPILED is None:
        nc = build_program()
        nc.compile()
        _COMPILED = nc
    return _COMPILED


TRACE = False
LAST_RESULT = None


def _install_axon_ntff_hook():
    """The container's antenv lacks axon_hooks; recreate it from trn_boot's
    ctypes implementation so trace=True can capture NTFF profiles."""
    import sys
    import types
    try:
        from antenv.axon_hooks import get_axon_ntff_profile_hook  # noqa: F401
        return
    except ImportError:
        pass
    from trn_agent_boot.trn_boot import _ntff_profile_via_ctypes
    hook = _ntff_profile_via_ctypes("/opt/axon/libaxon_pjrt.so")
    m = types.ModuleType("antenv.axon_hooks")
    m.get_axon_ntff_profile_hook = lambda: hook
    sys.modules["antenv.axon_hooks"] = m


def kernel(**inputs) -> np.ndarray:
    import concourse.bass_utils as bu
    from concourse.bass_utils import run_bass_kernel_spmd
    global LAST_RESULT
    if TRACE:
        _install_axon_ntff_hook()
        bu.upload_artifacts = lambda tmpdir: str(tmpdir)  # no bucket here
    nc = _get_program()
    in_maps = prep_in_maps(inputs)
    res = run_bass_kernel_spmd(nc, in_maps, list(range(NCORES)), trace=TRACE)
    LAST_RESULT = res
    outs = [res.results[c]["out"] for c in range(NCORES)]
    return np.concatenate(outs, axis=0).astype(np.float32)


# revision 33
# speedup vs baseline: 1.0504x; 1.0504x over previous
"""Trainium2 Bass kernel for nn_ARTLayer (gnn_message_passing).

Math (reference):
    j(i,t) = t + (t>=i)                                    # [K, K-1] neighbor index
    alpha  = sigmoid(x@wa [i] + x@wb [j] + pf@wc + b_att)  # [K, K-1]
    msgs   = mean_t alpha * ((x@WobjT + b_obj)[j] + pf@WpairT + b_pair)
    out    = LN(x + msgs); out = LN(out + FFN(out))

Key algebraic rewrite (removes the 34-GFLOP [P,PD]x[PD,H] einsum):
    sum_t a*(pf@WpT)  = (sum_t a*pf) @ WpT              -> U[i,:] @ WpT
    sum_t a*oj[j]     = (A @ x) @ WobjT                  -> Gx[i,:] @ WoT
      with A[i,j] decomposed via lo/hi shifted views of x and a t>=i mask
    sum_t a*(b_obj+b_pair) = s_alpha[i] * bop

Sharding: rows i split across 8 cores (64 each); small tensors replicated;
host concatenates the per-core [64, 512] outputs.

Per-edge work (pf pass) runs in fp16 (DVE 2x mode + half DMA traffic);
sigmoid argument assembly, PSUM accumulation and the tail are fp32.

v2 optimizations (from NTFF profile of v1 @148us):
  - wc is folded into pf on the host (pf_sent[:,j] = pf[:,j]*scale_j with
    scale_j = sign(wc_j)*max(|wc_j|, 6e-5)); sc becomes a pure add-reduce and
    U is recovered exactly via W_pairT rows pre-divided by scale_j.
  - the sc reduce runs as an fp16 halving tree (2x DVE mode) instead of a
    1x-mode tensor_reduce with fp32 output.
  - U accumulation is alpha-stationary: lhsT = alpha column (1-wide weight
    load), rhs = pf tile streaming at N=128; U rows land packed in 4 PSUM
    banks at 32-aligned partition slots and are reassembled via a small
    DRAM bounce + one PE transpose.
"""
import numpy as np

import concourse.bass as bass
import concourse.tile as tile
from concourse import bacc, mybir

F32, F16 = mybir.dt.float32, mybir.dt.float16
AX = mybir.AxisListType
OP = mybir.AluOpType
AF = mybir.ActivationFunctionType

K, D, H, PD = 512, 512, 512, 128
T = K - 1                      # 511 neighbors per row
NCORES, IPC = 8, 64            # rows per core
NCH = 4                        # t-chunks of 128 (last chunk row 127 is t=511 pad)
IB, NIB = 32, 2                # i-block within a core
EPS = 1e-5


def build_program() -> bacc.Bacc:
    nc = bacc.Bacc("TRN2", target_bir_lowering=False, debug=False)

    def inp(name, shape, dt):
        return nc.dram_tensor(name, shape, dt, kind="ExternalInput").ap()

    pf = inp("pf", [IPC * T + 1, PD], F16)      # per-core edge shard, +1 zero pad row
    xf = inp("xf", [K + 1, D], F16)             # x with one zero pad row
    dxf = inp("dxf", [K, D], F16)               # x[t+1] - x[t], host computed
    xi = inp("xi", [IPC, D], F32)               # this core's rows of x
    xi16 = inp("xi16", [IPC, D], F16)
    mask_lt = inp("mask_lt", [128, NCH, IPC], F32)   # t <  i_global
    mask_ge = inp("mask_ge", [128, NCH, IPC], F16)   # t >= i_global and t <= 510
    ident = inp("ident", [128, 128], F32)
    ones = inp("ones", [128, 128], F32)
    shift1 = inp("shift1", [128, 128], F32)     # [q,p] = (q == p+1)
    shift2 = inp("shift2", [128, 128], F32)     # [q,p] = (q==0)&(p==127)
    ones16 = inp("ones16", [128, 8], F16)
    poison = inp("poison", [1, 128], F32)       # -1e9 at slot 127, else 0
    zeros = inp("zeros", [8, 8], F32)
    wa = inp("wa", [D], F16)
    wb = inp("wb", [D], F16)
    b_att = inp("b_att", [1], F32)
    bop = inp("bop", [H], F32)                  # b_obj + b_pair
    b1 = inp("b1", [H], F32)
    b2 = inp("b2", [H], F32)
    lng = inp("lng", [H], F32)
    lnb = inp("lnb", [H], F32)
    WpT = inp("WpT", [PD, H], F32)              # W_pair.T
    WoT = inp("WoT", [D, H], F32)               # W_obj.T
    W1T = inp("W1T", [H, H], F32)
    W2T = inp("W2T", [H, H], F32)

    out_d = nc.dram_tensor("out", [IPC, H], F32, kind="ExternalOutput").ap()
    u_dram = nc.dram_tensor("u_dram", [4 * 128 * 512], F32).ap()  # U bank dumps

    with tile.TileContext(nc) as tc:
        with (
            tc.tile_pool(name="const", bufs=1) as cpool,
            tc.tile_pool(name="pfp", bufs=3) as pfp,
            tc.tile_pool(name="scrp", bufs=2) as scrp,
            tc.tile_pool(name="smallp", bufs=4) as smallp,
            tc.tile_pool(name="postp", bufs=2) as postp,
            tc.tile_pool(name="pss", bufs=2, space="PSUM") as pss,
            tc.tile_pool(name="psflex", bufs=4, space="PSUM") as psflex,
            tc.tile_pool(name="psmp", bufs=1, space="PSUM") as psmp,
            tc.tile_pool(name="psgp", bufs=1, space="PSUM") as psgp,
        ):
            # ---- constants & weights to SBUF ----
            wa_b = cpool.tile([IPC, D], F16)
            nc.sync.dma_start(out=wa_b, in_=wa[None, :].to_broadcast([IPC, D]))
            wb_b = cpool.tile([128, D], F16)
            nc.sync.dma_start(out=wb_b, in_=wb[None, :].to_broadcast([128, D]))
            id_sb = cpool.tile([128, 128], F32)
            nc.sync.dma_start(out=id_sb, in_=ident)
            ones_sb = cpool.tile([128, 128], F32)
            nc.sync.dma_start(out=ones_sb, in_=ones)
            sh1_sb = cpool.tile([128, 128], F32)
            nc.sync.dma_start(out=sh1_sb, in_=shift1)
            sh2_sb = cpool.tile([128, 128], F32)
            nc.sync.dma_start(out=sh2_sb, in_=shift2)
            ones16_sb = cpool.tile([128, 8], F16)
            nc.sync.dma_start(out=ones16_sb, in_=ones16)
            poison_sb = cpool.tile([1, 128], F32)
            nc.sync.dma_start(out=poison_sb, in_=poison)
            mlt_sb = cpool.tile([128, NCH, IPC], F32)
            nc.sync.dma_start(out=mlt_sb, in_=mask_lt)
            mge_sb = cpool.tile([128, NCH, IPC], F16)
            nc.sync.dma_start(out=mge_sb, in_=mask_ge)
            WpT_sb = cpool.tile([128, H], F32)
            nc.sync.dma_start(out=WpT_sb, in_=WpT)
            WoT_sb = cpool.tile([128, NCH, H], F32)
            nc.sync.dma_start(out=WoT_sb, in_=WoT.rearrange("(c p) h -> p c h", p=128))
            W1T_sb = cpool.tile([128, NCH, H], F32)
            nc.sync.dma_start(out=W1T_sb, in_=W1T.rearrange("(c p) h -> p c h", p=128))
            W2T_sb = cpool.tile([128, NCH, H], F32)
            nc.sync.dma_start(out=W2T_sb, in_=W2T.rearrange("(c p) h -> p c h", p=128))
            gb_sb = cpool.tile([IPC, H], F32)
            nc.sync.dma_start(out=gb_sb, in_=lng[None, :].to_broadcast([IPC, H]))
            bb_sb = cpool.tile([IPC, H], F32)
            nc.sync.dma_start(out=bb_sb, in_=lnb[None, :].to_broadcast([IPC, H]))
            b1_sb = cpool.tile([IPC, H], F32)
            nc.sync.dma_start(out=b1_sb, in_=b1[None, :].to_broadcast([IPC, H]))
            b2_sb = cpool.tile([IPC, H], F32)
            nc.sync.dma_start(out=b2_sb, in_=b2[None, :].to_broadcast([IPC, H]))
            bop_row = cpool.tile([1, H], F32)
            nc.sync.dma_start(out=bop_row, in_=bop[None, :])
            xi_sb = cpool.tile([IPC, D], F32)
            nc.sync.dma_start(out=xi_sb, in_=xi)
            xi16_sb = cpool.tile([IPC, D], F16)
            nc.sync.dma_start(out=xi16_sb, in_=xi16)
            xlo = cpool.tile([128, NCH, D], F16)
            nc.sync.dma_start(out=xlo, in_=xf[0:K].rearrange("(c p) d -> p c d", p=128))
            dx = cpool.tile([128, NCH, D], F16)
            nc.sync.dma_start(out=dx, in_=dxf.rearrange("(c p) d -> p c d", p=128))

            eps_col = cpool.tile([IPC, 1], F32)
            nc.vector.memset(eps_col, EPS)

            # ---- sa (this core's rows) and sb (all rows) ----
            scr_sa = smallp.tile([IPC, D], F16)
            nc.vector.tensor_mul(scr_sa, xi16_sb, wa_b)
            sa_col = smallp.tile([IPC, 1], F32)
            nc.vector.tensor_reduce(sa_col, scr_sa, axis=AX.X, op=OP.add)
            b_att_col = smallp.tile([IPC, 1], F32)
            nc.sync.dma_start(out=b_att_col, in_=b_att[None, :].to_broadcast([IPC, 1]))
            nc.vector.tensor_add(sa_col, sa_col, b_att_col)
            sa_diag = smallp.tile([IPC, IPC], F32)
            nc.vector.tensor_mul(sa_diag, id_sb[0:IPC, 0:IPC], sa_col.to_broadcast([IPC, IPC]))

            sb_cols = smallp.tile([128, NCH], F32)
            for c in range(NCH):
                scr_sb = smallp.tile([128, D], F16)
                nc.vector.tensor_mul(scr_sb, xlo[:, c, :], wb_b)
                nc.vector.tensor_reduce(
                    sb_cols[:, c:c + 1], scr_sb, axis=AX.X, op=OP.add)

            # sb_hi[p, c] = sb[c*128+p+1] via shift matmuls (no DRAM bounce);
            # slot t=511 -> sb[512] stays 0, as required.
            sbhi_ps = pss.tile([128, NCH], F32, tag="ps_small")
            nc.tensor.matmul(sbhi_ps, sh1_sb, sb_cols, start=True, stop=False)
            nc.tensor.matmul(sbhi_ps[:, 0:NCH - 1], sh2_sb, sb_cols[:, 1:NCH],
                             start=False, stop=True)
            sbhi_cols = smallp.tile([128, NCH], F32)
            nc.vector.tensor_copy(sbhi_cols, sbhi_ps)

            # ---- SBJ[t, i] = sa[i] + b_att + sb_hi[t] + mask_lt*(sb_lo - sb_hi) ----
            sbj = cpool.tile([128, NCH, IPC], F32)
            for c in range(NCH):
                diffc = smallp.tile([128, 1], F32)
                nc.vector.tensor_tensor(
                    diffc, sb_cols[:, c:c + 1], sbhi_cols[:, c:c + 1], OP.subtract)
                diagc = smallp.tile([128, 128], F32)
                nc.vector.tensor_mul(diagc, id_sb, diffc.to_broadcast([128, 128]))
                diagb = smallp.tile([128, 128], F32)
                nc.vector.tensor_mul(
                    diagb, id_sb, sbhi_cols[:, c:c + 1].to_broadcast([128, 128]))
                ps_sbj = pss.tile([128, IPC], F32, tag="ps_small")
                nc.tensor.matmul(ps_sbj, ones_sb[0:IPC, :], sa_diag, start=True, stop=False)
                nc.tensor.matmul(ps_sbj, diagb, ones_sb[:, 0:IPC], start=False, stop=False)
                if c == NCH - 1:
                    # poison the t=511 pad slot: sigmoid(-1e9) = 0 exactly, so
                    # the pad row drops out of U/G/s without a mask multiply
                    nc.tensor.matmul(ps_sbj, poison_sb, ones_sb[0:1, 0:IPC],
                                     start=False, stop=False)
                nc.tensor.matmul(ps_sbj, diagc, mlt_sb[:, c, :], start=False, stop=True)
                nc.vector.tensor_copy(sbj[:, c, :], ps_sbj)

            # ---- main edge pass ----
            alpha_full = cpool.tile([128, NCH, IPC], F16)   # alpha / 511
            age_full = cpool.tile([128, NCH, IPC], F16)     # masked (t>=i) alpha / 511
            gx_ps = psgp.tile([IPC, D], F32)                # Gx[i, d] = sum_t a*x[j]
            msg_ps = psmp.tile([IPC, H], F32)
            # U rows: 16 i per PSUM bank, at 32-aligned partition slots x 4
            # free offsets of 128. u_ps[b] row (s, f) holds U[i] for
            # i = 16 b + 4 (s/32) + f.
            u_ps = [psflex.tile([128, 512], F32, tag="flex", name=f"u_ps{b}")
                    for b in range(4)]
            for b in range(4):
                # init unused partitions so the post-loop full-bank copy is clean
                nc.vector.memset(u_ps[b], 0.0)

            for c in range(NCH):
                for ib in range(NIB):
                    i0 = ib * IB
                    pf_t = pfp.tile([128, IB, PD], F16)
                    for h in range(2):
                        src = bass.AP(
                            tensor=pf.tensor,
                            offset=((i0 + 16 * h) * T + c * 128) * PD,
                            ap=[[PD, 128], [T * PD, 16], [1, PD]],
                        )
                        nc.sync.dma_start(out=pf_t[:, 16 * h:16 * (h + 1), :], in_=src)
                    # sc = sum_pd pf_sent (wc pre-folded): fp16 halving tree
                    scr = scrp.tile([128, IB, 64], F16)
                    nc.vector.tensor_add(scr, pf_t[:, :, 0:64], pf_t[:, :, 64:128])
                    w = 32
                    while w >= 2:
                        nc.vector.tensor_add(
                            scr[:, :, 0:w], scr[:, :, 0:w], scr[:, :, w:2 * w])
                        w //= 2
                    sc_t = smallp.tile([128, IB], F32)
                    nc.vector.tensor_add(sc_t, scr[:, :, 0], scr[:, :, 1])
                    aarg = smallp.tile([128, IB], F32)
                    nc.vector.tensor_add(aarg, sc_t, sbj[:, c, i0:i0 + IB])
                    # alpha (unscaled; /511 is folded into WpT/WoT/bop on host)
                    nc.scalar.activation(
                        alpha_full[:, c, i0:i0 + IB], aarg, AF.Sigmoid)
                    nc.vector.tensor_mul(
                        age_full[:, c, i0:i0 + IB], alpha_full[:, c, i0:i0 + IB],
                        mge_sb[:, c, i0:i0 + IB])
                    # U quads: lhsT = 4 alpha columns, rhs = 4 pf blocks, the
                    # wanted U rows sit on the diagonal (extracted via DRAM AP)
                    for q in range(IB // 4):
                        i = i0 + 4 * q
                        b, sp = divmod(i // 4, 4)
                        nc.tensor.matmul(
                            u_ps[b][32 * sp:32 * sp + 4, :],
                            alpha_full[:, c, i:i + 4],
                            pf_t[:, 4 * q:4 * q + 4, :],
                            start=(c == 0), stop=(c == NCH - 1),
                            tile_position=(0, 32 * sp))
                nc.tensor.matmul(gx_ps, alpha_full[:, c, :], xlo[:, c, :],
                                 start=(c == 0), stop=False)
                nc.tensor.matmul(gx_ps, age_full[:, c, :], dx[:, c, :],
                                 start=False, stop=(c == NCH - 1))

            # ---- messages = U@WpT + Gx@WoT + s_alpha x bop  (all alpha-scaled 1/511) ----
            s_ps = pss.tile([IPC, 1], F32, tag="ps_small")
            for c in range(NCH):
                nc.tensor.matmul(s_ps, alpha_full[:, c, :], ones16_sb[:, 0:1],
                                 start=(c == 0), stop=(c == NCH - 1))
            s_col = smallp.tile([IPC, 1], F32)
            nc.vector.tensor_copy(s_col, s_ps)
            ps_sr = pss.tile([1, IPC], F32, tag="ps_small")
            nc.tensor.transpose(ps_sr, s_col, id_sb[0:IPC, 0:IPC])
            s_row = smallp.tile([1, IPC], F32)
            nc.vector.tensor_copy(s_row, ps_sr)

            # U reassembly: full-bank psum->sbuf copies, DRAM dump, then one
            # gather DMA picks row (b, s, f) -> i = 16b + 4(s/32) + f.
            for b in range(4):
                u_cp = postp.tile([128, 512], F32, tag="u_cp")
                nc.vector.tensor_copy(u_cp, u_ps[b])
                nc.sync.dma_start(
                    out=u_dram[b * 65536:(b + 1) * 65536].rearrange(
                        "(p f) -> p f", p=128),
                    in_=u_cp)
            u_sb64 = postp.tile([IPC, PD], F32)
            gather = bass.AP(tensor=u_dram.tensor, offset=0,
                             ap=[[65536, 4], [16384, 4], [640, 4], [1, 128]])
            nc.sync.dma_start(out=u_sb64, in_=gather)
            ps_ut = pss.tile([128, IPC], F32, tag="ps_small")
            nc.tensor.transpose(ps_ut, u_sb64, id_sb[0:IPC, 0:IPC])
            u_sb = postp.tile([128, IPC], F32)
            nc.vector.tensor_copy(u_sb, ps_ut)
            gx_sb = postp.tile([IPC, D], F32)
            nc.vector.tensor_copy(gx_sb, gx_ps)
            gxT = postp.tile([128, NCH, IPC], F32)
            for c in range(NCH):
                ptg = pss.tile([128, IPC], F32, tag="ps_small")
                nc.tensor.transpose(ptg, gx_sb[:, c * 128:(c + 1) * 128], id_sb[0:IPC, 0:IPC])
                nc.vector.tensor_copy(gxT[:, c, :], ptg)

            nc.tensor.matmul(msg_ps, u_sb, WpT_sb, start=True, stop=False)
            for c in range(NCH):
                nc.tensor.matmul(msg_ps, gxT[:, c, :], WoT_sb[:, c, :],
                                 start=False, stop=False)
            nc.tensor.matmul(msg_ps, s_row, bop_row, start=False, stop=True)

            # ---- residual + LN1 ----
            def layer_norm(v):
                stats = smallp.tile([IPC, 6], F32)
                nc.vector.bn_stats(out=stats, in_=v)
                mv = smallp.tile([IPC, 2], F32)
                nc.vector.bn_aggr(out=mv, in_=stats)
                std = smallp.tile([IPC, 1], F32)
                nc.scalar.activation(std, mv[:, 1:2], AF.Sqrt, bias=eps_col)
                rstd = smallp.tile([IPC, 1], F32)
                nc.vector.reciprocal(rstd, std)
                cen = postp.tile([IPC, H], F32)
                nc.vector.tensor_scalar(cen, v, mv[:, 0:1], rstd, OP.subtract, OP.mult)
                o = postp.tile([IPC, H], F32)
                nc.vector.tensor_mul(o, cen, gb_sb)
                nc.vector.tensor_add(o, o, bb_sb)
                return o

            h_sb = postp.tile([IPC, H], F32)
            nc.vector.tensor_add(h_sb, xi_sb, msg_ps)
            out1 = layer_norm(h_sb)

            # ---- FFN ----
            def transpose_rows(v):
                vT = postp.tile([128, NCH, IPC], F32, tag="vT")
                for c in range(NCH):
                    ptt = pss.tile([128, IPC], F32, tag="ps_small")
                    nc.tensor.transpose(ptt, v[:, c * 128:(c + 1) * 128], id_sb[0:IPC, 0:IPC])
                    nc.vector.tensor_copy(vT[:, c, :], ptt)
                return vT

            o1T = transpose_rows(out1)
            f1_ps = psflex.tile([IPC, H], F32, tag="flex")
            for c in range(NCH):
                nc.tensor.matmul(f1_ps, o1T[:, c, :], W1T_sb[:, c, :],
                                 start=(c == 0), stop=(c == NCH - 1))
            f1 = postp.tile([IPC, H], F32)
            nc.vector.tensor_add(f1, f1_ps, b1_sb)
            nc.vector.tensor_scalar_max(f1, f1, 0.0)

            f1T = transpose_rows(f1)
            f2_ps = psflex.tile([IPC, H], F32, tag="flex")
            for c in range(NCH):
                nc.tensor.matmul(f2_ps, f1T[:, c, :], W2T_sb[:, c, :],
                                 start=(c == 0), stop=(c == NCH - 1))
            h2 = postp.tile([IPC, H], F32)
            nc.vector.tensor_add(h2, f2_ps, b2_sb)
            nc.vector.tensor_add(h2, h2, out1)
            out2 = layer_norm(h2)

            nc.sync.dma_start(out=out_d, in_=out2)

    return nc


def _poison() -> np.ndarray:
    p = np.zeros((1, 128), np.float32)
    p[0, 127] = -1e9
    return p


def _shift2() -> np.ndarray:
    s = np.zeros((128, 128), np.float32)
    s[0, 127] = 1.0
    return s


def prep_in_maps(inputs) -> list[dict]:
    x = np.asarray(inputs["x"], np.float32)
    pf = np.asarray(inputs["pair_feats"], np.float32)
    W_att = np.asarray(inputs["W_att"], np.float32)
    b_att = np.asarray(inputs["b_att"], np.float32)
    W_obj = np.asarray(inputs["W_obj"], np.float32)
    b_obj = np.asarray(inputs["b_obj"], np.float32)
    W_pair = np.asarray(inputs["W_pair"], np.float32)
    b_pair = np.asarray(inputs["b_pair"], np.float32)
    ln_g = np.asarray(inputs["ln_g"], np.float32)
    ln_b = np.asarray(inputs["ln_b"], np.float32)
    W1 = np.asarray(inputs["W1"], np.float32)
    b1 = np.asarray(inputs["b1"], np.float32)
    W2 = np.asarray(inputs["W2"], np.float32)
    b2 = np.asarray(inputs["b2"], np.float32)

    wa, wb, wc = W_att[0, :D], W_att[0, D:2 * D], W_att[0, 2 * D:]
    xpad = np.concatenate([x, np.zeros((1, D), np.float32)], axis=0)

    # fold wc into pf columns; recover U via pre-divided W_pair.T rows.
    # Floor |scale| at fp16 min-normal so the recovery never divides by ~0;
    # the sc error from flooring is <= 6e-5 * |pf| (negligible in a sigmoid).
    colscale = np.sign(wc) * np.maximum(np.abs(wc), 6e-5)
    colscale[colscale == 0] = 6e-5
    # 1/511 (the mean over neighbors) is folded into the three weight paths
    # that consume raw alpha: U@WpT, (A@x)@WoT, and s_alpha*bop.
    WpT2 = (W_pair.T / colscale[:, None] / T).astype(np.float32)
    WoT2 = (W_obj.T / T).astype(np.float32)
    dxf = np.diff(xpad[:K + 1], axis=0)

    base = dict(
        xf=xpad.astype(np.float16),
        dxf=dxf.astype(np.float16),
        ident=np.eye(128, dtype=np.float32),
        ones=np.ones((128, 128), np.float32),
        shift1=np.eye(128, k=-1, dtype=np.float32),
        shift2=_shift2(),
        ones16=np.ones((128, 8), np.float16),
        poison=_poison(),
        zeros=np.zeros((8, 8), np.float32),
        wa=wa.astype(np.float16),
        wb=wb.astype(np.float16),
        b_att=b_att.astype(np.float32),
        bop=((b_obj + b_pair) / T).astype(np.float32),
        b1=b1.astype(np.float32),
        b2=b2.astype(np.float32),
        lng=ln_g.astype(np.float32),
        lnb=ln_b.astype(np.float32),
        WpT=np.ascontiguousarray(WpT2),
        WoT=np.ascontiguousarray(WoT2),
        W1T=np.ascontiguousarray(W1.T).astype(np.float32),
        W2T=np.ascontiguousarray(W2.T).astype(np.float32),
    )

    pfr = pf.reshape(K, T, PD)
    tgrid = np.arange(128)[:, None] + 128 * np.arange(NCH)[None, :]   # [128, NCH]

    in_maps = []
    for core in range(NCORES):
        ig = np.arange(core * IPC, (core + 1) * IPC)
        mlt = (tgrid[:, :, None] < ig[None, None, :]).astype(np.float32)
        mge = ((tgrid[:, :, None] >= ig[None, None, :])
               & (tgrid[:, :, None] <= T - 1)).astype(np.float16)
        shard = pfr[ig].reshape(-1, PD) * colscale[None, :]
        shard = np.concatenate([shard, np.zeros((1, PD), np.float32)], axis=0)
        xi = x[ig]
        m = dict(base)
        m.update(
            pf=shard.astype(np.float16),
            xi=xi.astype(np.float32),
            xi16=xi.astype(np.float16),
            mask_lt=mlt,
            mask_ge=mge,
        )
        in_maps.append(m)
    return in_maps


_COMPILED = None


def _get_program() -> bacc.Bacc:
    global _COMPILED
    if _COMPILED is None:
        nc = build_program()
        nc.compile()
        _COMPILED = nc
    return _COMPILED


TRACE = False
LAST_RESULT = None


def _install_axon_ntff_hook():
    """The container's antenv lacks axon_hooks; recreate it from trn_boot's
    ctypes implementation so trace=True can capture NTFF profiles."""
    import sys
    import types
    try:
        from antenv.axon_hooks import get_axon_ntff_profile_hook  # noqa: F401
        return
    except ImportError:
        pass
    from trn_agent_boot.trn_boot import _ntff_profile_via_ctypes
    hook = _ntff_profile_via_ctypes("/opt/axon/libaxon_pjrt.so")
    m = types.ModuleType("antenv.axon_hooks")
    m.get_axon_ntff_profile_hook = lambda: hook
    sys.modules["antenv.axon_hooks"] = m


def kernel(**inputs) -> np.ndarray:
    import concourse.bass_utils as bu
    from concourse.bass_utils import run_bass_kernel_spmd
    global LAST_RESULT
    if TRACE:
        _install_axon_ntff_hook()
        bu.upload_artifacts = lambda tmpdir: str(tmpdir)  # no bucket here
    nc = _get_program()
    in_maps = prep_in_maps(inputs)
    res = run_bass_kernel_spmd(nc, in_maps, list(range(NCORES)), trace=TRACE)
    LAST_RESULT = res
    outs = [res.results[c]["out"] for c in range(NCORES)]
    return np.concatenate(outs, axis=0).astype(np.float32)


# revision 34
# speedup vs baseline: 1.0689x; 1.0175x over previous
"""Trainium2 Bass kernel for nn_ARTLayer (gnn_message_passing).

Math (reference):
    j(i,t) = t + (t>=i)                                    # [K, K-1] neighbor index
    alpha  = sigmoid(x@wa [i] + x@wb [j] + pf@wc + b_att)  # [K, K-1]
    msgs   = mean_t alpha * ((x@WobjT + b_obj)[j] + pf@WpairT + b_pair)
    out    = LN(x + msgs); out = LN(out + FFN(out))

Key algebraic rewrite (removes the 34-GFLOP [P,PD]x[PD,H] einsum):
    sum_t a*(pf@WpT)  = (sum_t a*pf) @ WpT               -> U[i,:] @ WpT
    sum_t a*oj[j]     = (A @ x) @ WobjT                  -> Gx[i,:] @ WoT
      with A[i,j] decomposed via lo/hi shifted views of x and a t>=i mask
    sum_t a*(b_obj+b_pair) = s_alpha[i] * bop

Sharding: rows i split across 8 cores (64 each); small tensors replicated;
host concatenates the per-core [64, 512] outputs.

Implementation notes (driven by NTFF profiles):
  - wc is folded into pf on the host (column scales, floored at fp16
    min-normal); sc becomes a pure fp16 halving-tree add-reduce and U is
    recovered exactly via W_pairT rows pre-divided by the scales.
  - 1/511 (the neighbor mean) is folded into WpT/WoT/bop on the host, and
    the t=511 pad slot is poisoned with -1e9 pre-sigmoid, so raw sigmoid
    output is used directly with no mask/scale multiplies.
  - U accumulation runs as M=4 quad matmuls (alpha quad stationary, four pf
    blocks streaming at N=512); the wanted rows sit on the block diagonal
    and are gathered by a stride-640 DRAM access pattern after a bank dump.
  - pf is re-laid-out on the host to [chunk, t, i, pd] so each tile DMA is
    one fully-contiguous 8KB-per-partition burst, and all DMAs are spread
    round-robin over the three DMA-capable engines (sync/scalar HWDGE,
    gpsimd SWDGE) instead of serializing on one queue.
"""
import numpy as np

import concourse.bass as bass
import concourse.tile as tile
from concourse import bacc, mybir

F32, F16 = mybir.dt.float32, mybir.dt.float16
AX = mybir.AxisListType
OP = mybir.AluOpType
AF = mybir.ActivationFunctionType

K, D, H, PD = 512, 512, 512, 128
T = K - 1                      # 511 neighbors per row
NCORES, IPC = 8, 64            # rows per core
NCH = 4                        # t-chunks of 128 (last chunk row 127 is t=511 pad)
IB, NIB = 32, 2                # i-block within a core
EPS = 1e-5


def build_program() -> bacc.Bacc:
    nc = bacc.Bacc("TRN2", target_bir_lowering=False, debug=False)

    def inp(name, shape, dt):
        return nc.dram_tensor(name, shape, dt, kind="ExternalInput").ap()

    pf = inp("pf", [NCH, 128, IPC, PD], F16)    # [chunk, t-in-chunk, i, pd]
    xf = inp("xf", [K + 1, D], F16)             # x with one zero pad row
    dxf = inp("dxf", [K, D], F16)               # x[t+1] - x[t], host computed
    xi = inp("xi", [IPC, D], F32)               # this core's rows of x
    xi16 = inp("xi16", [IPC, D], F16)
    mask_lt = inp("mask_lt", [128, NCH, IPC], F32)   # t <  i_global
    mask_ge = inp("mask_ge", [128, NCH, IPC], F16)   # t >= i_global and t <= 510
    cmat = inp("cmat", [128, 4, 128], F32)      # [ident | ones | shift1 | shift2]
    ones16 = inp("ones16", [128, 8], F16)
    poison = inp("poison", [1, 128], F32)       # -1e9 at slot 127, else 0
    wab = inp("wab", [2, D], F16)               # [wa; wb]
    b_att = inp("b_att", [1], F32)
    bias5 = inp("bias5", [5, H], F32)           # [ln_g; ln_b; b1; b2; bop/511]
    WpT = inp("WpT", [PD, H], F16)              # W_pair.T / colscale / 511
    WoT = inp("WoT", [D, H], F16)               # W_obj.T / 511
    W1T = inp("W1T", [H, H], F16)
    W2T = inp("W2T", [H, H], F16)

    out_d = nc.dram_tensor("out", [IPC, H], F32, kind="ExternalOutput").ap()
    u_dram = nc.dram_tensor("u_dram", [4 * 128 * 512], F32).ap()  # U bank dumps

    with tile.TileContext(nc) as tc:
        with (
            tc.tile_pool(name="const", bufs=1) as cpool,
            tc.tile_pool(name="pfp", bufs=3) as pfp,
            tc.tile_pool(name="scrp", bufs=2) as scrp,
            tc.tile_pool(name="smallp", bufs=4) as smallp,
            tc.tile_pool(name="postp", bufs=2) as postp,
            tc.tile_pool(name="pss", bufs=2, space="PSUM") as pss,
            tc.tile_pool(name="psflex", bufs=4, space="PSUM") as psflex,
            tc.tile_pool(name="psmp", bufs=1, space="PSUM") as psmp,
            tc.tile_pool(name="psgp", bufs=1, space="PSUM") as psgp,
        ):
            # round-robin DMA issue over the three DMA-capable engines
            dmaq = [nc.sync, nc.scalar, nc.gpsimd]
            qi = [0]

            def dma(out, in_):
                eng = dmaq[qi[0] % len(dmaq)]
                qi[0] += 1
                eng.dma_start(out=out, in_=in_)

            # ---- constants & weights to SBUF ----
            cm_sb = cpool.tile([128, 4, 128], F32)
            dma(cm_sb, cmat)
            id_sb = cm_sb[:, 0, :]
            ones_sb = cm_sb[:, 1, :]
            sh1_sb = cm_sb[:, 2, :]
            sh2_sb = cm_sb[:, 3, :]
            wab_sb = cpool.tile([128, 2, D], F16)
            dma(wab_sb, wab[None, :, :].to_broadcast([128, 2, D]))
            wa_b = wab_sb[0:IPC, 0, :]
            wb_b = wab_sb[:, 1, :]
            bias_sb = cpool.tile([IPC, 5, H], F32)
            dma(bias_sb, bias5[None, :, :].to_broadcast([IPC, 5, H]))
            gb_sb = bias_sb[:, 0, :]
            bb_sb = bias_sb[:, 1, :]
            b1_sb = bias_sb[:, 2, :]
            b2_sb = bias_sb[:, 3, :]
            bop_row = bias_sb[0:1, 4, :]
            ones16_sb = cpool.tile([128, 8], F16)
            dma(ones16_sb, ones16)
            poison_sb = cpool.tile([1, 128], F32)
            dma(poison_sb, poison)
            mlt_sb = cpool.tile([128, NCH, IPC], F32)
            dma(mlt_sb, mask_lt)
            mge_sb = cpool.tile([128, NCH, IPC], F16)
            dma(mge_sb, mask_ge)
            WpT_sb = cpool.tile([128, H], F16)
            dma(WpT_sb, WpT)
            WoT_sb = cpool.tile([128, NCH, H], F16)
            dma(WoT_sb, WoT.rearrange("(c p) h -> p c h", p=128))
            W1T_sb = cpool.tile([128, NCH, H], F16)
            dma(W1T_sb, W1T.rearrange("(c p) h -> p c h", p=128))
            W2T_sb = cpool.tile([128, NCH, H], F16)
            dma(W2T_sb, W2T.rearrange("(c p) h -> p c h", p=128))
            xi_sb = cpool.tile([IPC, D], F32)
            dma(xi_sb, xi)
            xi16_sb = cpool.tile([IPC, D], F16)
            dma(xi16_sb, xi16)
            xlo = cpool.tile([128, NCH, D], F16)
            dma(xlo, xf[0:K].rearrange("(c p) d -> p c d", p=128))
            dx = cpool.tile([128, NCH, D], F16)
            dma(dx, dxf.rearrange("(c p) d -> p c d", p=128))
            b_att_col = cpool.tile([IPC, 1], F32)
            dma(b_att_col, b_att[None, :].to_broadcast([IPC, 1]))

            eps_col = cpool.tile([IPC, 1], F32)
            nc.vector.memset(eps_col, EPS)

            # ---- sa (this core's rows) and sb (all rows) ----
            scr_sa = smallp.tile([IPC, D], F16)
            nc.vector.tensor_mul(scr_sa, xi16_sb, wa_b)
            sa_col = smallp.tile([IPC, 1], F32)
            nc.vector.tensor_reduce(sa_col, scr_sa, axis=AX.X, op=OP.add)
            nc.vector.tensor_add(sa_col, sa_col, b_att_col)
            sa_diag = smallp.tile([IPC, IPC], F32)
            nc.vector.tensor_mul(sa_diag, id_sb[0:IPC, 0:IPC],
                                 sa_col.to_broadcast([IPC, IPC]))

            sb_cols = smallp.tile([128, NCH], F32)
            for c in range(NCH):
                scr_sb = smallp.tile([128, D], F16)
                nc.vector.tensor_mul(scr_sb, xlo[:, c, :], wb_b)
                nc.vector.tensor_reduce(
                    sb_cols[:, c:c + 1], scr_sb, axis=AX.X, op=OP.add)

            # sb_hi[p, c] = sb[c*128+p+1] via shift matmuls; slot 511 stays 0
            sbhi_ps = pss.tile([128, NCH], F32, tag="ps_small")
            nc.tensor.matmul(sbhi_ps, sh1_sb, sb_cols, start=True, stop=False)
            nc.tensor.matmul(sbhi_ps[:, 0:NCH - 1], sh2_sb, sb_cols[:, 1:NCH],
                             start=False, stop=True)
            sbhi_cols = smallp.tile([128, NCH], F32)
            nc.vector.tensor_copy(sbhi_cols, sbhi_ps)

            # ---- SBJ[t, i] = sa[i] + b_att + sb_hi[t] + mask_lt*(sb_lo-sb_hi),
            #      with -1e9 poison at the t=511 pad slot ----
            sbj = cpool.tile([128, NCH, IPC], F32)
            for c in range(NCH):
                diffc = smallp.tile([128, 1], F32)
                nc.vector.tensor_tensor(
                    diffc, sb_cols[:, c:c + 1], sbhi_cols[:, c:c + 1], OP.subtract)
                diagc = smallp.tile([128, 128], F32)
                nc.vector.tensor_mul(diagc, id_sb, diffc.to_broadcast([128, 128]))
                diagb = smallp.tile([128, 128], F32)
                nc.vector.tensor_mul(
                    diagb, id_sb, sbhi_cols[:, c:c + 1].to_broadcast([128, 128]))
                ps_sbj = pss.tile([128, IPC], F32, tag="ps_small")
                nc.tensor.matmul(ps_sbj, ones_sb[0:IPC, :], sa_diag,
                                 start=True, stop=False)
                nc.tensor.matmul(ps_sbj, diagb, ones_sb[:, 0:IPC],
                                 start=False, stop=False)
                if c == NCH - 1:
                    # poison: sigmoid(-1e9) = 0 exactly, pad row drops out
                    nc.tensor.matmul(ps_sbj, poison_sb, ones_sb[0:1, 0:IPC],
                                     start=False, stop=False)
                nc.tensor.matmul(ps_sbj, diagc, mlt_sb[:, c, :],
                                 start=False, stop=True)
                nc.vector.tensor_copy(sbj[:, c, :], ps_sbj)

            # ---- main edge pass ----
            alpha_full = cpool.tile([128, NCH, IPC], F16)   # raw sigmoid out
            age_full = cpool.tile([128, NCH, IPC], F16)     # masked (t>=i) alpha
            gx_ps = psgp.tile([IPC, D], F32)                # sum_t a*x[j]
            msg_ps = psmp.tile([IPC, H], F32)
            # U quad rows: bank b, partition slot 32s..32s+3 holds i=16b+4s+j
            u_ps = [psflex.tile([128, 512], F32, tag="flex", name=f"u_ps{b}")
                    for b in range(4)]
            for b in range(4):
                nc.vector.memset(u_ps[b], 0.0)

            for c in range(NCH):
                for ib in range(NIB):
                    i0 = ib * IB
                    pf_t = pfp.tile([128, IB, PD], F16)
                    for hh in range(2):
                        dma(pf_t[:, 16 * hh:16 * (hh + 1), :],
                            pf[c, :, i0 + 16 * hh:i0 + 16 * (hh + 1), :])
                    # sc = sum_pd pf_sent (wc pre-folded): fp16 halving tree
                    scr = scrp.tile([128, IB, 64], F16)
                    nc.vector.tensor_add(scr, pf_t[:, :, 0:64], pf_t[:, :, 64:128])
                    w = 32
                    while w >= 2:
                        nc.vector.tensor_add(
                            scr[:, :, 0:w], scr[:, :, 0:w], scr[:, :, w:2 * w])
                        w //= 2
                    sc_t = smallp.tile([128, IB], F32)
                    nc.vector.tensor_add(sc_t, scr[:, :, 0], scr[:, :, 1])
                    aarg = smallp.tile([128, IB], F32)
                    nc.vector.tensor_add(aarg, sc_t, sbj[:, c, i0:i0 + IB])
                    nc.scalar.activation(
                        alpha_full[:, c, i0:i0 + IB], aarg, AF.Sigmoid)
                    nc.vector.tensor_mul(
                        age_full[:, c, i0:i0 + IB], alpha_full[:, c, i0:i0 + IB],
                        mge_sb[:, c, i0:i0 + IB])
                    # U quads: lhsT = 4 alpha columns, rhs = 4 pf blocks; the
                    # wanted rows sit on the diagonal (gathered via DRAM AP)
                    for q in range(IB // 4):
                        i = i0 + 4 * q
                        b, sp = divmod(i // 4, 4)
                        nc.tensor.matmul(
                            u_ps[b][32 * sp:32 * sp + 4, :],
                            alpha_full[:, c, i:i + 4],
                            pf_t[:, 4 * q:4 * q + 4, :],
                            start=(c == 0), stop=(c == NCH - 1),
                            tile_position=(0, 32 * sp))
                nc.tensor.matmul(gx_ps, alpha_full[:, c, :], xlo[:, c, :],
                                 start=(c == 0), stop=False)
                nc.tensor.matmul(gx_ps, age_full[:, c, :], dx[:, c, :],
                                 start=False, stop=(c == NCH - 1))

            # ---- messages = U@WpT + Gx@WoT + s_alpha x bop ----
            s_ps = pss.tile([IPC, 1], F32, tag="ps_small")
            for c in range(NCH):
                nc.tensor.matmul(s_ps, alpha_full[:, c, :], ones16_sb[:, 0:1],
                                 start=(c == 0), stop=(c == NCH - 1))
            s_col = smallp.tile([IPC, 1], F32)
            nc.vector.tensor_copy(s_col, s_ps)
            ps_sr = pss.tile([1, IPC], F32, tag="ps_small")
            nc.tensor.transpose(ps_sr, s_col, id_sb[0:IPC, 0:IPC])
            s_row = smallp.tile([1, IPC], F32)
            nc.vector.tensor_copy(s_row, ps_sr)

            # U reassembly: full-bank psum->sbuf copies, DRAM dump, then one
            # gather DMA picks the diagonal: row (b, s, j) -> i = 16b+4s+j at
            # element offset 65536b + 16384s + 640j (+pd).
            for b in range(4):
                u_cp = postp.tile([128, 512], F32, tag="u_cp")
                nc.vector.tensor_copy(u_cp, u_ps[b])
                dma(u_dram[b * 65536:(b + 1) * 65536].rearrange(
                    "(p f) -> p f", p=128), u_cp)
            u_sb64 = postp.tile([IPC, PD], F32)
            gather = bass.AP(tensor=u_dram.tensor, offset=0,
                             ap=[[65536, 4], [16384, 4], [640, 4], [1, 128]])
            dma(u_sb64, gather)
            ps_ut = pss.tile([128, IPC], F32, tag="ps_small")
            nc.tensor.transpose(ps_ut, u_sb64, id_sb[0:IPC, 0:IPC])
            u_sb = postp.tile([128, IPC], F16)
            nc.vector.tensor_copy(u_sb, ps_ut)

            gx_sb = postp.tile([IPC, D], F32)
            nc.vector.tensor_copy(gx_sb, gx_ps)
            gxT = postp.tile([128, NCH, IPC], F16)
            for c in range(NCH):
                ptg = pss.tile([128, IPC], F32, tag="ps_small")
                nc.tensor.transpose(ptg, gx_sb[:, c * 128:(c + 1) * 128],
                                    id_sb[0:IPC, 0:IPC])
                nc.vector.tensor_copy(gxT[:, c, :], ptg)

            nc.tensor.matmul(msg_ps, u_sb, WpT_sb, start=True, stop=False)
            for c in range(NCH):
                nc.tensor.matmul(msg_ps, gxT[:, c, :], WoT_sb[:, c, :],
                                 start=False, stop=False)
            nc.tensor.matmul(msg_ps, s_row, bop_row, start=False, stop=True)

            # ---- residual + LN1 ----
            def layer_norm(v):
                stats = smallp.tile([IPC, 6], F32)
                nc.vector.bn_stats(out=stats, in_=v)
                mv = smallp.tile([IPC, 2], F32)
                nc.vector.bn_aggr(out=mv, in_=stats)
                std = smallp.tile([IPC, 1], F32)
                nc.scalar.activation(std, mv[:, 1:2], AF.Sqrt, bias=eps_col)
                rstd = smallp.tile([IPC, 1], F32)
                nc.vector.reciprocal(rstd, std)
                cen = postp.tile([IPC, H], F32)
                nc.vector.tensor_scalar(cen, v, mv[:, 0:1], rstd,
                                        OP.subtract, OP.mult)
                o = postp.tile([IPC, H], F32)
                nc.vector.tensor_mul(o, cen, gb_sb)
                nc.vector.tensor_add(o, o, bb_sb)
                return o

            h_sb = postp.tile([IPC, H], F32)
            nc.vector.tensor_add(h_sb, xi_sb, msg_ps)
            out1 = layer_norm(h_sb)

            # ---- FFN ----
            def transpose_rows(v):
                vT = postp.tile([128, NCH, IPC], F16, tag="vT")
                for c in range(NCH):
                    ptt = pss.tile([128, IPC], F32, tag="ps_small")
                    nc.tensor.transpose(ptt, v[:, c * 128:(c + 1) * 128],
                                        id_sb[0:IPC, 0:IPC])
                    nc.vector.tensor_copy(vT[:, c, :], ptt)
                return vT

            o1T = transpose_rows(out1)
            f1_ps = psflex.tile([IPC, H], F32, tag="flex")
            for c in range(NCH):
                nc.tensor.matmul(f1_ps, o1T[:, c, :], W1T_sb[:, c, :],
                                 start=(c == 0), stop=(c == NCH - 1))
            f1 = postp.tile([IPC, H], F32)
            nc.vector.tensor_add(f1, f1_ps, b1_sb)
            nc.vector.tensor_scalar_max(f1, f1, 0.0)

            f1T = transpose_rows(f1)
            f2_ps = psflex.tile([IPC, H], F32, tag="flex")
            for c in range(NCH):
                nc.tensor.matmul(f2_ps, f1T[:, c, :], W2T_sb[:, c, :],
                                 start=(c == 0), stop=(c == NCH - 1))
            h2 = postp.tile([IPC, H], F32)
            nc.vector.tensor_add(h2, f2_ps, b2_sb)
            nc.vector.tensor_add(h2, h2, out1)
            out2 = layer_norm(h2)

            nc.sync.dma_start(out=out_d, in_=out2)

    return nc


def _poison() -> np.ndarray:
    p = np.zeros((1, 128), np.float32)
    p[0, 127] = -1e9
    return p


def _cmat() -> np.ndarray:
    c = np.zeros((128, 4, 128), np.float32)
    c[:, 0, :] = np.eye(128)
    c[:, 1, :] = 1.0
    c[:, 2, :] = np.eye(128, k=-1)     # shift1[q, p] = (q == p+1)
    c[0, 3, 127] = 1.0                  # shift2[q, p] = (q==0)&(p==127)
    return c


def prep_in_maps(inputs) -> list[dict]:
    x = np.asarray(inputs["x"], np.float32)
    pf = np.asarray(inputs["pair_feats"], np.float32)
    W_att = np.asarray(inputs["W_att"], np.float32)
    b_att = np.asarray(inputs["b_att"], np.float32)
    W_obj = np.asarray(inputs["W_obj"], np.float32)
    b_obj = np.asarray(inputs["b_obj"], np.float32)
    W_pair = np.asarray(inputs["W_pair"], np.float32)
    b_pair = np.asarray(inputs["b_pair"], np.float32)
    ln_g = np.asarray(inputs["ln_g"], np.float32)
    ln_b = np.asarray(inputs["ln_b"], np.float32)
    W1 = np.asarray(inputs["W1"], np.float32)
    b1 = np.asarray(inputs["b1"], np.float32)
    W2 = np.asarray(inputs["W2"], np.float32)
    b2 = np.asarray(inputs["b2"], np.float32)

    wa, wb, wc = W_att[0, :D], W_att[0, D:2 * D], W_att[0, 2 * D:]
    xpad = np.concatenate([x, np.zeros((1, D), np.float32)], axis=0)

    # fold wc into pf columns; recover U via pre-divided W_pair.T rows.
    colscale = np.sign(wc) * np.maximum(np.abs(wc), 6e-5)
    colscale[colscale == 0] = 6e-5
    # 1/511 (the mean over neighbors) is folded into the three weight paths
    # that consume raw alpha: U@WpT, (A@x)@WoT, and s_alpha*bop.
    WpT2 = (W_pair.T / colscale[:, None] / T).astype(np.float16)
    WoT2 = (W_obj.T / T).astype(np.float16)
    dxf = np.diff(xpad[:K + 1], axis=0)

    base = dict(
        xf=xpad.astype(np.float16),
        dxf=dxf.astype(np.float16),
        cmat=_cmat(),
        ones16=np.ones((128, 8), np.float16),
        poison=_poison(),
        wab=np.stack([wa, wb]).astype(np.float16),
        b_att=b_att.astype(np.float32),
        bias5=np.stack([ln_g, ln_b, b1, b2,
                        (b_obj + b_pair) / T]).astype(np.float32),
        WpT=np.ascontiguousarray(WpT2),
        WoT=np.ascontiguousarray(WoT2),
        W1T=np.ascontiguousarray(W1.T).astype(np.float16),
        W2T=np.ascontiguousarray(W2.T).astype(np.float16),
    )

    pfr = pf.reshape(K, T, PD)
    tgrid = np.arange(128)[:, None] + 128 * np.arange(NCH)[None, :]   # [128, NCH]

    in_maps = []
    for core in range(NCORES):
        ig = np.arange(core * IPC, (core + 1) * IPC)
        mlt = (tgrid[:, :, None] < ig[None, None, :]).astype(np.float32)
        mge = ((tgrid[:, :, None] >= ig[None, None, :])
               & (tgrid[:, :, None] <= T - 1)).astype(np.float16)
        # [chunk, t, i, pd] layout -> each tile DMA is one contiguous burst
        shard = np.zeros((NCH * 128, IPC, PD), np.float16)
        shard[:T] = (pfr[ig] * colscale[None, None, :]).transpose(1, 0, 2)
        xi = x[ig]
        m = dict(base)
        m.update(
            pf=shard.reshape(NCH, 128, IPC, PD),
            xi=xi.astype(np.float32),
            xi16=xi.astype(np.float16),
            mask_lt=mlt,
            mask_ge=mge,
        )
        in_maps.append(m)
    return in_maps


_COMPILED = None


def _get_program() -> bacc.Bacc:
    global _COMPILED
    if _COMPILED is None:
        nc = build_program()
        nc.compile()
        _COMPILED = nc
    return _COMPILED


TRACE = False
LAST_RESULT = None


def _install_axon_ntff_hook():
    """The container's antenv lacks axon_hooks; recreate it from trn_boot's
    ctypes implementation so trace=True can capture NTFF profiles."""
    import sys
    import types
    try:
        from antenv.axon_hooks import get_axon_ntff_profile_hook  # noqa: F401
        return
    except ImportError:
        pass
    from trn_agent_boot.trn_boot import _ntff_profile_via_ctypes
    hook = _ntff_profile_via_ctypes("/opt/axon/libaxon_pjrt.so")
    m = types.ModuleType("antenv.axon_hooks")
    m.get_axon_ntff_profile_hook = lambda: hook
    sys.modules["antenv.axon_hooks"] = m


def kernel(**inputs) -> np.ndarray:
    import concourse.bass_utils as bu
    from concourse.bass_utils import run_bass_kernel_spmd
    global LAST_RESULT
    if TRACE:
        _install_axon_ntff_hook()
        bu.upload_artifacts = lambda tmpdir: str(tmpdir)  # no bucket here
    nc = _get_program()
    in_maps = prep_in_maps(inputs)
    res = run_bass_kernel_spmd(nc, in_maps, list(range(NCORES)), trace=TRACE)
    LAST_RESULT = res
    outs = [res.results[c]["out"] for c in range(NCORES)]
    return np.concatenate(outs, axis=0).astype(np.float32)


# revision 35
# speedup vs baseline: 1.0825x; 1.0127x over previous
"""Trainium2 Bass kernel for nn_ARTLayer (gnn_message_passing).

Math (reference):
    j(i,t) = t + (t>=i)                                    # [K, K-1] neighbor index
    alpha  = sigmoid(x@wa [i] + x@wb [j] + pf@wc + b_att)  # [K, K-1]
    msgs   = mean_t alpha * ((x@WobjT + b_obj)[j] + pf@WpairT + b_pair)
    out    = LN(x + msgs); out = LN(out + FFN(out))

Key algebraic rewrite (removes the 34-GFLOP [P,PD]x[PD,H] einsum):
    sum_t a*(pf@WpT)  = (sum_t a*pf) @ WpT               -> U[i,:] @ WpT
    sum_t a*oj[j]     = (A @ x) @ WobjT                  -> Gx[i,:] @ WoT
      with A[i,j] decomposed via lo/hi shifted views of x and a t>=i mask
    sum_t a*(b_obj+b_pair) = s_alpha[i] * bop

Sharding: rows i split across 8 cores (64 each); small tensors replicated;
host concatenates the per-core [64, 512] outputs.

Implementation notes (driven by NTFF profiles):
  - wc is folded into pf on the host (column scales, floored at fp16
    min-normal); sc becomes a pure fp16 halving-tree add-reduce and U is
    recovered exactly via W_pairT rows pre-divided by the scales.
  - 1/511 (the neighbor mean) is folded into WpT/WoT/bop on the host, and
    the t=511 pad slot is poisoned with -1e9 pre-sigmoid, so raw sigmoid
    output is used directly with no mask/scale multiplies.
  - U accumulation runs as M=4 quad matmuls (alpha quad stationary, four pf
    blocks streaming at N=512); the wanted rows sit on the block diagonal
    and are gathered by a stride-640 DRAM access pattern after a bank dump.
  - pf is re-laid-out on the host to [chunk, t, i, pd] so each tile DMA is
    one fully-contiguous 8KB-per-partition burst, and all DMAs are spread
    round-robin over the three DMA-capable engines (sync/scalar HWDGE,
    gpsimd SWDGE) instead of serializing on one queue.
"""
import numpy as np

import concourse.bass as bass
import concourse.tile as tile
from concourse import bacc, mybir

F32, F16 = mybir.dt.float32, mybir.dt.float16
AX = mybir.AxisListType
OP = mybir.AluOpType
AF = mybir.ActivationFunctionType

K, D, H, PD = 512, 512, 512, 128
T = K - 1                      # 511 neighbors per row
NCORES, IPC = 8, 64            # rows per core
NCH = 4                        # t-chunks of 128 (last chunk row 127 is t=511 pad)
IB, NIB = 32, 2                # i-block within a core
EPS = 1e-5


def build_program() -> bacc.Bacc:
    nc = bacc.Bacc("TRN2", target_bir_lowering=False, debug=False)

    def inp(name, shape, dt):
        return nc.dram_tensor(name, shape, dt, kind="ExternalInput").ap()

    pf = inp("pf", [NCH, 128, IPC, PD], F16)    # [chunk, t-in-chunk, i, pd]
    xf = inp("xf", [K + 1, D], F16)             # x with one zero pad row
    dxf = inp("dxf", [K, D], F16)               # x[t+1] - x[t], host computed
    xi = inp("xi", [IPC, D], F32)               # this core's rows of x
    xi16 = inp("xi16", [IPC, D], F16)
    mask_lt = inp("mask_lt", [128, NCH, IPC], F32)   # t <  i_global
    mask_ge = inp("mask_ge", [128, NCH, IPC], F16)   # t >= i_global and t <= 510
    cmat = inp("cmat", [128, 4, 128], F32)      # [ident | ones | shift1 | shift2]
    ones16 = inp("ones16", [128, 8], F16)
    poison = inp("poison", [1, 128], F32)       # -1e9 at slot 127, else 0
    wab = inp("wab", [2, D], F16)               # [wa; wb]
    b_att = inp("b_att", [1], F32)
    bias5 = inp("bias5", [5, H], F32)           # [ln_g; ln_b; b1; b2; bop/511]
    WpT = inp("WpT", [PD, H], F16)              # W_pair.T / colscale / 511
    WoT = inp("WoT", [D, H], F16)               # W_obj.T / 511
    W1T = inp("W1T", [H, H], F16)
    W2T = inp("W2T", [H, H], F16)

    out_d = nc.dram_tensor("out", [IPC, H], F32, kind="ExternalOutput").ap()
    u_dram = nc.dram_tensor("u_dram", [4 * 128 * 512], F32).ap()  # U bank dumps

    with tile.TileContext(nc) as tc:
        with (
            tc.tile_pool(name="const", bufs=1) as cpool,
            tc.tile_pool(name="pfp", bufs=3) as pfp,
            tc.tile_pool(name="scrp", bufs=2) as scrp,
            tc.tile_pool(name="smallp", bufs=4) as smallp,
            tc.tile_pool(name="postp", bufs=2) as postp,
            tc.tile_pool(name="pss", bufs=2, space="PSUM") as pss,
            tc.tile_pool(name="psflex", bufs=4, space="PSUM") as psflex,
            tc.tile_pool(name="psmp", bufs=1, space="PSUM") as psmp,
            tc.tile_pool(name="psgp", bufs=1, space="PSUM") as psgp,
        ):
            # round-robin DMA issue over the three DMA-capable engines
            dmaq = [nc.sync, nc.scalar, nc.gpsimd]
            qi = [0]

            def dma(out, in_):
                eng = dmaq[qi[0] % len(dmaq)]
                qi[0] += 1
                eng.dma_start(out=out, in_=in_)

            # ---- constants & weights to SBUF ----
            cm_sb = cpool.tile([128, 4, 128], F32)
            dma(cm_sb, cmat)
            id_sb = cm_sb[:, 0, :]
            ones_sb = cm_sb[:, 1, :]
            sh1_sb = cm_sb[:, 2, :]
            sh2_sb = cm_sb[:, 3, :]
            wab_sb = cpool.tile([128, 2, D], F16)
            dma(wab_sb, wab[None, :, :].to_broadcast([128, 2, D]))
            wa_b = wab_sb[0:IPC, 0, :]
            wb_b = wab_sb[:, 1, :]
            bias_sb = cpool.tile([IPC, 5, H], F32)
            dma(bias_sb, bias5[None, :, :].to_broadcast([IPC, 5, H]))
            gb_sb = bias_sb[:, 0, :]
            bb_sb = bias_sb[:, 1, :]
            b1_sb = bias_sb[:, 2, :]
            b2_sb = bias_sb[:, 3, :]
            bop_row = bias_sb[0:1, 4, :]
            ones16_sb = cpool.tile([128, 8], F16)
            dma(ones16_sb, ones16)
            poison_sb = cpool.tile([1, 128], F32)
            dma(poison_sb, poison)
            mlt_sb = cpool.tile([128, NCH, IPC], F32)
            dma(mlt_sb, mask_lt)
            mge_sb = cpool.tile([128, NCH, IPC], F16)
            dma(mge_sb, mask_ge)
            WpT_sb = cpool.tile([128, H], F16)
            dma(WpT_sb, WpT)
            WoT_sb = cpool.tile([128, NCH, H], F16)
            dma(WoT_sb, WoT.rearrange("(c p) h -> p c h", p=128))
            W1T_sb = cpool.tile([128, NCH, H], F16)
            dma(W1T_sb, W1T.rearrange("(c p) h -> p c h", p=128))
            W2T_sb = cpool.tile([128, NCH, H], F16)
            dma(W2T_sb, W2T.rearrange("(c p) h -> p c h", p=128))
            xi_sb = cpool.tile([IPC, D], F32)
            dma(xi_sb, xi)
            xi16_sb = cpool.tile([IPC, D], F16)
            dma(xi16_sb, xi16)
            xlo = cpool.tile([128, NCH, D], F16)
            dma(xlo, xf[0:K].rearrange("(c p) d -> p c d", p=128))
            dx = cpool.tile([128, NCH, D], F16)
            dma(dx, dxf.rearrange("(c p) d -> p c d", p=128))
            b_att_col = cpool.tile([IPC, 1], F32)
            dma(b_att_col, b_att[None, :].to_broadcast([IPC, 1]))

            eps_col = cpool.tile([IPC, 1], F32)
            nc.vector.memset(eps_col, EPS)

            # ---- sa (this core's rows) and sb (all rows) ----
            scr_sa = smallp.tile([IPC, D], F16)
            nc.vector.tensor_mul(scr_sa, xi16_sb, wa_b)
            sa_col = smallp.tile([IPC, 1], F32)
            nc.vector.tensor_reduce(sa_col, scr_sa, axis=AX.X, op=OP.add)
            nc.vector.tensor_add(sa_col, sa_col, b_att_col)
            sa_diag = smallp.tile([IPC, IPC], F32)
            nc.vector.tensor_mul(sa_diag, id_sb[0:IPC, 0:IPC],
                                 sa_col.to_broadcast([IPC, IPC]))

            sb_cols = smallp.tile([128, NCH], F32)
            for c in range(NCH):
                scr_sb = smallp.tile([128, D], F16)
                nc.vector.tensor_mul(scr_sb, xlo[:, c, :], wb_b)
                nc.vector.tensor_reduce(
                    sb_cols[:, c:c + 1], scr_sb, axis=AX.X, op=OP.add)

            # sb_hi[p, c] = sb[c*128+p+1] via shift matmuls; slot 511 stays 0
            sbhi_ps = pss.tile([128, NCH], F32, tag="ps_small")
            nc.tensor.matmul(sbhi_ps, sh1_sb, sb_cols, start=True, stop=False)
            nc.tensor.matmul(sbhi_ps[:, 0:NCH - 1], sh2_sb, sb_cols[:, 1:NCH],
                             start=False, stop=True)
            sbhi_cols = smallp.tile([128, NCH], F32)
            nc.vector.tensor_copy(sbhi_cols, sbhi_ps)

            # ---- SBJ[t, i] = sa[i] + b_att + sb_hi[t] + mask_lt*(sb_lo-sb_hi),
            #      with -1e9 poison at the t=511 pad slot ----
            sbj = cpool.tile([128, NCH, IPC], F32)
            for c in range(NCH):
                diffc = smallp.tile([128, 1], F32)
                nc.vector.tensor_tensor(
                    diffc, sb_cols[:, c:c + 1], sbhi_cols[:, c:c + 1], OP.subtract)
                diagc = smallp.tile([128, 128], F32)
                nc.vector.tensor_mul(diagc, id_sb, diffc.to_broadcast([128, 128]))
                diagb = smallp.tile([128, 128], F32)
                nc.vector.tensor_mul(
                    diagb, id_sb, sbhi_cols[:, c:c + 1].to_broadcast([128, 128]))
                ps_sbj = pss.tile([128, IPC], F32, tag="ps_small")
                nc.tensor.matmul(ps_sbj, ones_sb[0:IPC, :], sa_diag,
                                 start=True, stop=False)
                nc.tensor.matmul(ps_sbj, diagb, ones_sb[:, 0:IPC],
                                 start=False, stop=False)
                if c == NCH - 1:
                    # poison: sigmoid(-1e9) = 0 exactly, pad row drops out
                    nc.tensor.matmul(ps_sbj, poison_sb, ones_sb[0:1, 0:IPC],
                                     start=False, stop=False)
                nc.tensor.matmul(ps_sbj, diagc, mlt_sb[:, c, :],
                                 start=False, stop=True)
                nc.vector.tensor_copy(sbj[:, c, :], ps_sbj)

            # ---- main edge pass ----
            alpha_full = cpool.tile([128, NCH, IPC], F16)   # raw sigmoid out
            age_full = cpool.tile([128, NCH, IPC], F16)     # masked (t>=i) alpha
            gx_ps = psgp.tile([IPC, D], F32)                # sum_t a*x[j]
            msg_ps = psmp.tile([IPC, H], F32)
            # U quad rows: bank b, partition slot 32s..32s+3 holds i=16b+4s+j
            u_ps = [psflex.tile([128, 512], F32, tag="flex", name=f"u_ps{b}")
                    for b in range(4)]
            for b in range(4):
                nc.vector.memset(u_ps[b], 0.0)

            for c in range(NCH):
                for ib in range(NIB):
                    i0 = ib * IB
                    pf_t = pfp.tile([128, IB, PD], F16)
                    for hh in range(2):
                        dma(pf_t[:, 16 * hh:16 * (hh + 1), :],
                            pf[c, :, i0 + 16 * hh:i0 + 16 * (hh + 1), :])
                    # sc = sum_pd pf_sent (wc pre-folded): fp16 halving tree
                    scr = scrp.tile([128, IB, 64], F16)
                    nc.vector.tensor_add(scr, pf_t[:, :, 0:64], pf_t[:, :, 64:128])
                    w = 32
                    while w >= 2:
                        nc.vector.tensor_add(
                            scr[:, :, 0:w], scr[:, :, 0:w], scr[:, :, w:2 * w])
                        w //= 2
                    sc_t = smallp.tile([128, IB], F32)
                    nc.vector.tensor_add(sc_t, scr[:, :, 0], scr[:, :, 1])
                    aarg = smallp.tile([128, IB], F32)
                    nc.vector.tensor_add(aarg, sc_t, sbj[:, c, i0:i0 + IB])
                    nc.scalar.activation(
                        alpha_full[:, c, i0:i0 + IB], aarg, AF.Sigmoid)
                    nc.vector.tensor_mul(
                        age_full[:, c, i0:i0 + IB], alpha_full[:, c, i0:i0 + IB],
                        mge_sb[:, c, i0:i0 + IB])
                    # U quads: lhsT = 4 alpha columns, rhs = 4 pf blocks; the
                    # wanted rows sit on the diagonal (gathered via DRAM AP)
                    for q in range(IB // 4):
                        i = i0 + 4 * q
                        b, sp = divmod(i // 4, 4)
                        nc.tensor.matmul(
                            u_ps[b][32 * sp:32 * sp + 4, :],
                            alpha_full[:, c, i:i + 4],
                            pf_t[:, 4 * q:4 * q + 4, :],
                            start=(c == 0), stop=(c == NCH - 1),
                            tile_position=(0, 32 * sp))
                nc.tensor.matmul(gx_ps, alpha_full[:, c, :], xlo[:, c, :],
                                 start=(c == 0), stop=False)
                nc.tensor.matmul(gx_ps, age_full[:, c, :], dx[:, c, :],
                                 start=False, stop=(c == NCH - 1))

            # ---- messages = U@WpT + Gx@WoT + s_alpha x bop ----
            s_ps = pss.tile([IPC, 1], F32, tag="ps_small")
            for c in range(NCH):
                nc.tensor.matmul(s_ps, alpha_full[:, c, :], ones16_sb[:, 0:1],
                                 start=(c == 0), stop=(c == NCH - 1))
            s_col = smallp.tile([IPC, 1], F32)
            nc.vector.tensor_copy(s_col, s_ps)
            ps_sr = pss.tile([1, IPC], F32, tag="ps_small")
            nc.tensor.transpose(ps_sr, s_col, id_sb[0:IPC, 0:IPC])
            s_row = smallp.tile([1, IPC], F32)
            nc.vector.tensor_copy(s_row, ps_sr)

            # U reassembly: full-bank psum->sbuf copies, DRAM dump, then one
            # gather DMA picks the diagonal: row (b, s, j) -> i = 16b+4s+j at
            # element offset 65536b + 16384s + 640j (+pd).
            for b in range(4):
                u_cp = postp.tile([128, 512], F32, tag="u_cp")
                nc.vector.tensor_copy(u_cp, u_ps[b])
                # keep the whole u_dram bounce on one queue: Tile does not
                # track ordering through raw DRAM tensors, same-queue FIFO does
                nc.sync.dma_start(
                    out=u_dram[b * 65536:(b + 1) * 65536].rearrange(
                        "(p f) -> p f", p=128), in_=u_cp)
            u_sb64 = postp.tile([IPC, PD], F32)
            gather = bass.AP(tensor=u_dram.tensor, offset=0,
                             ap=[[65536, 4], [16384, 4], [640, 4], [1, 128]])
            nc.sync.dma_start(out=u_sb64, in_=gather)
            ps_ut = pss.tile([128, IPC], F32, tag="ps_small")
            nc.tensor.transpose(ps_ut, u_sb64, id_sb[0:IPC, 0:IPC])
            u_sb = postp.tile([128, IPC], F16)
            nc.vector.tensor_copy(u_sb, ps_ut)

            gx_sb = postp.tile([IPC, D], F32)
            nc.vector.tensor_copy(gx_sb, gx_ps)
            gxT = postp.tile([128, NCH, IPC], F16)
            for c in range(NCH):
                ptg = pss.tile([128, IPC], F32, tag="ps_small")
                nc.tensor.transpose(ptg, gx_sb[:, c * 128:(c + 1) * 128],
                                    id_sb[0:IPC, 0:IPC])
                nc.vector.tensor_copy(gxT[:, c, :], ptg)

            nc.tensor.matmul(msg_ps, u_sb, WpT_sb, start=True, stop=False)
            for c in range(NCH):
                nc.tensor.matmul(msg_ps, gxT[:, c, :], WoT_sb[:, c, :],
                                 start=False, stop=False)
            nc.tensor.matmul(msg_ps, s_row, bop_row, start=False, stop=True)

            # ---- residual + LN1 ----
            def layer_norm(v):
                stats = smallp.tile([IPC, 6], F32)
                nc.vector.bn_stats(out=stats, in_=v)
                mv = smallp.tile([IPC, 2], F32)
                nc.vector.bn_aggr(out=mv, in_=stats)
                std = smallp.tile([IPC, 1], F32)
                nc.scalar.activation(std, mv[:, 1:2], AF.Sqrt, bias=eps_col)
                rstd = smallp.tile([IPC, 1], F32)
                nc.vector.reciprocal(rstd, std)
                cen = postp.tile([IPC, H], F32)
                nc.vector.tensor_scalar(cen, v, mv[:, 0:1], rstd,
                                        OP.subtract, OP.mult)
                o = postp.tile([IPC, H], F32)
                nc.vector.tensor_mul(o, cen, gb_sb)
                nc.vector.tensor_add(o, o, bb_sb)
                return o

            h_sb = postp.tile([IPC, H], F32)
            nc.vector.tensor_add(h_sb, xi_sb, msg_ps)
            out1 = layer_norm(h_sb)

            # ---- FFN ----
            def transpose_rows(v):
                vT = postp.tile([128, NCH, IPC], F16, tag="vT")
                for c in range(NCH):
                    ptt = pss.tile([128, IPC], F32, tag="ps_small")
                    nc.tensor.transpose(ptt, v[:, c * 128:(c + 1) * 128],
                                        id_sb[0:IPC, 0:IPC])
                    nc.vector.tensor_copy(vT[:, c, :], ptt)
                return vT

            o1T = transpose_rows(out1)
            f1_ps = psflex.tile([IPC, H], F32, tag="flex")
            for c in range(NCH):
                nc.tensor.matmul(f1_ps, o1T[:, c, :], W1T_sb[:, c, :],
                                 start=(c == 0), stop=(c == NCH - 1))
            f1 = postp.tile([IPC, H], F32)
            nc.vector.tensor_add(f1, f1_ps, b1_sb)
            nc.vector.tensor_scalar_max(f1, f1, 0.0)

            f1T = transpose_rows(f1)
            f2_ps = psflex.tile([IPC, H], F32, tag="flex")
            for c in range(NCH):
                nc.tensor.matmul(f2_ps, f1T[:, c, :], W2T_sb[:, c, :],
                                 start=(c == 0), stop=(c == NCH - 1))
            h2 = postp.tile([IPC, H], F32)
            nc.vector.tensor_add(h2, f2_ps, b2_sb)
            nc.vector.tensor_add(h2, h2, out1)
            out2 = layer_norm(h2)

            nc.sync.dma_start(out=out_d, in_=out2)

    return nc


def _poison() -> np.ndarray:
    p = np.zeros((1, 128), np.float32)
    p[0, 127] = -1e9
    return p


def _cmat() -> np.ndarray:
    c = np.zeros((128, 4, 128), np.float32)
    c[:, 0, :] = np.eye(128)
    c[:, 1, :] = 1.0
    c[:, 2, :] = np.eye(128, k=-1)     # shift1[q, p] = (q == p+1)
    c[0, 3, 127] = 1.0                  # shift2[q, p] = (q==0)&(p==127)
    return c


def prep_in_maps(inputs) -> list[dict]:
    x = np.asarray(inputs["x"], np.float32)
    pf = np.asarray(inputs["pair_feats"], np.float32)
    W_att = np.asarray(inputs["W_att"], np.float32)
    b_att = np.asarray(inputs["b_att"], np.float32)
    W_obj = np.asarray(inputs["W_obj"], np.float32)
    b_obj = np.asarray(inputs["b_obj"], np.float32)
    W_pair = np.asarray(inputs["W_pair"], np.float32)
    b_pair = np.asarray(inputs["b_pair"], np.float32)
    ln_g = np.asarray(inputs["ln_g"], np.float32)
    ln_b = np.asarray(inputs["ln_b"], np.float32)
    W1 = np.asarray(inputs["W1"], np.float32)
    b1 = np.asarray(inputs["b1"], np.float32)
    W2 = np.asarray(inputs["W2"], np.float32)
    b2 = np.asarray(inputs["b2"], np.float32)

    wa, wb, wc = W_att[0, :D], W_att[0, D:2 * D], W_att[0, 2 * D:]
    xpad = np.concatenate([x, np.zeros((1, D), np.float32)], axis=0)

    # fold wc into pf columns; recover U via pre-divided W_pair.T rows.
    colscale = np.sign(wc) * np.maximum(np.abs(wc), 6e-5)
    colscale[colscale == 0] = 6e-5
    # 1/511 (the mean over neighbors) is folded into the three weight paths
    # that consume raw alpha: U@WpT, (A@x)@WoT, and s_alpha*bop.
    WpT2 = (W_pair.T / colscale[:, None] / T).astype(np.float16)
    WoT2 = (W_obj.T / T).astype(np.float16)
    dxf = np.diff(xpad[:K + 1], axis=0)

    base = dict(
        xf=xpad.astype(np.float16),
        dxf=dxf.astype(np.float16),
        cmat=_cmat(),
        ones16=np.ones((128, 8), np.float16),
        poison=_poison(),
        wab=np.stack([wa, wb]).astype(np.float16),
        b_att=b_att.astype(np.float32),
        bias5=np.stack([ln_g, ln_b, b1, b2,
                        (b_obj + b_pair) / T]).astype(np.float32),
        WpT=np.ascontiguousarray(WpT2),
        WoT=np.ascontiguousarray(WoT2),
        W1T=np.ascontiguousarray(W1.T).astype(np.float16),
        W2T=np.ascontiguousarray(W2.T).astype(np.float16),
    )

    pfr = pf.reshape(K, T, PD)
    tgrid = np.arange(128)[:, None] + 128 * np.arange(NCH)[None, :]   # [128, NCH]

    in_maps = []
    for core in range(NCORES):
        ig = np.arange(core * IPC, (core + 1) * IPC)
        mlt = (tgrid[:, :, None] < ig[None, None, :]).astype(np.float32)
        mge = ((tgrid[:, :, None] >= ig[None, None, :])
               & (tgrid[:, :, None] <= T - 1)).astype(np.float16)
        # [chunk, t, i, pd] layout -> each tile DMA is one contiguous burst
        shard = np.zeros((NCH * 128, IPC, PD), np.float16)
        shard[:T] = (pfr[ig] * colscale[None, None, :]).transpose(1, 0, 2)
        xi = x[ig]
        m = dict(base)
        m.update(
            pf=shard.reshape(NCH, 128, IPC, PD),
            xi=xi.astype(np.float32),
            xi16=xi.astype(np.float16),
            mask_lt=mlt,
            mask_ge=mge,
        )
        in_maps.append(m)
    return in_maps


_COMPILED = None


def _get_program() -> bacc.Bacc:
    global _COMPILED
    if _COMPILED is None:
        nc = build_program()
        nc.compile()
        _COMPILED = nc
    return _COMPILED


TRACE = False
LAST_RESULT = None


def _install_axon_ntff_hook():
    """The container's antenv lacks axon_hooks; recreate it from trn_boot's
    ctypes implementation so trace=True can capture NTFF profiles."""
    import sys
    import types
    try:
        from antenv.axon_hooks import get_axon_ntff_profile_hook  # noqa: F401
        return
    except ImportError:
        pass
    from trn_agent_boot.trn_boot import _ntff_profile_via_ctypes
    hook = _ntff_profile_via_ctypes("/opt/axon/libaxon_pjrt.so")
    m = types.ModuleType("antenv.axon_hooks")
    m.get_axon_ntff_profile_hook = lambda: hook
    sys.modules["antenv.axon_hooks"] = m


def kernel(**inputs) -> np.ndarray:
    import concourse.bass_utils as bu
    from concourse.bass_utils import run_bass_kernel_spmd
    global LAST_RESULT
    if TRACE:
        _install_axon_ntff_hook()
        bu.upload_artifacts = lambda tmpdir: str(tmpdir)  # no bucket here
    nc = _get_program()
    in_maps = prep_in_maps(inputs)
    res = run_bass_kernel_spmd(nc, in_maps, list(range(NCORES)), trace=TRACE)
    LAST_RESULT = res
    outs = [res.results[c]["out"] for c in range(NCORES)]
    return np.concatenate(outs, axis=0).astype(np.float32)


# revision 36
# speedup vs baseline: 1.1896x; 1.0990x over previous
"""Trainium2 Bass kernel for nn_ARTLayer (gnn_message_passing).

Math (reference):
    j(i,t) = t + (t>=i)                                    # [K, K-1] neighbor index
    alpha  = sigmoid(x@wa [i] + x@wb [j] + pf@wc + b_att)  # [K, K-1]
    msgs   = mean_t alpha * ((x@WobjT + b_obj)[j] + pf@WpairT + b_pair)
    out    = LN(x + msgs); out = LN(out + FFN(out))

Key algebraic rewrite (removes the 34-GFLOP [P,PD]x[PD,H] einsum):
    sum_t a*(pf@WpT)  = (sum_t a*pf) @ WpT               -> U[i,:] @ WpT
    sum_t a*oj[j]     = (A @ x) @ WobjT                  -> Gx[i,:] @ WoT
      with A[i,j] decomposed via lo/hi shifted views of x and a t>=i mask
    sum_t a*(b_obj+b_pair) = s_alpha[i] * bop

Sharding: rows i split across 8 cores (64 each); small tensors replicated;
host concatenates the per-core [64, 512] outputs.

Implementation notes (driven by NTFF profiles):
  - wc is folded into pf on the host (column scales, floored at fp16
    min-normal); sc becomes a pure fp16 halving-tree add-reduce and U is
    recovered exactly via W_pairT rows pre-divided by the scales.
  - 1/511 (the neighbor mean) is folded into WpT/WoT/bop on the host, and
    the t=511 pad slot is poisoned with -1e9 pre-sigmoid, so raw sigmoid
    output is used directly with no mask/scale multiplies.
  - U accumulation runs as M=4 quad matmuls (alpha quad stationary, four pf
    blocks streaming at N=512); the wanted rows sit on the block diagonal
    and are gathered by a stride-640 DRAM access pattern after a bank dump.
  - pf is re-laid-out on the host to [chunk, t, i, pd] so each tile DMA is
    one fully-contiguous 8KB-per-partition burst, and all DMAs are spread
    round-robin over the three DMA-capable engines (sync/scalar HWDGE,
    gpsimd SWDGE) instead of serializing on one queue.
"""
import numpy as np

import concourse.bass as bass
import concourse.tile as tile
from concourse import bacc, mybir

F32, F16 = mybir.dt.float32, mybir.dt.float16
AX = mybir.AxisListType
OP = mybir.AluOpType
AF = mybir.ActivationFunctionType

K, D, H, PD = 512, 512, 512, 128
T = K - 1                      # 511 neighbors per row
NCORES, IPC = 8, 64            # rows per core
NCH = 4                        # t-chunks of 128 (last chunk row 127 is t=511 pad)
IB, NIB = 32, 2                # i-block within a core
EPS = 1e-5


def build_program() -> bacc.Bacc:
    nc = bacc.Bacc("TRN2", target_bir_lowering=False, debug=False)

    def inp(name, shape, dt):
        return nc.dram_tensor(name, shape, dt, kind="ExternalInput").ap()

    pf = inp("pf", [NCH, 128, IPC, PD], F16)    # [chunk, t-in-chunk, i, pd]
    xf = inp("xf", [K + 1, D], F16)             # x with one zero pad row
    dxf = inp("dxf", [K, D], F16)               # x[t+1] - x[t], host computed
    xi = inp("xi", [IPC, D], F32)               # this core's rows of x
    xi16 = inp("xi16", [IPC, D], F16)
    mask_lt = inp("mask_lt", [128, NCH, IPC], F32)   # t <  i_global
    mask_ge = inp("mask_ge", [128, NCH, IPC], F16)   # t >= i_global and t <= 510
    cmat = inp("cmat", [128, 4, 128], F32)      # [ident | ones | shift1 | shift2]
    ones16 = inp("ones16", [128, 8], F16)
    poison = inp("poison", [1, 128], F32)       # -1e9 at slot 127, else 0
    wab = inp("wab", [2, D], F16)               # [wa; wb]
    b_att = inp("b_att", [1], F32)
    bias5 = inp("bias5", [5, H], F32)           # [ln_g; ln_b; b1; b2; bop/511]
    WpT = inp("WpT", [PD, H], F16)              # W_pair.T / colscale / 511
    WoT = inp("WoT", [D, H], F16)               # W_obj.T / 511
    W1T = inp("W1T", [H, H], F16)
    W2T = inp("W2T", [H, H], F16)

    out_d = nc.dram_tensor("out", [IPC, H], F32, kind="ExternalOutput").ap()
    u_dram = nc.dram_tensor("u_dram", [4 * 128 * 512], F32).ap()  # U bank dumps

    with tile.TileContext(nc) as tc:
        with (
            tc.tile_pool(name="const", bufs=1) as cpool,
            tc.tile_pool(name="pfp", bufs=4) as pfp,
            tc.tile_pool(name="scrp", bufs=2) as scrp,
            tc.tile_pool(name="smallp", bufs=4) as smallp,
            tc.tile_pool(name="postp", bufs=2) as postp,
            tc.tile_pool(name="pss", bufs=2, space="PSUM") as pss,
            tc.tile_pool(name="psflex", bufs=4, space="PSUM") as psflex,
            tc.tile_pool(name="psmp", bufs=1, space="PSUM") as psmp,
            tc.tile_pool(name="psgp", bufs=1, space="PSUM") as psgp,
        ):
            # DMA policy: critical-path loads alternate the two HWDGE
            # queues (sync/scalar); bulky tail-only weights go to the gpsimd
            # SWDGE queue and are emitted at their use sites so the scheduler
            # does not front-load them ahead of pf tiles.
            qi = [0]

            def dma(out, in_):
                eng = nc.sync if qi[0] % 2 == 0 else nc.scalar
                qi[0] += 1
                eng.dma_start(out=out, in_=in_)

            def dma_late(out, in_):
                nc.gpsimd.dma_start(out=out, in_=in_)

            # ---- constants & weights to SBUF ----
            cm_sb = cpool.tile([128, 4, 128], F32)
            dma(cm_sb, cmat)
            id_sb = cm_sb[:, 0, :]
            ones_sb = cm_sb[:, 1, :]
            sh1_sb = cm_sb[:, 2, :]
            sh2_sb = cm_sb[:, 3, :]
            wab_sb = cpool.tile([128, 2, D], F16)
            dma(wab_sb, wab[None, :, :].to_broadcast([128, 2, D]))
            wa_b = wab_sb[0:IPC, 0, :]
            wb_b = wab_sb[:, 1, :]
            bias_sb = cpool.tile([IPC, 5, H], F32)
            dma_late(bias_sb, bias5[None, :, :].to_broadcast([IPC, 5, H]))
            gb_sb = bias_sb[:, 0, :]
            bb_sb = bias_sb[:, 1, :]
            b1_sb = bias_sb[:, 2, :]
            b2_sb = bias_sb[:, 3, :]
            bop_row = bias_sb[0:1, 4, :]
            ones16_sb = cpool.tile([128, 8], F16)
            dma(ones16_sb, ones16)
            poison_sb = cpool.tile([1, 128], F32)
            dma(poison_sb, poison)
            mlt_sb = cpool.tile([128, NCH, IPC], F32)
            dma(mlt_sb, mask_lt)
            mge_sb = cpool.tile([128, NCH, IPC], F16)
            dma(mge_sb, mask_ge)
            xi16_sb = cpool.tile([IPC, D], F16)
            dma(xi16_sb, xi16)
            xlo = cpool.tile([128, NCH, D], F16)
            dma(xlo, xf[0:K].rearrange("(c p) d -> p c d", p=128))
            b_att_col = cpool.tile([IPC, 1], F32)
            dma(b_att_col, b_att[None, :].to_broadcast([IPC, 1]))
            # tail-only loads on the slow queue, in rough use order
            dx = cpool.tile([128, NCH, D], F16)
            dma_late(dx, dxf.rearrange("(c p) d -> p c d", p=128))
            WpT_sb = cpool.tile([128, H], F16)
            dma_late(WpT_sb, WpT)
            WoT_sb = cpool.tile([128, NCH, H], F16)
            dma_late(WoT_sb, WoT.rearrange("(c p) h -> p c h", p=128))
            xi_sb = cpool.tile([IPC, D], F32)
            dma_late(xi_sb, xi)
            W1T_sb = cpool.tile([128, NCH, H], F16)
            dma_late(W1T_sb, W1T.rearrange("(c p) h -> p c h", p=128))
            W2T_sb = cpool.tile([128, NCH, H], F16)
            dma_late(W2T_sb, W2T.rearrange("(c p) h -> p c h", p=128))

            eps_col = cpool.tile([IPC, 1], F32)
            nc.vector.memset(eps_col, EPS)

            # ---- sa (this core's rows) and sb (all rows) ----
            scr_sa = smallp.tile([IPC, D], F16)
            nc.vector.tensor_mul(scr_sa, xi16_sb, wa_b)
            sa_col = smallp.tile([IPC, 1], F32)
            nc.vector.tensor_reduce(sa_col, scr_sa, axis=AX.X, op=OP.add)
            nc.vector.tensor_add(sa_col, sa_col, b_att_col)
            sa_diag = smallp.tile([IPC, IPC], F32)
            nc.vector.tensor_mul(sa_diag, id_sb[0:IPC, 0:IPC],
                                 sa_col.to_broadcast([IPC, IPC]))

            sb_cols = smallp.tile([128, NCH], F32)
            for c in range(NCH):
                scr_sb = smallp.tile([128, D], F16)
                nc.vector.tensor_mul(scr_sb, xlo[:, c, :], wb_b)
                nc.vector.tensor_reduce(
                    sb_cols[:, c:c + 1], scr_sb, axis=AX.X, op=OP.add)

            # sb_hi[p, c] = sb[c*128+p+1] via shift matmuls; slot 511 stays 0
            sbhi_ps = pss.tile([128, NCH], F32, tag="ps_small")
            nc.tensor.matmul(sbhi_ps, sh1_sb, sb_cols, start=True, stop=False)
            nc.tensor.matmul(sbhi_ps[:, 0:NCH - 1], sh2_sb, sb_cols[:, 1:NCH],
                             start=False, stop=True)
            sbhi_cols = smallp.tile([128, NCH], F32)
            nc.vector.tensor_copy(sbhi_cols, sbhi_ps)

            # ---- SBJ[t, i] = sa[i] + b_att + sb_hi[t] + mask_lt*(sb_lo-sb_hi),
            #      with -1e9 poison at the t=511 pad slot ----
            sbj = cpool.tile([128, NCH, IPC], F32)
            for c in range(NCH):
                diffc = smallp.tile([128, 1], F32)
                nc.vector.tensor_tensor(
                    diffc, sb_cols[:, c:c + 1], sbhi_cols[:, c:c + 1], OP.subtract)
                diagc = smallp.tile([128, 128], F32)
                nc.vector.tensor_mul(diagc, id_sb, diffc.to_broadcast([128, 128]))
                diagb = smallp.tile([128, 128], F32)
                nc.vector.tensor_mul(
                    diagb, id_sb, sbhi_cols[:, c:c + 1].to_broadcast([128, 128]))
                ps_sbj = pss.tile([128, IPC], F32, tag="ps_small")
                nc.tensor.matmul(ps_sbj, ones_sb[0:IPC, :], sa_diag,
                                 start=True, stop=False)
                nc.tensor.matmul(ps_sbj, diagb, ones_sb[:, 0:IPC],
                                 start=False, stop=False)
                if c == NCH - 1:
                    # poison: sigmoid(-1e9) = 0 exactly, pad row drops out
                    nc.tensor.matmul(ps_sbj, poison_sb, ones_sb[0:1, 0:IPC],
                                     start=False, stop=False)
                nc.tensor.matmul(ps_sbj, diagc, mlt_sb[:, c, :],
                                 start=False, stop=True)
                nc.vector.tensor_copy(sbj[:, c, :], ps_sbj)

            # ---- main edge pass ----
            alpha_full = cpool.tile([128, NCH, IPC], F16)   # raw sigmoid out
            age_full = cpool.tile([128, NCH, IPC], F16)     # masked (t>=i) alpha
            gx_ps = psgp.tile([IPC, D], F32)                # sum_t a*x[j]
            msg_ps = psmp.tile([IPC, H], F32)
            s_ps = pss.tile([IPC, 1], F32, tag="ps_small")
            # U quad rows: bank b, partition slot 32s..32s+3 holds i=16b+4s+j
            u_ps = [psflex.tile([128, 512], F32, tag="flex", name=f"u_ps{b}")
                    for b in range(4)]
            for b in range(4):
                nc.vector.memset(u_ps[b], 0.0)

            for c in range(NCH):
                for ib in range(NIB):
                    i0 = ib * IB
                    pf_t = pfp.tile([128, IB, PD], F16)
                    for hh in range(2):
                        dma(pf_t[:, 16 * hh:16 * (hh + 1), :],
                            pf[c, :, i0 + 16 * hh:i0 + 16 * (hh + 1), :])
                    # sc = sum_pd pf_sent (wc pre-folded): fp16 halving tree
                    scr = scrp.tile([128, IB, 64], F16)
                    nc.vector.tensor_add(scr, pf_t[:, :, 0:64], pf_t[:, :, 64:128])
                    w = 32
                    while w >= 2:
                        nc.vector.tensor_add(
                            scr[:, :, 0:w], scr[:, :, 0:w], scr[:, :, w:2 * w])
                        w //= 2
                    sc_t = smallp.tile([128, IB], F32)
                    nc.vector.tensor_add(sc_t, scr[:, :, 0], scr[:, :, 1])
                    aarg = smallp.tile([128, IB], F32)
                    nc.vector.tensor_add(aarg, sc_t, sbj[:, c, i0:i0 + IB])
                    nc.scalar.activation(
                        alpha_full[:, c, i0:i0 + IB], aarg, AF.Sigmoid)
                    nc.vector.tensor_mul(
                        age_full[:, c, i0:i0 + IB], alpha_full[:, c, i0:i0 + IB],
                        mge_sb[:, c, i0:i0 + IB])
                    # U quads: lhsT = 4 alpha columns, rhs = 4 pf blocks; the
                    # wanted rows sit on the diagonal (gathered via DRAM AP)
                    for q in range(IB // 4):
                        i = i0 + 4 * q
                        b, sp = divmod(i // 4, 4)
                        nc.tensor.matmul(
                            u_ps[b][32 * sp:32 * sp + 4, :],
                            alpha_full[:, c, i:i + 4],
                            pf_t[:, 4 * q:4 * q + 4, :],
                            start=(c == 0), stop=(c == NCH - 1),
                            tile_position=(0, 32 * sp))
                nc.tensor.matmul(gx_ps, alpha_full[:, c, :], xlo[:, c, :],
                                 start=(c == 0), stop=(c == NCH - 1 and False))
                nc.tensor.matmul(s_ps, alpha_full[:, c, :], ones16_sb[:, 0:1],
                                 start=(c == 0), stop=(c == NCH - 1))

            # G2 (shifted-x correction) after the loop: dx arrives on the slow
            # queue and age_full persists, so this overlaps the loop tail
            for c in range(NCH):
                nc.tensor.matmul(gx_ps, age_full[:, c, :], dx[:, c, :],
                                 start=False, stop=(c == NCH - 1))

            # ---- messages = U@WpT + Gx@WoT + s_alpha x bop ----
            s_col = smallp.tile([IPC, 1], F32)
            nc.vector.tensor_copy(s_col, s_ps)
            ps_sr = pss.tile([1, IPC], F32, tag="ps_small")
            nc.tensor.transpose(ps_sr, s_col, id_sb[0:IPC, 0:IPC])
            s_row = smallp.tile([1, IPC], F32)
            nc.vector.tensor_copy(s_row, ps_sr)

            # U reassembly: full-bank psum->sbuf copies, DRAM dump, then one
            # gather DMA picks the diagonal: row (b, s, j) -> i = 16b+4s+j at
            # element offset 65536b + 16384s + 640j (+pd).
            for b in range(4):
                u_cp = postp.tile([128, 512], F32, tag="u_cp")
                nc.vector.tensor_copy(u_cp, u_ps[b])
                # keep the whole u_dram bounce on one queue: Tile does not
                # track ordering through raw DRAM tensors, same-queue FIFO does
                nc.sync.dma_start(
                    out=u_dram[b * 65536:(b + 1) * 65536].rearrange(
                        "(p f) -> p f", p=128), in_=u_cp)
            u_sb64 = postp.tile([IPC, PD], F32)
            gather = bass.AP(tensor=u_dram.tensor, offset=0,
                             ap=[[65536, 4], [16384, 4], [640, 4], [1, 128]])
            nc.sync.dma_start(out=u_sb64, in_=gather)
            ps_ut = pss.tile([128, IPC], F32, tag="ps_small")
            nc.tensor.transpose(ps_ut, u_sb64, id_sb[0:IPC, 0:IPC])
            u_sb = postp.tile([128, IPC], F16)
            nc.vector.tensor_copy(u_sb, ps_ut)

            gx_sb = postp.tile([IPC, D], F32)
            nc.vector.tensor_copy(gx_sb, gx_ps)
            gxT = postp.tile([128, NCH, IPC], F16)
            for c in range(NCH):
                ptg = pss.tile([128, IPC], F32, tag="ps_small")
                nc.tensor.transpose(ptg, gx_sb[:, c * 128:(c + 1) * 128],
                                    id_sb[0:IPC, 0:IPC])
                nc.vector.tensor_copy(gxT[:, c, :], ptg)

            nc.tensor.matmul(msg_ps, u_sb, WpT_sb, start=True, stop=False)
            for c in range(NCH):
                nc.tensor.matmul(msg_ps, gxT[:, c, :], WoT_sb[:, c, :],
                                 start=False, stop=False)
            nc.tensor.matmul(msg_ps, s_row, bop_row, start=False, stop=True)

            # ---- residual + LN1 ----
            def layer_norm(v):
                stats = smallp.tile([IPC, 6], F32)
                nc.vector.bn_stats(out=stats, in_=v)
                mv = smallp.tile([IPC, 2], F32)
                nc.vector.bn_aggr(out=mv, in_=stats)
                std = smallp.tile([IPC, 1], F32)
                nc.scalar.activation(std, mv[:, 1:2], AF.Sqrt, bias=eps_col)
                rstd = smallp.tile([IPC, 1], F32)
                nc.vector.reciprocal(rstd, std)
                cen = postp.tile([IPC, H], F32)
                nc.vector.tensor_scalar(cen, v, mv[:, 0:1], rstd,
                                        OP.subtract, OP.mult)
                o = postp.tile([IPC, H], F32)
                nc.vector.tensor_mul(o, cen, gb_sb)
                nc.vector.tensor_add(o, o, bb_sb)
                return o

            h_sb = postp.tile([IPC, H], F32)
            nc.vector.tensor_add(h_sb, xi_sb, msg_ps)
            out1 = layer_norm(h_sb)

            # ---- FFN ----
            def transpose_rows(v):
                vT = postp.tile([128, NCH, IPC], F16, tag="vT")
                for c in range(NCH):
                    ptt = pss.tile([128, IPC], F32, tag="ps_small")
                    nc.tensor.transpose(ptt, v[:, c * 128:(c + 1) * 128],
                                        id_sb[0:IPC, 0:IPC])
                    nc.vector.tensor_copy(vT[:, c, :], ptt)
                return vT

            o1T = transpose_rows(out1)
            f1_ps = psflex.tile([IPC, H], F32, tag="flex")
            for c in range(NCH):
                nc.tensor.matmul(f1_ps, o1T[:, c, :], W1T_sb[:, c, :],
                                 start=(c == 0), stop=(c == NCH - 1))
            f1 = postp.tile([IPC, H], F32)
            nc.vector.tensor_add(f1, f1_ps, b1_sb)
            nc.vector.tensor_scalar_max(f1, f1, 0.0)

            f1T = transpose_rows(f1)
            f2_ps = psflex.tile([IPC, H], F32, tag="flex")
            for c in range(NCH):
                nc.tensor.matmul(f2_ps, f1T[:, c, :], W2T_sb[:, c, :],
                                 start=(c == 0), stop=(c == NCH - 1))
            h2 = postp.tile([IPC, H], F32)
            nc.vector.tensor_add(h2, f2_ps, b2_sb)
            nc.vector.tensor_add(h2, h2, out1)
            out2 = layer_norm(h2)

            nc.sync.dma_start(out=out_d, in_=out2)

    return nc


def _poison() -> np.ndarray:
    p = np.zeros((1, 128), np.float32)
    p[0, 127] = -1e9
    return p


def _cmat() -> np.ndarray:
    c = np.zeros((128, 4, 128), np.float32)
    c[:, 0, :] = np.eye(128)
    c[:, 1, :] = 1.0
    c[:, 2, :] = np.eye(128, k=-1)     # shift1[q, p] = (q == p+1)
    c[0, 3, 127] = 1.0                  # shift2[q, p] = (q==0)&(p==127)
    return c


def prep_in_maps(inputs) -> list[dict]:
    x = np.asarray(inputs["x"], np.float32)
    pf = np.asarray(inputs["pair_feats"], np.float32)
    W_att = np.asarray(inputs["W_att"], np.float32)
    b_att = np.asarray(inputs["b_att"], np.float32)
    W_obj = np.asarray(inputs["W_obj"], np.float32)
    b_obj = np.asarray(inputs["b_obj"], np.float32)
    W_pair = np.asarray(inputs["W_pair"], np.float32)
    b_pair = np.asarray(inputs["b_pair"], np.float32)
    ln_g = np.asarray(inputs["ln_g"], np.float32)
    ln_b = np.asarray(inputs["ln_b"], np.float32)
    W1 = np.asarray(inputs["W1"], np.float32)
    b1 = np.asarray(inputs["b1"], np.float32)
    W2 = np.asarray(inputs["W2"], np.float32)
    b2 = np.asarray(inputs["b2"], np.float32)

    wa, wb, wc = W_att[0, :D], W_att[0, D:2 * D], W_att[0, 2 * D:]
    xpad = np.concatenate([x, np.zeros((1, D), np.float32)], axis=0)

    # fold wc into pf columns; recover U via pre-divided W_pair.T rows.
    colscale = np.sign(wc) * np.maximum(np.abs(wc), 6e-5)
    colscale[colscale == 0] = 6e-5
    # 1/511 (the mean over neighbors) is folded into the three weight paths
    # that consume raw alpha: U@WpT, (A@x)@WoT, and s_alpha*bop.
    WpT2 = (W_pair.T / colscale[:, None] / T).astype(np.float16)
    WoT2 = (W_obj.T / T).astype(np.float16)
    dxf = np.diff(xpad[:K + 1], axis=0)

    base = dict(
        xf=xpad.astype(np.float16),
        dxf=dxf.astype(np.float16),
        cmat=_cmat(),
        ones16=np.ones((128, 8), np.float16),
        poison=_poison(),
        wab=np.stack([wa, wb]).astype(np.float16),
        b_att=b_att.astype(np.float32),
        bias5=np.stack([ln_g, ln_b, b1, b2,
                        (b_obj + b_pair) / T]).astype(np.float32),
        WpT=np.ascontiguousarray(WpT2),
        WoT=np.ascontiguousarray(WoT2),
        W1T=np.ascontiguousarray(W1.T).astype(np.float16),
        W2T=np.ascontiguousarray(W2.T).astype(np.float16),
    )

    pfr = pf.reshape(K, T, PD)
    tgrid = np.arange(128)[:, None] + 128 * np.arange(NCH)[None, :]   # [128, NCH]

    in_maps = []
    for core in range(NCORES):
        ig = np.arange(core * IPC, (core + 1) * IPC)
        mlt = (tgrid[:, :, None] < ig[None, None, :]).astype(np.float32)
        mge = ((tgrid[:, :, None] >= ig[None, None, :])
               & (tgrid[:, :, None] <= T - 1)).astype(np.float16)
        # [chunk, t, i, pd] layout -> each tile DMA is one contiguous burst
        shard = np.zeros((NCH * 128, IPC, PD), np.float16)
        shard[:T] = (pfr[ig] * colscale[None, None, :]).transpose(1, 0, 2)
        xi = x[ig]
        m = dict(base)
        m.update(
            pf=shard.reshape(NCH, 128, IPC, PD),
            xi=xi.astype(np.float32),
            xi16=xi.astype(np.float16),
            mask_lt=mlt,
            mask_ge=mge,
        )
        in_maps.append(m)
    return in_maps


_COMPILED = None


def _get_program() -> bacc.Bacc:
    global _COMPILED
    if _COMPILED is None:
        nc = build_program()
        nc.compile()
        _COMPILED = nc
    return _COMPILED


TRACE = False
LAST_RESULT = None


def _install_axon_ntff_hook():
    """The container's antenv lacks axon_hooks; recreate it from trn_boot's
    ctypes implementation so trace=True can capture NTFF profiles."""
    import sys
    import types
    try:
        from antenv.axon_hooks import get_axon_ntff_profile_hook  # noqa: F401
        return
    except ImportError:
        pass
    from trn_agent_boot.trn_boot import _ntff_profile_via_ctypes
    hook = _ntff_profile_via_ctypes("/opt/axon/libaxon_pjrt.so")
    m = types.ModuleType("antenv.axon_hooks")
    m.get_axon_ntff_profile_hook = lambda: hook
    sys.modules["antenv.axon_hooks"] = m


def kernel(**inputs) -> np.ndarray:
    import concourse.bass_utils as bu
    from concourse.bass_utils import run_bass_kernel_spmd
    global LAST_RESULT
    if TRACE:
        _install_axon_ntff_hook()
        bu.upload_artifacts = lambda tmpdir: str(tmpdir)  # no bucket here
    nc = _get_program()
    in_maps = prep_in_maps(inputs)
    res = run_bass_kernel_spmd(nc, in_maps, list(range(NCORES)), trace=TRACE)
    LAST_RESULT = res
    outs = [res.results[c]["out"] for c in range(NCORES)]
    return np.concatenate(outs, axis=0).astype(np.float32)


# revision 37
# speedup vs baseline: 1.2205x; 1.0259x over previous
"""Trainium2 Bass kernel for nn_ARTLayer (gnn_message_passing).

Math (reference):
    j(i,t) = t + (t>=i)                                    # [K, K-1] neighbor index
    alpha  = sigmoid(x@wa [i] + x@wb [j] + pf@wc + b_att)  # [K, K-1]
    msgs   = mean_t alpha * ((x@WobjT + b_obj)[j] + pf@WpairT + b_pair)
    out    = LN(x + msgs); out = LN(out + FFN(out))

Key algebraic rewrite (removes the 34-GFLOP [P,PD]x[PD,H] einsum):
    sum_t a*(pf@WpT)  = (sum_t a*pf) @ WpT               -> U[i,:] @ WpT
    sum_t a*oj[j]     = (A @ x) @ WobjT                  -> Gx[i,:] @ WoT
      with A[i,j] decomposed via lo/hi shifted views of x and a t>=i mask
    sum_t a*(b_obj+b_pair) = s_alpha[i] * bop

Sharding: rows i split across 8 cores (64 each); small tensors replicated;
host concatenates the per-core [64, 512] outputs.

Implementation notes (driven by NTFF profiles):
  - wc is folded into pf on the host (column scales, floored at fp16
    min-normal); sc becomes a pure fp16 halving-tree add-reduce and U is
    recovered exactly via W_pairT rows pre-divided by the scales.
  - 1/511 (the neighbor mean) is folded into WpT/WoT/bop on the host, and
    the t=511 pad slot is poisoned with -1e9 pre-sigmoid, so raw sigmoid
    output is used directly with no mask/scale multiplies.
  - U accumulation runs as M=4 quad matmuls (alpha quad stationary, four pf
    blocks streaming at N=512); the wanted rows sit on the block diagonal
    and are gathered by a stride-640 DRAM access pattern after a bank dump.
  - pf is re-laid-out on the host to [chunk, t, i, pd] so each tile DMA is
    one fully-contiguous 8KB-per-partition burst, and all DMAs are spread
    round-robin over the three DMA-capable engines (sync/scalar HWDGE,
    gpsimd SWDGE) instead of serializing on one queue.
"""
import numpy as np

import concourse.bass as bass
import concourse.tile as tile
from concourse import bacc, mybir

F32, F16 = mybir.dt.float32, mybir.dt.float16
AX = mybir.AxisListType
OP = mybir.AluOpType
AF = mybir.ActivationFunctionType

K, D, H, PD = 512, 512, 512, 128
T = K - 1                      # 511 neighbors per row
NCORES, IPC = 8, 64            # rows per core
NCH = 4                        # t-chunks of 128 (last chunk row 127 is t=511 pad)
IB, NIB = 32, 2                # i-block within a core
EPS = 1e-5


def build_program() -> bacc.Bacc:
    nc = bacc.Bacc("TRN2", target_bir_lowering=False, debug=False)

    def inp(name, shape, dt):
        return nc.dram_tensor(name, shape, dt, kind="ExternalInput").ap()

    pf = inp("pf", [NCH, 128, IPC, PD], F16)    # [chunk, t-in-chunk, i, pd]
    xf = inp("xf", [K + 1, D], F16)             # x with one zero pad row
    dxf = inp("dxf", [K, D], F16)               # x[t+1] - x[t], host computed
    xi = inp("xi", [IPC, D], F32)               # this core's rows of x
    xi16 = inp("xi16", [IPC, D], F16)
    mask_lt = inp("mask_lt", [128, NCH, IPC], F32)   # t <  i_global
    mask_ge = inp("mask_ge", [128, NCH, IPC], F16)   # t >= i_global and t <= 510
    cmat = inp("cmat", [128, 4, 128], F32)      # [ident | ones | shift1 | shift2]
    ones16 = inp("ones16", [128, 8], F16)
    poison = inp("poison", [1, 128], F32)       # -1e9 at slot 127, else 0
    wab = inp("wab", [2, D], F16)               # [wa; wb]
    b_att = inp("b_att", [1], F32)
    bias5 = inp("bias5", [5, H], F32)           # [ln_g; ln_b; b1; b2; bop/511]
    WpT = inp("WpT", [PD, H], F16)              # W_pair.T / colscale / 511
    WoT = inp("WoT", [D, H], F16)               # W_obj.T / 511
    W1T = inp("W1T", [H, H], F16)
    W2T = inp("W2T", [H, H], F16)

    out_d = nc.dram_tensor("out", [IPC, H], F32, kind="ExternalOutput").ap()
    u_dram = nc.dram_tensor("u_dram", [4 * 128 * 512], F32).ap()  # U bank dumps

    with tile.TileContext(nc) as tc:
        with (
            tc.tile_pool(name="const", bufs=1) as cpool,
            tc.tile_pool(name="pfp", bufs=4) as pfp,
            tc.tile_pool(name="scrp", bufs=2) as scrp,
            tc.tile_pool(name="smallp", bufs=4) as smallp,
            tc.tile_pool(name="postp", bufs=2) as postp,
            tc.tile_pool(name="pss", bufs=2, space="PSUM") as pss,
            tc.tile_pool(name="psflex", bufs=4, space="PSUM") as psflex,
            tc.tile_pool(name="psmp", bufs=1, space="PSUM") as psmp,
            tc.tile_pool(name="psgp", bufs=1, space="PSUM") as psgp,
        ):
            # DMA policy: critical-path loads alternate the two HWDGE
            # queues (sync/scalar); bulky tail-only weights go to the gpsimd
            # SWDGE queue and are emitted at their use sites so the scheduler
            # does not front-load them ahead of pf tiles.
            qi = [0]

            def dma(out, in_):
                eng = nc.sync if qi[0] % 2 == 0 else nc.scalar
                qi[0] += 1
                eng.dma_start(out=out, in_=in_)

            def dma_late(out, in_):
                nc.gpsimd.dma_start(out=out, in_=in_)

            # ---- constants & weights to SBUF ----
            hp = tc.high_priority()
            hp.__enter__()
            cm_sb = cpool.tile([128, 4, 128], F32)
            dma(cm_sb, cmat)
            id_sb = cm_sb[:, 0, :]
            ones_sb = cm_sb[:, 1, :]
            sh1_sb = cm_sb[:, 2, :]
            sh2_sb = cm_sb[:, 3, :]
            wab_sb = cpool.tile([128, 2, D], F16)
            dma(wab_sb, wab[None, :, :].to_broadcast([128, 2, D]))
            wa_b = wab_sb[0:IPC, 0, :]
            wb_b = wab_sb[:, 1, :]
            bias_sb = cpool.tile([IPC, 5, H], F32)
            dma_late(bias_sb, bias5[None, :, :].to_broadcast([IPC, 5, H]))
            gb_sb = bias_sb[:, 0, :]
            bb_sb = bias_sb[:, 1, :]
            b1_sb = bias_sb[:, 2, :]
            b2_sb = bias_sb[:, 3, :]
            bop_row = bias_sb[0:1, 4, :]
            ones16_sb = cpool.tile([128, 8], F16)
            dma(ones16_sb, ones16)
            poison_sb = cpool.tile([1, 128], F32)
            dma(poison_sb, poison)
            mlt_sb = cpool.tile([128, NCH, IPC], F32)
            dma(mlt_sb, mask_lt)
            mge_sb = cpool.tile([128, NCH, IPC], F16)
            dma(mge_sb, mask_ge)
            xi16_sb = cpool.tile([IPC, D], F16)
            dma(xi16_sb, xi16)
            xlo = cpool.tile([128, NCH, D], F16)
            dma(xlo, xf[0:K].rearrange("(c p) d -> p c d", p=128))
            b_att_col = cpool.tile([IPC, 1], F32)
            dma(b_att_col, b_att[None, :].to_broadcast([IPC, 1]))
            hp.__exit__(None, None, None)
            # tail-only loads on the slow queue, in rough use order
            dx = cpool.tile([128, NCH, D], F16)
            dma_late(dx, dxf.rearrange("(c p) d -> p c d", p=128))
            WpT_sb = cpool.tile([128, H], F16)
            dma_late(WpT_sb, WpT)
            WoT_sb = cpool.tile([128, NCH, H], F16)
            dma_late(WoT_sb, WoT.rearrange("(c p) h -> p c h", p=128))
            xi_sb = cpool.tile([IPC, D], F32)
            dma_late(xi_sb, xi)
            W1T_sb = cpool.tile([128, NCH, H], F16)
            dma_late(W1T_sb, W1T.rearrange("(c p) h -> p c h", p=128))
            W2T_sb = cpool.tile([128, NCH, H], F16)
            dma_late(W2T_sb, W2T.rearrange("(c p) h -> p c h", p=128))

            eps_col = cpool.tile([IPC, 1], F32)
            nc.vector.memset(eps_col, EPS)

            # ---- sa (this core's rows) and sb (all rows) ----
            scr_sa = smallp.tile([IPC, D], F16)
            nc.vector.tensor_mul(scr_sa, xi16_sb, wa_b)
            sa_col = smallp.tile([IPC, 1], F32)
            nc.vector.tensor_reduce(sa_col, scr_sa, axis=AX.X, op=OP.add)
            nc.vector.tensor_add(sa_col, sa_col, b_att_col)
            sa_diag = smallp.tile([IPC, IPC], F32)
            nc.vector.tensor_mul(sa_diag, id_sb[0:IPC, 0:IPC],
                                 sa_col.to_broadcast([IPC, IPC]))

            sb_cols = smallp.tile([128, NCH], F32)
            for c in range(NCH):
                scr_sb = smallp.tile([128, D], F16)
                nc.vector.tensor_mul(scr_sb, xlo[:, c, :], wb_b)
                nc.vector.tensor_reduce(
                    sb_cols[:, c:c + 1], scr_sb, axis=AX.X, op=OP.add)

            # sb_hi[p, c] = sb[c*128+p+1] via shift matmuls; slot 511 stays 0
            sbhi_ps = pss.tile([128, NCH], F32, tag="ps_small")
            nc.tensor.matmul(sbhi_ps, sh1_sb, sb_cols, start=True, stop=False)
            nc.tensor.matmul(sbhi_ps[:, 0:NCH - 1], sh2_sb, sb_cols[:, 1:NCH],
                             start=False, stop=True)
            sbhi_cols = smallp.tile([128, NCH], F32)
            nc.vector.tensor_copy(sbhi_cols, sbhi_ps)

            # ---- SBJ[t, i] = sa[i] + b_att + sb_hi[t] + mask_lt*(sb_lo-sb_hi),
            #      with -1e9 poison at the t=511 pad slot ----
            sbj = cpool.tile([128, NCH, IPC], F32)
            for c in range(NCH):
                diffc = smallp.tile([128, 1], F32)
                nc.vector.tensor_tensor(
                    diffc, sb_cols[:, c:c + 1], sbhi_cols[:, c:c + 1], OP.subtract)
                diagc = smallp.tile([128, 128], F32)
                nc.vector.tensor_mul(diagc, id_sb, diffc.to_broadcast([128, 128]))
                diagb = smallp.tile([128, 128], F32)
                nc.vector.tensor_mul(
                    diagb, id_sb, sbhi_cols[:, c:c + 1].to_broadcast([128, 128]))
                ps_sbj = pss.tile([128, IPC], F32, tag="ps_small")
                nc.tensor.matmul(ps_sbj, ones_sb[0:IPC, :], sa_diag,
                                 start=True, stop=False)
                nc.tensor.matmul(ps_sbj, diagb, ones_sb[:, 0:IPC],
                                 start=False, stop=False)
                if c == NCH - 1:
                    # poison: sigmoid(-1e9) = 0 exactly, pad row drops out
                    nc.tensor.matmul(ps_sbj, poison_sb, ones_sb[0:1, 0:IPC],
                                     start=False, stop=False)
                nc.tensor.matmul(ps_sbj, diagc, mlt_sb[:, c, :],
                                 start=False, stop=True)
                nc.vector.tensor_copy(sbj[:, c, :], ps_sbj)

            # ---- main edge pass ----
            alpha_full = cpool.tile([128, NCH, IPC], F16)   # raw sigmoid out
            age_full = cpool.tile([128, NCH, IPC], F16)     # masked (t>=i) alpha
            gx_ps = psgp.tile([IPC, D], F32)                # sum_t a*x[j]
            msg_ps = psmp.tile([IPC, H], F32)
            s_ps = pss.tile([IPC, 1], F32, tag="ps_small")
            # U quad rows: bank b, partition slot 32s..32s+3 holds i=16b+4s+j
            u_ps = [psflex.tile([128, 512], F32, tag="flex", name=f"u_ps{b}")
                    for b in range(4)]
            for b in range(4):
                nc.vector.memset(u_ps[b], 0.0)

            for c in range(NCH):
                for ib in range(NIB):
                    i0 = ib * IB
                    pf_t = pfp.tile([128, IB, PD], F16)
                    for hh in range(2):
                        dma(pf_t[:, 16 * hh:16 * (hh + 1), :],
                            pf[c, :, i0 + 16 * hh:i0 + 16 * (hh + 1), :])
                    # sc = sum_pd pf_sent (wc pre-folded): fp16 halving tree
                    scr = scrp.tile([128, IB, 64], F16)
                    nc.vector.tensor_add(scr, pf_t[:, :, 0:64], pf_t[:, :, 64:128])
                    w = 32
                    while w >= 2:
                        nc.vector.tensor_add(
                            scr[:, :, 0:w], scr[:, :, 0:w], scr[:, :, w:2 * w])
                        w //= 2
                    sc_t = smallp.tile([128, IB], F32)
                    nc.vector.tensor_add(sc_t, scr[:, :, 0], scr[:, :, 1])
                    aarg = smallp.tile([128, IB], F32)
                    nc.vector.tensor_add(aarg, sc_t, sbj[:, c, i0:i0 + IB])
                    nc.scalar.activation(
                        alpha_full[:, c, i0:i0 + IB], aarg, AF.Sigmoid)
                    nc.vector.tensor_mul(
                        age_full[:, c, i0:i0 + IB], alpha_full[:, c, i0:i0 + IB],
                        mge_sb[:, c, i0:i0 + IB])
                    # U quads: lhsT = 4 alpha columns, rhs = 4 pf blocks; the
                    # wanted rows sit on the diagonal (gathered via DRAM AP)
                    for q in range(IB // 4):
                        i = i0 + 4 * q
                        b, sp = divmod(i // 4, 4)
                        nc.tensor.matmul(
                            u_ps[b][32 * sp:32 * sp + 4, :],
                            alpha_full[:, c, i:i + 4],
                            pf_t[:, 4 * q:4 * q + 4, :],
                            start=(c == 0), stop=(c == NCH - 1),
                            tile_position=(0, 32 * sp))
                nc.tensor.matmul(gx_ps, alpha_full[:, c, :], xlo[:, c, :],
                                 start=(c == 0), stop=(c == NCH - 1 and False))
                nc.tensor.matmul(s_ps, alpha_full[:, c, :], ones16_sb[:, 0:1],
                                 start=(c == 0), stop=(c == NCH - 1))

            # G2 (shifted-x correction) after the loop: dx arrives on the slow
            # queue and age_full persists, so this overlaps the loop tail
            for c in range(NCH):
                nc.tensor.matmul(gx_ps, age_full[:, c, :], dx[:, c, :],
                                 start=False, stop=(c == NCH - 1))

            # ---- messages = U@WpT + Gx@WoT + s_alpha x bop ----
            s_col = smallp.tile([IPC, 1], F32)
            nc.vector.tensor_copy(s_col, s_ps)
            ps_sr = pss.tile([1, IPC], F32, tag="ps_small")
            nc.tensor.transpose(ps_sr, s_col, id_sb[0:IPC, 0:IPC])
            s_row = smallp.tile([1, IPC], F32)
            nc.vector.tensor_copy(s_row, ps_sr)

            # U reassembly: full-bank psum->sbuf copies, DRAM dump, then one
            # gather DMA picks the diagonal: row (b, s, j) -> i = 16b+4s+j at
            # element offset 65536b + 16384s + 640j (+pd).
            for b in range(4):
                u_cp = postp.tile([128, 512], F32, tag="u_cp")
                nc.vector.tensor_copy(u_cp, u_ps[b])
                # keep the whole u_dram bounce on one queue: Tile does not
                # track ordering through raw DRAM tensors, same-queue FIFO does
                nc.sync.dma_start(
                    out=u_dram[b * 65536:(b + 1) * 65536].rearrange(
                        "(p f) -> p f", p=128), in_=u_cp)
            u_sb64 = postp.tile([IPC, PD], F32)
            gather = bass.AP(tensor=u_dram.tensor, offset=0,
                             ap=[[65536, 4], [16384, 4], [640, 4], [1, 128]])
            nc.sync.dma_start(out=u_sb64, in_=gather)
            ps_ut = pss.tile([128, IPC], F32, tag="ps_small")
            nc.tensor.transpose(ps_ut, u_sb64, id_sb[0:IPC, 0:IPC])
            u_sb = postp.tile([128, IPC], F16)
            nc.vector.tensor_copy(u_sb, ps_ut)

            gx_sb = postp.tile([IPC, D], F32)
            nc.vector.tensor_copy(gx_sb, gx_ps)
            gxT = postp.tile([128, NCH, IPC], F16)
            for c in range(NCH):
                ptg = pss.tile([128, IPC], F32, tag="ps_small")
                nc.tensor.transpose(ptg, gx_sb[:, c * 128:(c + 1) * 128],
                                    id_sb[0:IPC, 0:IPC])
                nc.vector.tensor_copy(gxT[:, c, :], ptg)

            nc.tensor.matmul(msg_ps, u_sb, WpT_sb, start=True, stop=False)
            for c in range(NCH):
                nc.tensor.matmul(msg_ps, gxT[:, c, :], WoT_sb[:, c, :],
                                 start=False, stop=False)
            nc.tensor.matmul(msg_ps, s_row, bop_row, start=False, stop=True)

            # ---- residual + LN1 ----
            def layer_norm(v):
                stats = smallp.tile([IPC, 6], F32)
                nc.vector.bn_stats(out=stats, in_=v)
                mv = smallp.tile([IPC, 2], F32)
                nc.vector.bn_aggr(out=mv, in_=stats)
                std = smallp.tile([IPC, 1], F32)
                nc.scalar.activation(std, mv[:, 1:2], AF.Sqrt, bias=eps_col)
                rstd = smallp.tile([IPC, 1], F32)
                nc.vector.reciprocal(rstd, std)
                cen = postp.tile([IPC, H], F32)
                nc.vector.tensor_scalar(cen, v, mv[:, 0:1], rstd,
                                        OP.subtract, OP.mult)
                o = postp.tile([IPC, H], F32)
                nc.vector.tensor_mul(o, cen, gb_sb)
                nc.vector.tensor_add(o, o, bb_sb)
                return o

            h_sb = postp.tile([IPC, H], F32)
            nc.vector.tensor_add(h_sb, xi_sb, msg_ps)
            out1 = layer_norm(h_sb)

            # ---- FFN ----
            def transpose_rows(v):
                vT = postp.tile([128, NCH, IPC], F16, tag="vT")
                for c in range(NCH):
                    ptt = pss.tile([128, IPC], F32, tag="ps_small")
                    nc.tensor.transpose(ptt, v[:, c * 128:(c + 1) * 128],
                                        id_sb[0:IPC, 0:IPC])
                    nc.vector.tensor_copy(vT[:, c, :], ptt)
                return vT

            o1T = transpose_rows(out1)
            f1_ps = psflex.tile([IPC, H], F32, tag="flex")
            for c in range(NCH):
                nc.tensor.matmul(f1_ps, o1T[:, c, :], W1T_sb[:, c, :],
                                 start=(c == 0), stop=(c == NCH - 1))
            f1 = postp.tile([IPC, H], F32)
            nc.vector.tensor_add(f1, f1_ps, b1_sb)
            nc.vector.tensor_scalar_max(f1, f1, 0.0)

            f1T = transpose_rows(f1)
            f2_ps = psflex.tile([IPC, H], F32, tag="flex")
            for c in range(NCH):
                nc.tensor.matmul(f2_ps, f1T[:, c, :], W2T_sb[:, c, :],
                                 start=(c == 0), stop=(c == NCH - 1))
            h2 = postp.tile([IPC, H], F32)
            nc.vector.tensor_add(h2, f2_ps, b2_sb)
            nc.vector.tensor_add(h2, h2, out1)
            out2 = layer_norm(h2)

            nc.sync.dma_start(out=out_d, in_=out2)

    return nc


def _poison() -> np.ndarray:
    p = np.zeros((1, 128), np.float32)
    p[0, 127] = -1e9
    return p


def _cmat() -> np.ndarray:
    c = np.zeros((128, 4, 128), np.float32)
    c[:, 0, :] = np.eye(128)
    c[:, 1, :] = 1.0
    c[:, 2, :] = np.eye(128, k=-1)     # shift1[q, p] = (q == p+1)
    c[0, 3, 127] = 1.0                  # shift2[q, p] = (q==0)&(p==127)
    return c


def prep_in_maps(inputs) -> list[dict]:
    x = np.asarray(inputs["x"], np.float32)
    pf = np.asarray(inputs["pair_feats"], np.float32)
    W_att = np.asarray(inputs["W_att"], np.float32)
    b_att = np.asarray(inputs["b_att"], np.float32)
    W_obj = np.asarray(inputs["W_obj"], np.float32)
    b_obj = np.asarray(inputs["b_obj"], np.float32)
    W_pair = np.asarray(inputs["W_pair"], np.float32)
    b_pair = np.asarray(inputs["b_pair"], np.float32)
    ln_g = np.asarray(inputs["ln_g"], np.float32)
    ln_b = np.asarray(inputs["ln_b"], np.float32)
    W1 = np.asarray(inputs["W1"], np.float32)
    b1 = np.asarray(inputs["b1"], np.float32)
    W2 = np.asarray(inputs["W2"], np.float32)
    b2 = np.asarray(inputs["b2"], np.float32)

    wa, wb, wc = W_att[0, :D], W_att[0, D:2 * D], W_att[0, 2 * D:]
    xpad = np.concatenate([x, np.zeros((1, D), np.float32)], axis=0)

    # fold wc into pf columns; recover U via pre-divided W_pair.T rows.
    colscale = np.sign(wc) * np.maximum(np.abs(wc), 6e-5)
    colscale[colscale == 0] = 6e-5
    # 1/511 (the mean over neighbors) is folded into the three weight paths
    # that consume raw alpha: U@WpT, (A@x)@WoT, and s_alpha*bop.
    WpT2 = (W_pair.T / colscale[:, None] / T).astype(np.float16)
    WoT2 = (W_obj.T / T).astype(np.float16)
    dxf = np.diff(xpad[:K + 1], axis=0)

    base = dict(
        xf=xpad.astype(np.float16),
        dxf=dxf.astype(np.float16),
        cmat=_cmat(),
        ones16=np.ones((128, 8), np.float16),
        poison=_poison(),
        wab=np.stack([wa, wb]).astype(np.float16),
        b_att=b_att.astype(np.float32),
        bias5=np.stack([ln_g, ln_b, b1, b2,
                        (b_obj + b_pair) / T]).astype(np.float32),
        WpT=np.ascontiguousarray(WpT2),
        WoT=np.ascontiguousarray(WoT2),
        W1T=np.ascontiguousarray(W1.T).astype(np.float16),
        W2T=np.ascontiguousarray(W2.T).astype(np.float16),
    )

    pfr = pf.reshape(K, T, PD)
    tgrid = np.arange(128)[:, None] + 128 * np.arange(NCH)[None, :]   # [128, NCH]

    in_maps = []
    for core in range(NCORES):
        ig = np.arange(core * IPC, (core + 1) * IPC)
        mlt = (tgrid[:, :, None] < ig[None, None, :]).astype(np.float32)
        mge = ((tgrid[:, :, None] >= ig[None, None, :])
               & (tgrid[:, :, None] <= T - 1)).astype(np.float16)
        # [chunk, t, i, pd] layout -> each tile DMA is one contiguous burst
        shard = np.zeros((NCH * 128, IPC, PD), np.float16)
        shard[:T] = (pfr[ig] * colscale[None, None, :]).transpose(1, 0, 2)
        xi = x[ig]
        m = dict(base)
        m.update(
            pf=shard.reshape(NCH, 128, IPC, PD),
            xi=xi.astype(np.float32),
            xi16=xi.astype(np.float16),
            mask_lt=mlt,
            mask_ge=mge,
        )
        in_maps.append(m)
    return in_maps


_COMPILED = None


def _get_program() -> bacc.Bacc:
    global _COMPILED
    if _COMPILED is None:
        nc = build_program()
        nc.compile()
        _COMPILED = nc
    return _COMPILED


TRACE = False
LAST_RESULT = None


def _install_axon_ntff_hook():
    """The container's antenv lacks axon_hooks; recreate it from trn_boot's
    ctypes implementation so trace=True can capture NTFF profiles."""
    import sys
    import types
    try:
        from antenv.axon_hooks import get_axon_ntff_profile_hook  # noqa: F401
        return
    except ImportError:
        pass
    from trn_agent_boot.trn_boot import _ntff_profile_via_ctypes
    hook = _ntff_profile_via_ctypes("/opt/axon/libaxon_pjrt.so")
    m = types.ModuleType("antenv.axon_hooks")
    m.get_axon_ntff_profile_hook = lambda: hook
    sys.modules["antenv.axon_hooks"] = m


def kernel(**inputs) -> np.ndarray:
    import concourse.bass_utils as bu
    from concourse.bass_utils import run_bass_kernel_spmd
    global LAST_RESULT
    if TRACE:
        _install_axon_ntff_hook()
        bu.upload_artifacts = lambda tmpdir: str(tmpdir)  # no bucket here
    nc = _get_program()
    in_maps = prep_in_maps(inputs)
    res = run_bass_kernel_spmd(nc, in_maps, list(range(NCORES)), trace=TRACE)
    LAST_RESULT = res
    outs = [res.results[c]["out"] for c in range(NCORES)]
    return np.concatenate(outs, axis=0).astype(np.float32)


# revision 38
# speedup vs baseline: 1.2324x; 1.0098x over previous
"""Trainium2 Bass kernel for nn_ARTLayer (gnn_message_passing).

Math (reference):
    j(i,t) = t + (t>=i)                                    # [K, K-1] neighbor index
    alpha  = sigmoid(x@wa [i] + x@wb [j] + pf@wc + b_att)  # [K, K-1]
    msgs   = mean_t alpha * ((x@WobjT + b_obj)[j] + pf@WpairT + b_pair)
    out    = LN(x + msgs); out = LN(out + FFN(out))

Key algebraic rewrite (removes the 34-GFLOP [P,PD]x[PD,H] einsum):
    sum_t a*(pf@WpT)  = (sum_t a*pf) @ WpT               -> U[i,:] @ WpT
    sum_t a*oj[j]     = (A @ x) @ WobjT                  -> Gx[i,:] @ WoT
      with A[i,j] decomposed via lo/hi shifted views of x and a t>=i mask
    sum_t a*(b_obj+b_pair) = s_alpha[i] * bop

Sharding: rows i split across 8 cores (64 each); small tensors replicated;
host concatenates the per-core [64, 512] outputs.

Implementation notes (driven by NTFF profiles):
  - wc is folded into pf on the host (column scales, floored at fp16
    min-normal); sc becomes a pure fp16 halving-tree add-reduce and U is
    recovered exactly via W_pairT rows pre-divided by the scales.
  - 1/511 (the neighbor mean) is folded into WpT/WoT/bop on the host, and
    the t=511 pad slot is poisoned with -1e9 pre-sigmoid, so raw sigmoid
    output is used directly with no mask/scale multiplies.
  - U accumulation runs as M=4 quad matmuls (alpha quad stationary, four pf
    blocks streaming at N=512); the wanted rows sit on the block diagonal
    and are gathered by a stride-640 DRAM access pattern after a bank dump.
  - pf is re-laid-out on the host to [chunk, t, i, pd] so each tile DMA is
    one fully-contiguous 8KB-per-partition burst, and all DMAs are spread
    round-robin over the three DMA-capable engines (sync/scalar HWDGE,
    gpsimd SWDGE) instead of serializing on one queue.
"""
import numpy as np

import concourse.bass as bass
import concourse.tile as tile
from concourse import bacc, mybir

F32, F16 = mybir.dt.float32, mybir.dt.float16
AX = mybir.AxisListType
OP = mybir.AluOpType
AF = mybir.ActivationFunctionType

K, D, H, PD = 512, 512, 512, 128
T = K - 1                      # 511 neighbors per row
NCORES, IPC = 8, 64            # rows per core
NCH = 4                        # t-chunks of 128 (last chunk row 127 is t=511 pad)
IB, NIB = 64, 1                # i-block within a core
EPS = 1e-5


def build_program() -> bacc.Bacc:
    nc = bacc.Bacc("TRN2", target_bir_lowering=False, debug=False)

    def inp(name, shape, dt):
        return nc.dram_tensor(name, shape, dt, kind="ExternalInput").ap()

    pf = inp("pf", [NCH, 128, IPC, PD], F16)    # [chunk, t-in-chunk, i, pd]
    xf = inp("xf", [K + 1, D], F16)             # x with one zero pad row
    dxf = inp("dxf", [K, D], F16)               # x[t+1] - x[t], host computed
    xi = inp("xi", [IPC, D], F32)               # this core's rows of x
    xi16 = inp("xi16", [IPC, D], F16)
    mask_lt = inp("mask_lt", [128, NCH, IPC], F32)   # t <  i_global
    mask_ge = inp("mask_ge", [128, NCH, IPC], F16)   # t >= i_global and t <= 510
    cmat = inp("cmat", [128, 4, 128], F32)      # [ident | ones | shift1 | shift2]
    ones16 = inp("ones16", [128, 8], F16)
    poison = inp("poison", [1, 128], F32)       # -1e9 at slot 127, else 0
    wab = inp("wab", [2, D], F16)               # [wa; wb]
    b_att = inp("b_att", [1], F32)
    bias5 = inp("bias5", [5, H], F32)           # [ln_g; ln_b; b1; b2; bop/511]
    WpT = inp("WpT", [PD, H], F16)              # W_pair.T / colscale / 511
    WoT = inp("WoT", [D, H], F16)               # W_obj.T / 511
    W1T = inp("W1T", [H, H], F16)
    W2T = inp("W2T", [H, H], F16)

    out_d = nc.dram_tensor("out", [IPC, H], F32, kind="ExternalOutput").ap()
    u_dram = nc.dram_tensor("u_dram", [4 * 128 * 512], F32).ap()  # U bank dumps

    with tile.TileContext(nc) as tc:
        with (
            tc.tile_pool(name="const", bufs=1) as cpool,
            tc.tile_pool(name="pfp", bufs=4) as pfp,
            tc.tile_pool(name="scrp", bufs=3) as scrp,
            tc.tile_pool(name="smallp", bufs=4) as smallp,
            tc.tile_pool(name="postp", bufs=2) as postp,
            tc.tile_pool(name="pss", bufs=2, space="PSUM") as pss,
            tc.tile_pool(name="psflex", bufs=4, space="PSUM") as psflex,
            tc.tile_pool(name="psmp", bufs=1, space="PSUM") as psmp,
            tc.tile_pool(name="psgp", bufs=1, space="PSUM") as psgp,
        ):
            # DMA policy: critical-path loads alternate the two HWDGE
            # queues (sync/scalar); bulky tail-only weights go to the gpsimd
            # SWDGE queue and are emitted at their use sites so the scheduler
            # does not front-load them ahead of pf tiles.
            qi = [0]

            def dma(out, in_):
                eng = nc.sync if qi[0] % 2 == 0 else nc.scalar
                qi[0] += 1
                eng.dma_start(out=out, in_=in_)

            def dma_late(out, in_):
                nc.gpsimd.dma_start(out=out, in_=in_)

            # ---- constants & weights to SBUF ----
            hp = tc.high_priority()
            hp.__enter__()
            cm_sb = cpool.tile([128, 4, 128], F32)
            dma(cm_sb, cmat)
            id_sb = cm_sb[:, 0, :]
            ones_sb = cm_sb[:, 1, :]
            sh1_sb = cm_sb[:, 2, :]
            sh2_sb = cm_sb[:, 3, :]
            wab_sb = cpool.tile([128, 2, D], F16)
            dma(wab_sb, wab[None, :, :].to_broadcast([128, 2, D]))
            wa_b = wab_sb[0:IPC, 0, :]
            wb_b = wab_sb[:, 1, :]
            bias_sb = cpool.tile([IPC, 5, H], F32)
            dma_late(bias_sb, bias5[None, :, :].to_broadcast([IPC, 5, H]))
            gb_sb = bias_sb[:, 0, :]
            bb_sb = bias_sb[:, 1, :]
            b1_sb = bias_sb[:, 2, :]
            b2_sb = bias_sb[:, 3, :]
            bop_row = bias_sb[0:1, 4, :]
            ones16_sb = cpool.tile([128, 8], F16)
            dma(ones16_sb, ones16)
            poison_sb = cpool.tile([1, 128], F32)
            dma(poison_sb, poison)
            mlt_sb = cpool.tile([128, NCH, IPC], F32)
            dma(mlt_sb, mask_lt)
            mge_sb = cpool.tile([128, NCH, IPC], F16)
            dma(mge_sb, mask_ge)
            xi16_sb = cpool.tile([IPC, D], F16)
            dma(xi16_sb, xi16)
            xlo = cpool.tile([128, NCH, D], F16)
            dma(xlo, xf[0:K].rearrange("(c p) d -> p c d", p=128))
            b_att_col = cpool.tile([IPC, 1], F32)
            dma(b_att_col, b_att[None, :].to_broadcast([IPC, 1]))
            hp.__exit__(None, None, None)
            # tail-only loads on the slow queue, in rough use order
            dx = cpool.tile([128, NCH, D], F16)
            dma_late(dx, dxf.rearrange("(c p) d -> p c d", p=128))
            WpT_sb = cpool.tile([128, H], F16)
            dma_late(WpT_sb, WpT)
            WoT_sb = cpool.tile([128, NCH, H], F16)
            dma_late(WoT_sb, WoT.rearrange("(c p) h -> p c h", p=128))
            xi_sb = cpool.tile([IPC, D], F32)
            dma_late(xi_sb, xi)
            W1T_sb = cpool.tile([128, NCH, H], F16)
            dma_late(W1T_sb, W1T.rearrange("(c p) h -> p c h", p=128))
            W2T_sb = cpool.tile([128, NCH, H], F16)
            dma_late(W2T_sb, W2T.rearrange("(c p) h -> p c h", p=128))

            eps_col = cpool.tile([IPC, 1], F32)
            nc.vector.memset(eps_col, EPS)

            # ---- sa (this core's rows) and sb (all rows) ----
            scr_sa = smallp.tile([IPC, D], F16)
            nc.vector.tensor_mul(scr_sa, xi16_sb, wa_b)
            sa_col = smallp.tile([IPC, 1], F32)
            nc.vector.tensor_reduce(sa_col, scr_sa, axis=AX.X, op=OP.add)
            nc.vector.tensor_add(sa_col, sa_col, b_att_col)
            sa_diag = smallp.tile([IPC, IPC], F32)
            nc.vector.tensor_mul(sa_diag, id_sb[0:IPC, 0:IPC],
                                 sa_col.to_broadcast([IPC, IPC]))

            sb_cols = smallp.tile([128, NCH], F32)
            for c in range(NCH):
                scr_sb = smallp.tile([128, D], F16)
                nc.vector.tensor_mul(scr_sb, xlo[:, c, :], wb_b)
                nc.vector.tensor_reduce(
                    sb_cols[:, c:c + 1], scr_sb, axis=AX.X, op=OP.add)

            # sb_hi[p, c] = sb[c*128+p+1] via shift matmuls; slot 511 stays 0
            sbhi_ps = pss.tile([128, NCH], F32, tag="ps_small")
            nc.tensor.matmul(sbhi_ps, sh1_sb, sb_cols, start=True, stop=False)
            nc.tensor.matmul(sbhi_ps[:, 0:NCH - 1], sh2_sb, sb_cols[:, 1:NCH],
                             start=False, stop=True)
            sbhi_cols = smallp.tile([128, NCH], F32)
            nc.vector.tensor_copy(sbhi_cols, sbhi_ps)

            # ---- SBJ[t, i] = sa[i] + b_att + sb_hi[t] + mask_lt*(sb_lo-sb_hi),
            #      with -1e9 poison at the t=511 pad slot ----
            sbj = cpool.tile([128, NCH, IPC], F32)
            for c in range(NCH):
                diffc = smallp.tile([128, 1], F32)
                nc.vector.tensor_tensor(
                    diffc, sb_cols[:, c:c + 1], sbhi_cols[:, c:c + 1], OP.subtract)
                diagc = smallp.tile([128, 128], F32)
                nc.vector.tensor_mul(diagc, id_sb, diffc.to_broadcast([128, 128]))
                diagb = smallp.tile([128, 128], F32)
                nc.vector.tensor_mul(
                    diagb, id_sb, sbhi_cols[:, c:c + 1].to_broadcast([128, 128]))
                ps_sbj = pss.tile([128, IPC], F32, tag="ps_small")
                nc.tensor.matmul(ps_sbj, ones_sb[0:IPC, :], sa_diag,
                                 start=True, stop=False)
                nc.tensor.matmul(ps_sbj, diagb, ones_sb[:, 0:IPC],
                                 start=False, stop=False)
                if c == NCH - 1:
                    # poison: sigmoid(-1e9) = 0 exactly, pad row drops out
                    nc.tensor.matmul(ps_sbj, poison_sb, ones_sb[0:1, 0:IPC],
                                     start=False, stop=False)
                nc.tensor.matmul(ps_sbj, diagc, mlt_sb[:, c, :],
                                 start=False, stop=True)
                nc.vector.tensor_copy(sbj[:, c, :], ps_sbj)

            # ---- main edge pass ----
            alpha_full = cpool.tile([128, NCH, IPC], F16)   # raw sigmoid out
            age_full = cpool.tile([128, NCH, IPC], F16)     # masked (t>=i) alpha
            gx_ps = psgp.tile([IPC, D], F32)                # sum_t a*x[j]
            msg_ps = psmp.tile([IPC, H], F32)
            s_ps = pss.tile([1, IPC], F32, tag="ps_small")
            # U quad rows: bank b, partition slot 32s..32s+3 holds i=16b+4s+j
            u_ps = [psflex.tile([128, 512], F32, tag="flex", name=f"u_ps{b}")
                    for b in range(4)]
            for b in range(4):
                nc.vector.memset(u_ps[b], 0.0)

            for c in range(NCH):
                pf_t = pfp.tile([128, IB, PD], F16)
                for hh in range(2):
                    dma(pf_t[:, 32 * hh:32 * (hh + 1), :],
                        pf[c, :, 32 * hh:32 * (hh + 1), :])
                # sc = sum_pd pf_sent (wc pre-folded): fp16 halving tree
                scr = scrp.tile([128, IB, 64], F16)
                nc.vector.tensor_add(scr, pf_t[:, :, 0:64], pf_t[:, :, 64:128])
                w = 32
                while w >= 2:
                    nc.vector.tensor_add(
                        scr[:, :, 0:w], scr[:, :, 0:w], scr[:, :, w:2 * w])
                    w //= 2
                sc_t = smallp.tile([128, IB], F32)
                nc.vector.tensor_add(sc_t, scr[:, :, 0], scr[:, :, 1])
                aarg = smallp.tile([128, IB], F32)
                nc.vector.tensor_add(aarg, sc_t, sbj[:, c, :])
                nc.scalar.activation(alpha_full[:, c, :], aarg, AF.Sigmoid)
                nc.vector.tensor_mul(age_full[:, c, :], alpha_full[:, c, :],
                                     mge_sb[:, c, :])
                # U quads: lhsT = 4 alpha columns, rhs = 4 pf blocks; the
                # wanted rows sit on the diagonal (gathered via DRAM AP)
                for q in range(IB // 4):
                    b, sp = divmod(q, 4)
                    nc.tensor.matmul(
                        u_ps[b][32 * sp:32 * sp + 4, :],
                        alpha_full[:, c, 4 * q:4 * q + 4],
                        pf_t[:, 4 * q:4 * q + 4, :],
                        start=(c == 0), stop=(c == NCH - 1),
                        tile_position=(0, 32 * sp))
                nc.tensor.matmul(gx_ps, alpha_full[:, c, :], xlo[:, c, :],
                                 start=(c == 0), stop=False)
                nc.tensor.matmul(s_ps, ones16_sb[:, 0:1], alpha_full[:, c, :],
                                 start=(c == 0), stop=(c == NCH - 1))

            # G2 (shifted-x correction) after the loop: dx arrives on the slow
            # queue and age_full persists, so this overlaps the loop tail
            for c in range(NCH):
                nc.tensor.matmul(gx_ps, age_full[:, c, :], dx[:, c, :],
                                 start=False, stop=(c == NCH - 1))

            # ---- messages = U@WpT + Gx@WoT + s_alpha x bop ----
            s_row = smallp.tile([1, IPC], F32)
            nc.vector.tensor_copy(s_row, s_ps)

            # U reassembly: full-bank psum->sbuf copies, DRAM dump, then one
            # gather DMA picks the diagonal: row (b, s, j) -> i = 16b+4s+j at
            # element offset 65536b + 16384s + 640j (+pd).
            for b in range(4):
                u_cp = postp.tile([128, 512], F32, tag="u_cp")
                nc.vector.tensor_copy(u_cp, u_ps[b])
                # keep the whole u_dram bounce on one queue: Tile does not
                # track ordering through raw DRAM tensors, same-queue FIFO does
                nc.sync.dma_start(
                    out=u_dram[b * 65536:(b + 1) * 65536].rearrange(
                        "(p f) -> p f", p=128), in_=u_cp)
            u_sb64 = postp.tile([IPC, PD], F32)
            gather = bass.AP(tensor=u_dram.tensor, offset=0,
                             ap=[[65536, 4], [16384, 4], [640, 4], [1, 128]])
            nc.sync.dma_start(out=u_sb64, in_=gather)
            ps_ut = pss.tile([128, IPC], F32, tag="ps_small")
            nc.tensor.transpose(ps_ut, u_sb64, id_sb[0:IPC, 0:IPC])
            u_sb = postp.tile([128, IPC], F16)
            nc.vector.tensor_copy(u_sb, ps_ut)

            gx_sb = postp.tile([IPC, D], F32)
            nc.vector.tensor_copy(gx_sb, gx_ps)
            gxT = postp.tile([128, NCH, IPC], F16)
            for c in range(NCH):
                ptg = pss.tile([128, IPC], F32, tag="ps_small")
                nc.tensor.transpose(ptg, gx_sb[:, c * 128:(c + 1) * 128],
                                    id_sb[0:IPC, 0:IPC])
                nc.vector.tensor_copy(gxT[:, c, :], ptg)

            nc.tensor.matmul(msg_ps, u_sb, WpT_sb, start=True, stop=False)
            for c in range(NCH):
                nc.tensor.matmul(msg_ps, gxT[:, c, :], WoT_sb[:, c, :],
                                 start=False, stop=False)
            nc.tensor.matmul(msg_ps, s_row, bop_row, start=False, stop=True)

            # ---- residual + LN1 ----
            def layer_norm(v):
                stats = smallp.tile([IPC, 6], F32)
                nc.vector.bn_stats(out=stats, in_=v)
                mv = smallp.tile([IPC, 2], F32)
                nc.vector.bn_aggr(out=mv, in_=stats)
                std = smallp.tile([IPC, 1], F32)
                nc.scalar.activation(std, mv[:, 1:2], AF.Sqrt, bias=eps_col)
                rstd = smallp.tile([IPC, 1], F32)
                nc.vector.reciprocal(rstd, std)
                cen = postp.tile([IPC, H], F32)
                nc.vector.tensor_scalar(cen, v, mv[:, 0:1], rstd,
                                        OP.subtract, OP.mult)
                o = postp.tile([IPC, H], F32)
                nc.vector.tensor_mul(o, cen, gb_sb)
                nc.vector.tensor_add(o, o, bb_sb)
                return o

            h_sb = postp.tile([IPC, H], F32)
            nc.vector.tensor_add(h_sb, xi_sb, msg_ps)
            out1 = layer_norm(h_sb)

            # ---- FFN ----
            def transpose_rows(v):
                vT = postp.tile([128, NCH, IPC], F16, tag="vT")
                for c in range(NCH):
                    ptt = pss.tile([128, IPC], F32, tag="ps_small")
                    nc.tensor.transpose(ptt, v[:, c * 128:(c + 1) * 128],
                                        id_sb[0:IPC, 0:IPC])
                    nc.vector.tensor_copy(vT[:, c, :], ptt)
                return vT

            o1T = transpose_rows(out1)
            f1_ps = psflex.tile([IPC, H], F32, tag="flex")
            for c in range(NCH):
                nc.tensor.matmul(f1_ps, o1T[:, c, :], W1T_sb[:, c, :],
                                 start=(c == 0), stop=(c == NCH - 1))
            f1 = postp.tile([IPC, H], F32)
            nc.vector.tensor_add(f1, f1_ps, b1_sb)
            nc.vector.tensor_scalar_max(f1, f1, 0.0)

            f1T = transpose_rows(f1)
            f2_ps = psflex.tile([IPC, H], F32, tag="flex")
            for c in range(NCH):
                nc.tensor.matmul(f2_ps, f1T[:, c, :], W2T_sb[:, c, :],
                                 start=(c == 0), stop=(c == NCH - 1))
            h2 = postp.tile([IPC, H], F32)
            nc.vector.tensor_add(h2, f2_ps, b2_sb)
            nc.vector.tensor_add(h2, h2, out1)
            out2 = layer_norm(h2)

            nc.sync.dma_start(out=out_d, in_=out2)

    return nc


def _poison() -> np.ndarray:
    p = np.zeros((1, 128), np.float32)
    p[0, 127] = -1e9
    return p


def _cmat() -> np.ndarray:
    c = np.zeros((128, 4, 128), np.float32)
    c[:, 0, :] = np.eye(128)
    c[:, 1, :] = 1.0
    c[:, 2, :] = np.eye(128, k=-1)     # shift1[q, p] = (q == p+1)
    c[0, 3, 127] = 1.0                  # shift2[q, p] = (q==0)&(p==127)
    return c


def prep_in_maps(inputs) -> list[dict]:
    x = np.asarray(inputs["x"], np.float32)
    pf = np.asarray(inputs["pair_feats"], np.float32)
    W_att = np.asarray(inputs["W_att"], np.float32)
    b_att = np.asarray(inputs["b_att"], np.float32)
    W_obj = np.asarray(inputs["W_obj"], np.float32)
    b_obj = np.asarray(inputs["b_obj"], np.float32)
    W_pair = np.asarray(inputs["W_pair"], np.float32)
    b_pair = np.asarray(inputs["b_pair"], np.float32)
    ln_g = np.asarray(inputs["ln_g"], np.float32)
    ln_b = np.asarray(inputs["ln_b"], np.float32)
    W1 = np.asarray(inputs["W1"], np.float32)
    b1 = np.asarray(inputs["b1"], np.float32)
    W2 = np.asarray(inputs["W2"], np.float32)
    b2 = np.asarray(inputs["b2"], np.float32)

    wa, wb, wc = W_att[0, :D], W_att[0, D:2 * D], W_att[0, 2 * D:]
    xpad = np.concatenate([x, np.zeros((1, D), np.float32)], axis=0)

    # fold wc into pf columns; recover U via pre-divided W_pair.T rows.
    colscale = np.sign(wc) * np.maximum(np.abs(wc), 6e-5)
    colscale[colscale == 0] = 6e-5
    # 1/511 (the mean over neighbors) is folded into the three weight paths
    # that consume raw alpha: U@WpT, (A@x)@WoT, and s_alpha*bop.
    WpT2 = (W_pair.T / colscale[:, None] / T).astype(np.float16)
    WoT2 = (W_obj.T / T).astype(np.float16)
    dxf = np.diff(xpad[:K + 1], axis=0)

    base = dict(
        xf=xpad.astype(np.float16),
        dxf=dxf.astype(np.float16),
        cmat=_cmat(),
        ones16=np.ones((128, 8), np.float16),
        poison=_poison(),
        wab=np.stack([wa, wb]).astype(np.float16),
        b_att=b_att.astype(np.float32),
        bias5=np.stack([ln_g, ln_b, b1, b2,
                        (b_obj + b_pair) / T]).astype(np.float32),
        WpT=np.ascontiguousarray(WpT2),
        WoT=np.ascontiguousarray(WoT2),
        W1T=np.ascontiguousarray(W1.T).astype(np.float16),
        W2T=np.ascontiguousarray(W2.T).astype(np.float16),
    )

    pfr = pf.reshape(K, T, PD)
    tgrid = np.arange(128)[:, None] + 128 * np.arange(NCH)[None, :]   # [128, NCH]

    in_maps = []
    for core in range(NCORES):
        ig = np.arange(core * IPC, (core + 1) * IPC)
        mlt = (tgrid[:, :, None] < ig[None, None, :]).astype(np.float32)
        mge = ((tgrid[:, :, None] >= ig[None, None, :])
               & (tgrid[:, :, None] <= T - 1)).astype(np.float16)
        # [chunk, t, i, pd] layout -> each tile DMA is one contiguous burst
        shard = np.zeros((NCH * 128, IPC, PD), np.float16)
        shard[:T] = (pfr[ig] * colscale[None, None, :]).transpose(1, 0, 2)
        xi = x[ig]
        m = dict(base)
        m.update(
            pf=shard.reshape(NCH, 128, IPC, PD),
            xi=xi.astype(np.float32),
            xi16=xi.astype(np.float16),
            mask_lt=mlt,
            mask_ge=mge,
        )
        in_maps.append(m)
    return in_maps


_COMPILED = None


def _get_program() -> bacc.Bacc:
    global _COMPILED
    if _COMPILED is None:
        nc = build_program()
        nc.compile()
        _COMPILED = nc
    return _COMPILED


TRACE = False
LAST_RESULT = None


def _install_axon_ntff_hook():
    """The container's antenv lacks axon_hooks; recreate it from trn_boot's
    ctypes implementation so trace=True can capture NTFF profiles."""
    import sys
    import types
    try:
        from antenv.axon_hooks import get_axon_ntff_profile_hook  # noqa: F401
        return
    except ImportError:
        pass
    from trn_agent_boot.trn_boot import _ntff_profile_via_ctypes
    hook = _ntff_profile_via_ctypes("/opt/axon/libaxon_pjrt.so")
    m = types.ModuleType("antenv.axon_hooks")
    m.get_axon_ntff_profile_hook = lambda: hook
    sys.modules["antenv.axon_hooks"] = m


def kernel(**inputs) -> np.ndarray:
    import concourse.bass_utils as bu
    from concourse.bass_utils import run_bass_kernel_spmd
    global LAST_RESULT
    if TRACE:
        _install_axon_ntff_hook()
        bu.upload_artifacts = lambda tmpdir: str(tmpdir)  # no bucket here
    nc = _get_program()
    in_maps = prep_in_maps(inputs)
    res = run_bass_kernel_spmd(nc, in_maps, list(range(NCORES)), trace=TRACE)
    LAST_RESULT = res
    outs = [res.results[c]["out"] for c in range(NCORES)]
    return np.concatenate(outs, axis=0).astype(np.float32)


# revision 40
# speedup vs baseline: 1.3220x; 1.0727x over previous
"""Trainium2 Bass kernel for nn_ARTLayer (gnn_message_passing).

Math (reference):
    j(i,t) = t + (t>=i)                                    # [K, K-1] neighbor index
    alpha  = sigmoid(x@wa [i] + x@wb [j] + pf@wc + b_att)  # [K, K-1]
    msgs   = mean_t alpha * ((x@WobjT + b_obj)[j] + pf@WpairT + b_pair)
    out    = LN(x + msgs); out = LN(out + FFN(out))

Key algebraic rewrite (removes the 34-GFLOP [P,PD]x[PD,H] einsum):
    sum_t a*(pf@WpT)  = (sum_t a*pf) @ WpT               -> U[i,:] @ WpT
    sum_t a*oj[j]     = (A @ x) @ WobjT                  -> Gx[i,:] @ WoT
      with A[i,j] decomposed via lo/hi shifted views of x and a t>=i mask
    sum_t a*(b_obj+b_pair) = s_alpha[i] * bop

Sharding: rows i split across 8 cores (64 each); small tensors replicated;
host concatenates the per-core [64, 512] outputs.

Implementation notes (driven by NTFF profiles):
  - wc is folded into pf on the host (column scales, floored at fp16
    min-normal); sc becomes a pure fp16 halving-tree add-reduce and U is
    recovered exactly via W_pairT rows pre-divided by the scales.
  - 1/511 (the neighbor mean) is folded into WpT/WoT/bop on the host, and
    the t=511 pad slot is poisoned with -1e9 pre-sigmoid, so raw sigmoid
    output is used directly with no mask/scale multiplies.
  - U accumulation runs as M=4 quad matmuls (alpha quad stationary, four pf
    blocks streaming at N=512); the wanted rows sit on the block diagonal
    and are gathered by a stride-640 DRAM access pattern after a bank dump.
  - pf is re-laid-out on the host to [chunk, t, i, pd] so each tile DMA is
    one fully-contiguous 8KB-per-partition burst, and all DMAs are spread
    round-robin over the three DMA-capable engines (sync/scalar HWDGE,
    gpsimd SWDGE) instead of serializing on one queue.
"""
import numpy as np

import concourse.bass as bass
import concourse.tile as tile
from concourse import bacc, mybir

F32, F16 = mybir.dt.float32, mybir.dt.float16
AX = mybir.AxisListType
OP = mybir.AluOpType
AF = mybir.ActivationFunctionType

K, D, H, PD = 512, 512, 512, 128
T = K - 1                      # 511 neighbors per row
NCORES, IPC = 8, 64            # rows per core
NCH = 4                        # t-chunks of 128 (last chunk row 127 is t=511 pad)
IB, NIB = 64, 1                # i-block within a core
EPS = 1e-5


def build_program() -> bacc.Bacc:
    nc = bacc.Bacc("TRN2", target_bir_lowering=False, debug=False)

    def inp(name, shape, dt):
        return nc.dram_tensor(name, shape, dt, kind="ExternalInput").ap()

    pf = inp("pf", [NCH, 128, IPC, PD], F16)    # [chunk, t-in-chunk, i, pd]
    xf = inp("xf", [K + 1, D], F16)             # x with one zero pad row
    dxf = inp("dxf", [K, D], F16)               # x[t+1] - x[t], host computed
    xi = inp("xi", [IPC, D], F32)               # this core's rows of x
    xi16 = inp("xi16", [IPC, D], F16)
    mask_lt = inp("mask_lt", [128, NCH, IPC], F32)   # t <  i_global
    mask_ge = inp("mask_ge", [128, NCH, IPC], F16)   # t >= i_global and t <= 510
    cmat = inp("cmat", [128, 4, 128], F32)      # [ident | ones | shift1 | shift2]
    ones16 = inp("ones16", [128, 8], F16)
    poison = inp("poison", [1, 128], F32)       # -1e9 at slot 127, else 0
    wab = inp("wab", [2, D], F16)               # [wa; wb]
    b_att = inp("b_att", [1], F32)
    bias5 = inp("bias5", [5, H], F32)           # [ln_g; ln_b; b1; b2; bop/511]
    WpT = inp("WpT", [PD, H], F16)              # W_pair.T / colscale / 511
    WoT = inp("WoT", [D, H], F16)               # W_obj.T / 511
    W1T = inp("W1T", [H, H], F16)
    W2T = inp("W2T", [H, H], F16)

    out_d = nc.dram_tensor("out", [IPC, H], F32, kind="ExternalOutput").ap()
    u_dram = nc.dram_tensor("u_dram", [4 * 128 * 512], F32).ap()  # U bank dumps

    with tile.TileContext(nc) as tc:
        with (
            tc.tile_pool(name="const", bufs=1) as cpool,
            tc.tile_pool(name="pfp", bufs=4) as pfp,
            tc.tile_pool(name="scrp", bufs=3) as scrp,
            tc.tile_pool(name="smallp", bufs=4) as smallp,
            tc.tile_pool(name="postp", bufs=2) as postp,
            tc.tile_pool(name="pss", bufs=2, space="PSUM") as pss,
            tc.tile_pool(name="psflex", bufs=4, space="PSUM") as psflex,
            tc.tile_pool(name="psmp", bufs=1, space="PSUM") as psmp,
            tc.tile_pool(name="psgp", bufs=1, space="PSUM") as psgp,
        ):
            # DMA policy: one HWDGE queue (sync) carries the critical path
            # in program order (a single queue reaches ~313 GB/s here and
            # multi-queue round-robin measures WORSE); bulky tail-only
            # weights stream on the gpsimd SWDGE queue in parallel.
            def dma(out, in_):
                nc.sync.dma_start(out=out, in_=in_)

            def dma_late(out, in_):
                nc.gpsimd.dma_start(out=out, in_=in_)

            # ---- constants & weights to SBUF ----
            hp = tc.high_priority()
            hp.__enter__()
            cm_sb = cpool.tile([128, 4, 128], F32)
            dma(cm_sb, cmat)
            id_sb = cm_sb[:, 0, :]
            ones_sb = cm_sb[:, 1, :]
            sh1_sb = cm_sb[:, 2, :]
            sh2_sb = cm_sb[:, 3, :]
            wab_sb = cpool.tile([128, 2, D], F16)
            dma(wab_sb, wab[None, :, :].to_broadcast([128, 2, D]))
            wa_b = wab_sb[0:IPC, 0, :]
            wb_b = wab_sb[:, 1, :]
            bias_sb = cpool.tile([IPC, 5, H], F32)
            dma_late(bias_sb, bias5[None, :, :].to_broadcast([IPC, 5, H]))
            gb_sb = bias_sb[:, 0, :]
            bb_sb = bias_sb[:, 1, :]
            b1_sb = bias_sb[:, 2, :]
            b2_sb = bias_sb[:, 3, :]
            bop_row = bias_sb[0:1, 4, :]
            ones16_sb = cpool.tile([128, 8], F16)
            dma(ones16_sb, ones16)
            poison_sb = cpool.tile([1, 128], F32)
            dma(poison_sb, poison)
            mlt_sb = cpool.tile([128, NCH, IPC], F32)
            dma(mlt_sb, mask_lt)
            mge_sb = cpool.tile([128, NCH, IPC], F16)
            dma(mge_sb, mask_ge)
            xi16_sb = cpool.tile([IPC, D], F16)
            dma(xi16_sb, xi16)
            xlo = cpool.tile([128, NCH, D], F16)
            dma(xlo, xf[0:K].rearrange("(c p) d -> p c d", p=128))
            b_att_col = cpool.tile([IPC, 1], F32)
            dma(b_att_col, b_att[None, :].to_broadcast([IPC, 1]))
            hp.__exit__(None, None, None)
            tc.no_sync_barrier()   # keep pf DMAs behind the critical consts
            # tail-only loads on the slow queue, in rough use order
            dx = cpool.tile([128, NCH, D], F16)
            dma_late(dx, dxf.rearrange("(c p) d -> p c d", p=128))
            WpT_sb = cpool.tile([128, H], F16)
            dma_late(WpT_sb, WpT)
            WoT_sb = cpool.tile([128, NCH, H], F16)
            dma_late(WoT_sb, WoT.rearrange("(c p) h -> p c h", p=128))
            xi_sb = cpool.tile([IPC, D], F32)
            dma_late(xi_sb, xi)
            W1T_sb = cpool.tile([128, NCH, H], F16)
            dma_late(W1T_sb, W1T.rearrange("(c p) h -> p c h", p=128))
            W2T_sb = cpool.tile([128, NCH, H], F16)
            dma_late(W2T_sb, W2T.rearrange("(c p) h -> p c h", p=128))

            eps_col = cpool.tile([IPC, 1], F32)
            nc.vector.memset(eps_col, EPS)

            # ---- sa (this core's rows) and sb (all rows) ----
            scr_sa = smallp.tile([IPC, D], F16)
            nc.vector.tensor_mul(scr_sa, xi16_sb, wa_b)
            sa_col = smallp.tile([IPC, 1], F32)
            nc.vector.tensor_reduce(sa_col, scr_sa, axis=AX.X, op=OP.add)
            nc.vector.tensor_add(sa_col, sa_col, b_att_col)
            sa_diag = smallp.tile([IPC, IPC], F32)
            nc.vector.tensor_mul(sa_diag, id_sb[0:IPC, 0:IPC],
                                 sa_col.to_broadcast([IPC, IPC]))

            sb_cols = smallp.tile([128, NCH], F32)
            for c in range(NCH):
                scr_sb = smallp.tile([128, D], F16)
                nc.vector.tensor_mul(scr_sb, xlo[:, c, :], wb_b)
                nc.vector.tensor_reduce(
                    sb_cols[:, c:c + 1], scr_sb, axis=AX.X, op=OP.add)

            # sb_hi[p, c] = sb[c*128+p+1] via shift matmuls; slot 511 stays 0
            sbhi_ps = pss.tile([128, NCH], F32, tag="ps_small")
            nc.tensor.matmul(sbhi_ps, sh1_sb, sb_cols, start=True, stop=False)
            nc.tensor.matmul(sbhi_ps[:, 0:NCH - 1], sh2_sb, sb_cols[:, 1:NCH],
                             start=False, stop=True)
            sbhi_cols = smallp.tile([128, NCH], F32)
            nc.vector.tensor_copy(sbhi_cols, sbhi_ps)

            # ---- SBJ[t, i] = sa[i] + b_att + sb_hi[t] + mask_lt*(sb_lo-sb_hi),
            #      with -1e9 poison at the t=511 pad slot ----
            sbj = cpool.tile([128, NCH, IPC], F32)
            for c in range(NCH):
                diffc = smallp.tile([128, 1], F32)
                nc.vector.tensor_tensor(
                    diffc, sb_cols[:, c:c + 1], sbhi_cols[:, c:c + 1], OP.subtract)
                diagc = smallp.tile([128, 128], F32)
                nc.vector.tensor_mul(diagc, id_sb, diffc.to_broadcast([128, 128]))
                diagb = smallp.tile([128, 128], F32)
                nc.vector.tensor_mul(
                    diagb, id_sb, sbhi_cols[:, c:c + 1].to_broadcast([128, 128]))
                ps_sbj = pss.tile([128, IPC], F32, tag="ps_small")
                nc.tensor.matmul(ps_sbj, ones_sb[0:IPC, :], sa_diag,
                                 start=True, stop=False)
                nc.tensor.matmul(ps_sbj, diagb, ones_sb[:, 0:IPC],
                                 start=False, stop=False)
                if c == NCH - 1:
                    # poison: sigmoid(-1e9) = 0 exactly, pad row drops out
                    nc.tensor.matmul(ps_sbj, poison_sb, ones_sb[0:1, 0:IPC],
                                     start=False, stop=False)
                nc.tensor.matmul(ps_sbj, diagc, mlt_sb[:, c, :],
                                 start=False, stop=True)
                nc.vector.tensor_copy(sbj[:, c, :], ps_sbj)

            # ---- main edge pass ----
            alpha_full = cpool.tile([128, NCH, IPC], F16)   # raw sigmoid out
            age_full = cpool.tile([128, NCH, IPC], F16)     # masked (t>=i) alpha
            gx_ps = psgp.tile([IPC, D], F32)                # sum_t a*x[j]
            msg_ps = psmp.tile([IPC, H], F32)
            s_ps = pss.tile([1, IPC], F32, tag="ps_small")
            # U quad rows: bank b, partition slot 32s..32s+3 holds i=16b+4s+j
            u_ps = [psflex.tile([128, 512], F32, tag="flex", name=f"u_ps{b}")
                    for b in range(4)]
            for b in range(4):
                nc.vector.memset(u_ps[b], 0.0)

            for c in range(NCH):
                pf_t = pfp.tile([128, IB, PD], F16)
                dma(pf_t, pf[c, :, :, :])
                # sc = sum_pd pf_sent (wc pre-folded): fp16 halving tree
                scr = scrp.tile([128, IB, 64], F16)
                nc.vector.tensor_add(scr, pf_t[:, :, 0:64], pf_t[:, :, 64:128])
                w = 32
                while w >= 2:
                    nc.vector.tensor_add(
                        scr[:, :, 0:w], scr[:, :, 0:w], scr[:, :, w:2 * w])
                    w //= 2
                sc_t = smallp.tile([128, IB], F32)
                nc.vector.tensor_add(sc_t, scr[:, :, 0], scr[:, :, 1])
                aarg = smallp.tile([128, IB], F32)
                nc.vector.tensor_add(aarg, sc_t, sbj[:, c, :])
                nc.scalar.activation(alpha_full[:, c, :], aarg, AF.Sigmoid)
                nc.vector.tensor_mul(age_full[:, c, :], alpha_full[:, c, :],
                                     mge_sb[:, c, :])
                # U quads: lhsT = 4 alpha columns, rhs = 4 pf blocks; the
                # wanted rows sit on the diagonal (gathered via DRAM AP)
                for q in range(IB // 4):
                    b, sp = divmod(q, 4)
                    nc.tensor.matmul(
                        u_ps[b][32 * sp:32 * sp + 4, :],
                        alpha_full[:, c, 4 * q:4 * q + 4],
                        pf_t[:, 4 * q:4 * q + 4, :],
                        start=(c == 0), stop=(c == NCH - 1),
                        tile_position=(0, 32 * sp))
                nc.tensor.matmul(gx_ps, alpha_full[:, c, :], xlo[:, c, :],
                                 start=(c == 0), stop=False)
                nc.tensor.matmul(s_ps, ones16_sb[:, 0:1], alpha_full[:, c, :],
                                 start=(c == 0), stop=(c == NCH - 1))

            # scheduler fence: keep every tail instruction after the loop in
            # each engine stream (strict-FIFO engines head-of-line block if
            # e.g. an LN Sqrt lands between loop sigmoids in the ACT queue)
            tc.no_sync_barrier()

            # G2 (shifted-x correction) after the loop: dx arrives on the slow
            # queue and age_full persists, so this overlaps the loop tail
            for c in range(NCH):
                nc.tensor.matmul(gx_ps, age_full[:, c, :], dx[:, c, :],
                                 start=False, stop=(c == NCH - 1))

            # ---- messages = U@WpT + Gx@WoT + s_alpha x bop ----
            s_row = smallp.tile([1, IPC], F32)
            nc.vector.tensor_copy(s_row, s_ps)

            # U reassembly: full-bank psum->sbuf copies, DRAM dump, then one
            # gather DMA picks the diagonal: row (b, s, j) -> i = 16b+4s+j at
            # element offset 65536b + 16384s + 640j (+pd).
            for b in range(4):
                u_cp = postp.tile([128, 512], F32, tag="u_cp")
                nc.vector.tensor_copy(u_cp, u_ps[b])
                # keep the whole u_dram bounce on one queue: Tile does not
                # track ordering through raw DRAM tensors, same-queue FIFO does
                nc.sync.dma_start(
                    out=u_dram[b * 65536:(b + 1) * 65536].rearrange(
                        "(p f) -> p f", p=128), in_=u_cp)
            u_sb64 = postp.tile([IPC, PD], F32)
            gather = bass.AP(tensor=u_dram.tensor, offset=0,
                             ap=[[65536, 4], [16384, 4], [640, 4], [1, 128]])
            nc.sync.dma_start(out=u_sb64, in_=gather)
            ps_ut = pss.tile([128, IPC], F32, tag="ps_small")
            nc.tensor.transpose(ps_ut, u_sb64, id_sb[0:IPC, 0:IPC])
            u_sb = postp.tile([128, IPC], F16)
            nc.vector.tensor_copy(u_sb, ps_ut)

            gx_sb = postp.tile([IPC, D], F32)
            nc.vector.tensor_copy(gx_sb, gx_ps)
            gxT = postp.tile([128, NCH, IPC], F16)
            for c in range(NCH):
                ptg = pss.tile([128, IPC], F32, tag="ps_small")
                nc.tensor.transpose(ptg, gx_sb[:, c * 128:(c + 1) * 128],
                                    id_sb[0:IPC, 0:IPC])
                nc.vector.tensor_copy(gxT[:, c, :], ptg)

            nc.tensor.matmul(msg_ps, u_sb, WpT_sb, start=True, stop=False)
            for c in range(NCH):
                nc.tensor.matmul(msg_ps, gxT[:, c, :], WoT_sb[:, c, :],
                                 start=False, stop=False)
            nc.tensor.matmul(msg_ps, s_row, bop_row, start=False, stop=True)

            # ---- residual + LN1 ----
            def layer_norm(v):
                stats = smallp.tile([IPC, 6], F32)
                nc.vector.bn_stats(out=stats, in_=v)
                mv = smallp.tile([IPC, 2], F32)
                nc.vector.bn_aggr(out=mv, in_=stats)
                std = smallp.tile([IPC, 1], F32)
                nc.scalar.activation(std, mv[:, 1:2], AF.Sqrt, bias=eps_col)
                rstd = smallp.tile([IPC, 1], F32)
                nc.vector.reciprocal(rstd, std)
                cen = postp.tile([IPC, H], F32)
                nc.vector.tensor_scalar(cen, v, mv[:, 0:1], rstd,
                                        OP.subtract, OP.mult)
                o = postp.tile([IPC, H], F32)
                nc.vector.tensor_mul(o, cen, gb_sb)
                nc.vector.tensor_add(o, o, bb_sb)
                return o

            h_sb = postp.tile([IPC, H], F32)
            nc.vector.tensor_add(h_sb, xi_sb, msg_ps)
            out1 = layer_norm(h_sb)

            # ---- FFN ----
            def transpose_rows(v):
                vT = postp.tile([128, NCH, IPC], F16, tag="vT")
                for c in range(NCH):
                    ptt = pss.tile([128, IPC], F32, tag="ps_small")
                    nc.tensor.transpose(ptt, v[:, c * 128:(c + 1) * 128],
                                        id_sb[0:IPC, 0:IPC])
                    nc.vector.tensor_copy(vT[:, c, :], ptt)
                return vT

            o1T = transpose_rows(out1)
            f1_ps = psflex.tile([IPC, H], F32, tag="flex")
            for c in range(NCH):
                nc.tensor.matmul(f1_ps, o1T[:, c, :], W1T_sb[:, c, :],
                                 start=(c == 0), stop=(c == NCH - 1))
            f1 = postp.tile([IPC, H], F32)
            nc.vector.tensor_add(f1, f1_ps, b1_sb)
            nc.vector.tensor_scalar_max(f1, f1, 0.0)

            f1T = transpose_rows(f1)
            f2_ps = psflex.tile([IPC, H], F32, tag="flex")
            for c in range(NCH):
                nc.tensor.matmul(f2_ps, f1T[:, c, :], W2T_sb[:, c, :],
                                 start=(c == 0), stop=(c == NCH - 1))
            h2 = postp.tile([IPC, H], F32)
            nc.vector.tensor_add(h2, f2_ps, b2_sb)
            nc.vector.tensor_add(h2, h2, out1)
            out2 = layer_norm(h2)

            nc.sync.dma_start(out=out_d, in_=out2)

    return nc


def _poison() -> np.ndarray:
    p = np.zeros((1, 128), np.float32)
    p[0, 127] = -1e9
    return p


def _cmat() -> np.ndarray:
    c = np.zeros((128, 4, 128), np.float32)
    c[:, 0, :] = np.eye(128)
    c[:, 1, :] = 1.0
    c[:, 2, :] = np.eye(128, k=-1)     # shift1[q, p] = (q == p+1)
    c[0, 3, 127] = 1.0                  # shift2[q, p] = (q==0)&(p==127)
    return c


def prep_in_maps(inputs) -> list[dict]:
    x = np.asarray(inputs["x"], np.float32)
    pf = np.asarray(inputs["pair_feats"], np.float32)
    W_att = np.asarray(inputs["W_att"], np.float32)
    b_att = np.asarray(inputs["b_att"], np.float32)
    W_obj = np.asarray(inputs["W_obj"], np.float32)
    b_obj = np.asarray(inputs["b_obj"], np.float32)
    W_pair = np.asarray(inputs["W_pair"], np.float32)
    b_pair = np.asarray(inputs["b_pair"], np.float32)
    ln_g = np.asarray(inputs["ln_g"], np.float32)
    ln_b = np.asarray(inputs["ln_b"], np.float32)
    W1 = np.asarray(inputs["W1"], np.float32)
    b1 = np.asarray(inputs["b1"], np.float32)
    W2 = np.asarray(inputs["W2"], np.float32)
    b2 = np.asarray(inputs["b2"], np.float32)

    wa, wb, wc = W_att[0, :D], W_att[0, D:2 * D], W_att[0, 2 * D:]
    xpad = np.concatenate([x, np.zeros((1, D), np.float32)], axis=0)

    # fold wc into pf columns; recover U via pre-divided W_pair.T rows.
    colscale = np.sign(wc) * np.maximum(np.abs(wc), 6e-5)
    colscale[colscale == 0] = 6e-5
    # 1/511 (the mean over neighbors) is folded into the three weight paths
    # that consume raw alpha: U@WpT, (A@x)@WoT, and s_alpha*bop.
    WpT2 = (W_pair.T / colscale[:, None] / T).astype(np.float16)
    WoT2 = (W_obj.T / T).astype(np.float16)
    dxf = np.diff(xpad[:K + 1], axis=0)

    base = dict(
        xf=xpad.astype(np.float16),
        dxf=dxf.astype(np.float16),
        cmat=_cmat(),
        ones16=np.ones((128, 8), np.float16),
        poison=_poison(),
        wab=np.stack([wa, wb]).astype(np.float16),
        b_att=b_att.astype(np.float32),
        bias5=np.stack([ln_g, ln_b, b1, b2,
                        (b_obj + b_pair) / T]).astype(np.float32),
        WpT=np.ascontiguousarray(WpT2),
        WoT=np.ascontiguousarray(WoT2),
        W1T=np.ascontiguousarray(W1.T).astype(np.float16),
        W2T=np.ascontiguousarray(W2.T).astype(np.float16),
    )

    pfr = pf.reshape(K, T, PD)
    tgrid = np.arange(128)[:, None] + 128 * np.arange(NCH)[None, :]   # [128, NCH]

    in_maps = []
    for core in range(NCORES):
        ig = np.arange(core * IPC, (core + 1) * IPC)
        mlt = (tgrid[:, :, None] < ig[None, None, :]).astype(np.float32)
        mge = ((tgrid[:, :, None] >= ig[None, None, :])
               & (tgrid[:, :, None] <= T - 1)).astype(np.float16)
        # [chunk, t, i, pd] layout -> each tile DMA is one contiguous burst
        shard = np.zeros((NCH * 128, IPC, PD), np.float16)
        shard[:T] = (pfr[ig] * colscale[None, None, :]).transpose(1, 0, 2)
        xi = x[ig]
        m = dict(base)
        m.update(
            pf=shard.reshape(NCH, 128, IPC, PD),
            xi=xi.astype(np.float32),
            xi16=xi.astype(np.float16),
            mask_lt=mlt,
            mask_ge=mge,
        )
        in_maps.append(m)
    return in_maps


_COMPILED = None


def _get_program() -> bacc.Bacc:
    global _COMPILED
    if _COMPILED is None:
        nc = build_program()
        nc.compile()
        _COMPILED = nc
    return _COMPILED


TRACE = False
LAST_RESULT = None


def _install_axon_ntff_hook():
    """The container's antenv lacks axon_hooks; recreate it from trn_boot's
    ctypes implementation so trace=True can capture NTFF profiles."""
    import sys
    import types
    try:
        from antenv.axon_hooks import get_axon_ntff_profile_hook  # noqa: F401
        return
    except ImportError:
        pass
    from trn_agent_boot.trn_boot import _ntff_profile_via_ctypes
    hook = _ntff_profile_via_ctypes("/opt/axon/libaxon_pjrt.so")
    m = types.ModuleType("antenv.axon_hooks")
    m.get_axon_ntff_profile_hook = lambda: hook
    sys.modules["antenv.axon_hooks"] = m


def kernel(**inputs) -> np.ndarray:
    import concourse.bass_utils as bu
    from concourse.bass_utils import run_bass_kernel_spmd
    global LAST_RESULT
    if TRACE:
        _install_axon_ntff_hook()
        bu.upload_artifacts = lambda tmpdir: str(tmpdir)  # no bucket here
    nc = _get_program()
    in_maps = prep_in_maps(inputs)
    res = run_bass_kernel_spmd(nc, in_maps, list(range(NCORES)), trace=TRACE)
    LAST_RESULT = res
    outs = [res.results[c]["out"] for c in range(NCORES)]
    return np.concatenate(outs, axis=0).astype(np.float32)


# revision 41
# speedup vs baseline: 1.3789x; 1.0430x over previous
"""Trainium2 Bass kernel for nn_ARTLayer (gnn_message_passing).

Math (reference):
    j(i,t) = t + (t>=i)                                    # [K, K-1] neighbor index
    alpha  = sigmoid(x@wa [i] + x@wb [j] + pf@wc + b_att)  # [K, K-1]
    msgs   = mean_t alpha * ((x@WobjT + b_obj)[j] + pf@WpairT + b_pair)
    out    = LN(x + msgs); out = LN(out + FFN(out))

Key algebraic rewrite (removes the 34-GFLOP [P,PD]x[PD,H] einsum):
    sum_t a*(pf@WpT)  = (sum_t a*pf) @ WpT               -> U[i,:] @ WpT
    sum_t a*oj[j]     = (A @ x) @ WobjT                  -> Gx[i,:] @ WoT
      with A[i,j] decomposed via lo/hi shifted views of x and a t>=i mask
    sum_t a*(b_obj+b_pair) = s_alpha[i] * bop

Sharding: rows i split across 8 cores (64 each); small tensors replicated;
host concatenates the per-core [64, 512] outputs.

Implementation notes (driven by NTFF profiles):
  - wc is folded into pf on the host (column scales, floored at fp16
    min-normal); sc becomes a pure fp16 halving-tree add-reduce and U is
    recovered exactly via W_pairT rows pre-divided by the scales.
  - 1/511 (the neighbor mean) is folded into WpT/WoT/bop on the host, and
    the t=511 pad slot is poisoned with -1e9 pre-sigmoid, so raw sigmoid
    output is used directly with no mask/scale multiplies.
  - U accumulation runs as M=4 quad matmuls (alpha quad stationary, four pf
    blocks streaming at N=512); the wanted rows sit on the block diagonal
    and are gathered by a stride-640 DRAM access pattern after a bank dump.
  - pf is re-laid-out on the host to [chunk, t, i, pd] so each tile DMA is
    one fully-contiguous 8KB-per-partition burst, and all DMAs are spread
    round-robin over the three DMA-capable engines (sync/scalar HWDGE,
    gpsimd SWDGE) instead of serializing on one queue.
"""
import numpy as np

import concourse.bass as bass
import concourse.tile as tile
from concourse import bacc, mybir

F32, F16 = mybir.dt.float32, mybir.dt.float16
AX = mybir.AxisListType
OP = mybir.AluOpType
AF = mybir.ActivationFunctionType

K, D, H, PD = 512, 512, 512, 128
T = K - 1                      # 511 neighbors per row
NCORES, IPC = 8, 64            # rows per core
NCH = 4                        # t-chunks of 128 (last chunk row 127 is t=511 pad)
IB, NIB = 64, 1                # i-block within a core
EPS = 1e-5


def build_program() -> bacc.Bacc:
    nc = bacc.Bacc("TRN2", target_bir_lowering=False, debug=False)

    def inp(name, shape, dt):
        return nc.dram_tensor(name, shape, dt, kind="ExternalInput").ap()

    pf = inp("pf", [NCH, 128, IPC, PD], F16)    # [chunk, t-in-chunk, i, pd]
    xf = inp("xf", [K + 1, D], F16)             # x with one zero pad row
    dxf = inp("dxf", [K, D], F16)               # x[t+1] - x[t], host computed
    xi = inp("xi", [IPC, D], F32)               # this core's rows of x
    xi16 = inp("xi16", [IPC, D], F16)
    mask_lt = inp("mask_lt", [128, NCH, IPC], F32)   # t <  i_global
    mask_ge = inp("mask_ge", [128, NCH, IPC], F16)   # t >= i_global and t <= 510
    cmat = inp("cmat", [128, 4, 128], F32)      # [ident | ones | shift1 | shift2]
    ones16 = inp("ones16", [128, 8], F16)
    poison = inp("poison", [1, 128], F32)       # -1e9 at slot 127, else 0
    wab = inp("wab", [2, D], F16)               # [wa; wb]
    b_att = inp("b_att", [1], F32)
    bias5 = inp("bias5", [5, H], F32)           # [ln_g; ln_b; b1; b2; bop/511]
    WpT = inp("WpT", [PD, H], F16)              # W_pair.T / colscale / 511
    WoT = inp("WoT", [D, H], F16)               # W_obj.T / 511
    W1T = inp("W1T", [H, H], F16)
    W2T = inp("W2T", [H, H], F16)

    out_d = nc.dram_tensor("out", [IPC, H], F32, kind="ExternalOutput").ap()

    with tile.TileContext(nc) as tc:
        with (
            tc.tile_pool(name="const", bufs=1) as cpool,
            tc.tile_pool(name="pfp", bufs=4) as pfp,
            tc.tile_pool(name="scrp", bufs=3) as scrp,
            tc.tile_pool(name="smallp", bufs=4) as smallp,
            tc.tile_pool(name="postp", bufs=2) as postp,
            tc.tile_pool(name="pss", bufs=2, space="PSUM") as pss,
            tc.tile_pool(name="psflex", bufs=4, space="PSUM") as psflex,
            tc.tile_pool(name="psmp", bufs=1, space="PSUM") as psmp,
            tc.tile_pool(name="psgp", bufs=1, space="PSUM") as psgp,
        ):
            # DMA policy: one HWDGE queue (sync) carries the critical path
            # in program order (a single queue reaches ~313 GB/s here and
            # multi-queue round-robin measures WORSE); bulky tail-only
            # weights stream on the gpsimd SWDGE queue in parallel.
            def dma(out, in_):
                nc.sync.dma_start(out=out, in_=in_)

            def dma_late(out, in_):
                nc.scalar.dma_start(out=out, in_=in_)

            # ---- constants & weights to SBUF ----
            hp = tc.high_priority()
            hp.__enter__()
            cm_sb = cpool.tile([128, 4, 128], F32)
            dma(cm_sb, cmat)
            id_sb = cm_sb[:, 0, :]
            ones_sb = cm_sb[:, 1, :]
            sh1_sb = cm_sb[:, 2, :]
            sh2_sb = cm_sb[:, 3, :]
            wab_sb = cpool.tile([128, 2, D], F16)
            dma(wab_sb, wab[None, :, :].to_broadcast([128, 2, D]))
            wa_b = wab_sb[0:IPC, 0, :]
            wb_b = wab_sb[:, 1, :]
            bias_sb = cpool.tile([IPC, 5, H], F32)
            dma_late(bias_sb, bias5[None, :, :].to_broadcast([IPC, 5, H]))
            gb_sb = bias_sb[:, 0, :]
            bb_sb = bias_sb[:, 1, :]
            b1_sb = bias_sb[:, 2, :]
            b2_sb = bias_sb[:, 3, :]
            bop_row = bias_sb[0:1, 4, :]
            ones16_sb = cpool.tile([128, 8], F16)
            dma(ones16_sb, ones16)
            poison_sb = cpool.tile([1, 128], F32)
            dma(poison_sb, poison)
            mlt_sb = cpool.tile([128, NCH, IPC], F32)
            dma(mlt_sb, mask_lt)
            mge_sb = cpool.tile([128, NCH, IPC], F16)
            dma(mge_sb, mask_ge)
            xi16_sb = cpool.tile([IPC, D], F16)
            dma(xi16_sb, xi16)
            xlo = cpool.tile([128, NCH, D], F16)
            dma(xlo, xf[0:K].rearrange("(c p) d -> p c d", p=128))
            b_att_col = cpool.tile([IPC, 1], F32)
            dma(b_att_col, b_att[None, :].to_broadcast([IPC, 1]))
            hp.__exit__(None, None, None)
            tc.no_sync_barrier()   # keep pf DMAs behind the critical consts
            # tail-only loads on the slow queue, in rough use order
            dx = cpool.tile([128, NCH, D], F16)
            dma_late(dx, dxf.rearrange("(c p) d -> p c d", p=128))
            WpT_sb = cpool.tile([128, H], F16)
            dma_late(WpT_sb, WpT)
            WoT_sb = cpool.tile([128, NCH, H], F16)
            dma_late(WoT_sb, WoT.rearrange("(c p) h -> p c h", p=128))
            xi_sb = cpool.tile([IPC, D], F32)
            dma_late(xi_sb, xi)
            W1T_sb = cpool.tile([128, NCH, H], F16)
            dma_late(W1T_sb, W1T.rearrange("(c p) h -> p c h", p=128))
            W2T_sb = cpool.tile([128, NCH, H], F16)
            dma_late(W2T_sb, W2T.rearrange("(c p) h -> p c h", p=128))

            eps_col = cpool.tile([IPC, 1], F32)
            nc.vector.memset(eps_col, EPS)

            # ---- sa (this core's rows) and sb (all rows) ----
            scr_sa = smallp.tile([IPC, D], F16)
            nc.vector.tensor_mul(scr_sa, xi16_sb, wa_b)
            sa_col = smallp.tile([IPC, 1], F32)
            nc.vector.tensor_reduce(sa_col, scr_sa, axis=AX.X, op=OP.add)
            nc.vector.tensor_add(sa_col, sa_col, b_att_col)
            sa_diag = smallp.tile([IPC, IPC], F32)
            nc.vector.tensor_mul(sa_diag, id_sb[0:IPC, 0:IPC],
                                 sa_col.to_broadcast([IPC, IPC]))

            sb_cols = smallp.tile([128, NCH], F32)
            for c in range(NCH):
                scr_sb = smallp.tile([128, D], F16)
                nc.vector.tensor_mul(scr_sb, xlo[:, c, :], wb_b)
                nc.vector.tensor_reduce(
                    sb_cols[:, c:c + 1], scr_sb, axis=AX.X, op=OP.add)

            # sb_hi[p, c] = sb[c*128+p+1] via shift matmuls; slot 511 stays 0
            sbhi_ps = pss.tile([128, NCH], F32, tag="ps_small")
            nc.tensor.matmul(sbhi_ps, sh1_sb, sb_cols, start=True, stop=False)
            nc.tensor.matmul(sbhi_ps[:, 0:NCH - 1], sh2_sb, sb_cols[:, 1:NCH],
                             start=False, stop=True)
            sbhi_cols = smallp.tile([128, NCH], F32)
            nc.vector.tensor_copy(sbhi_cols, sbhi_ps)

            # ---- SBJ[t, i] = sa[i] + b_att + sb_hi[t] + mask_lt*(sb_lo-sb_hi),
            #      with -1e9 poison at the t=511 pad slot ----
            sbj = cpool.tile([128, NCH, IPC], F32)
            for c in range(NCH):
                diffc = smallp.tile([128, 1], F32)
                nc.vector.tensor_tensor(
                    diffc, sb_cols[:, c:c + 1], sbhi_cols[:, c:c + 1], OP.subtract)
                diagc = smallp.tile([128, 128], F32)
                nc.vector.tensor_mul(diagc, id_sb, diffc.to_broadcast([128, 128]))
                diagb = smallp.tile([128, 128], F32)
                nc.vector.tensor_mul(
                    diagb, id_sb, sbhi_cols[:, c:c + 1].to_broadcast([128, 128]))
                ps_sbj = pss.tile([128, IPC], F32, tag="ps_small")
                nc.tensor.matmul(ps_sbj, ones_sb[0:IPC, :], sa_diag,
                                 start=True, stop=False)
                nc.tensor.matmul(ps_sbj, diagb, ones_sb[:, 0:IPC],
                                 start=False, stop=False)
                if c == NCH - 1:
                    # poison: sigmoid(-1e9) = 0 exactly, pad row drops out
                    nc.tensor.matmul(ps_sbj, poison_sb, ones_sb[0:1, 0:IPC],
                                     start=False, stop=False)
                nc.tensor.matmul(ps_sbj, diagc, mlt_sb[:, c, :],
                                 start=False, stop=True)
                nc.vector.tensor_copy(sbj[:, c, :], ps_sbj)

            # ---- main edge pass ----
            alpha_full = cpool.tile([128, NCH, IPC], F16)   # raw sigmoid out
            age_full = cpool.tile([128, NCH, IPC], F16)     # masked (t>=i) alpha
            gx_ps = psgp.tile([IPC, D], F32)                # sum_t a*x[j]
            msg_ps = psmp.tile([IPC, H], F32)
            s_ps = pss.tile([1, IPC], F32, tag="ps_small")
            # U quad rows: bank b, partition slot 32s..32s+3 holds i=16b+4s+j
            u_ps = [psflex.tile([128, 512], F32, tag="flex", name=f"u_ps{b}")
                    for b in range(4)]
            for b in range(4):
                nc.vector.memset(u_ps[b], 0.0)

            for c in range(NCH):
                pf_t = pfp.tile([128, IB, PD], F16)
                dma(pf_t, pf[c, :, :, :])
                # sc = sum_pd pf_sent (wc pre-folded): fp16 halving tree
                scr = scrp.tile([128, IB, 64], F16)
                nc.vector.tensor_add(scr, pf_t[:, :, 0:64], pf_t[:, :, 64:128])
                w = 32
                while w >= 2:
                    nc.vector.tensor_add(
                        scr[:, :, 0:w], scr[:, :, 0:w], scr[:, :, w:2 * w])
                    w //= 2
                sc_t = smallp.tile([128, IB], F32)
                nc.vector.tensor_add(sc_t, scr[:, :, 0], scr[:, :, 1])
                aarg = smallp.tile([128, IB], F32)
                nc.vector.tensor_add(aarg, sc_t, sbj[:, c, :])
                nc.scalar.activation(alpha_full[:, c, :], aarg, AF.Sigmoid)
                nc.vector.tensor_mul(age_full[:, c, :], alpha_full[:, c, :],
                                     mge_sb[:, c, :])
                # U quads: lhsT = 4 alpha columns, rhs = 4 pf blocks; the
                # wanted rows sit on the diagonal (gathered via DRAM AP)
                for q in range(IB // 4):
                    b, sp = divmod(q, 4)
                    nc.tensor.matmul(
                        u_ps[b][32 * sp:32 * sp + 4, :],
                        alpha_full[:, c, 4 * q:4 * q + 4],
                        pf_t[:, 4 * q:4 * q + 4, :],
                        start=(c == 0), stop=(c == NCH - 1),
                        tile_position=(0, 32 * sp))
                nc.tensor.matmul(gx_ps, alpha_full[:, c, :], xlo[:, c, :],
                                 start=(c == 0), stop=False)
                nc.tensor.matmul(s_ps, ones16_sb[:, 0:1], alpha_full[:, c, :],
                                 start=(c == 0), stop=(c == NCH - 1))

            # scheduler fence: keep every tail instruction after the loop in
            # each engine stream (strict-FIFO engines head-of-line block if
            # e.g. an LN Sqrt lands between loop sigmoids in the ACT queue)
            tc.no_sync_barrier()

            # G2 (shifted-x correction) after the loop: dx arrives on the slow
            # queue and age_full persists, so this overlaps the loop tail
            for c in range(NCH):
                nc.tensor.matmul(gx_ps, age_full[:, c, :], dx[:, c, :],
                                 start=False, stop=(c == NCH - 1))

            # ---- messages = U@WpT + Gx@WoT + s_alpha x bop ----
            s_row = smallp.tile([1, IPC], F32)
            nc.vector.tensor_copy(s_row, s_ps)

            # U reassembly on-chip: bank copy -> PE transpose of each
            # 128-col block (diagonal quad becomes free-strided columns) ->
            # tiny strided copies assemble UT directly; no DRAM bounce.
            u_sb = postp.tile([128, IPC], F16)
            for b in range(4):
                u_cp = postp.tile([128, 512], F32, tag="u_cp")
                nc.vector.tensor_copy(u_cp, u_ps[b])
                for j in range(4):
                    ptu = pss.tile([128, 128], F32, tag="ps_small")
                    nc.tensor.transpose(ptu, u_cp[:, j * 128:(j + 1) * 128],
                                        id_sb)
                    # cols {j, 32+j, 64+j, 96+j} hold U rows i=16b+4s+j
                    src_ap = ptu.rearrange("p (s q) -> p s q", q=32)[:, :, j]
                    dst_ap = u_sb.rearrange("p (r s f) -> p r s f", r=4, s=4)[
                        :, b, :, j]
                    nc.vector.tensor_copy(dst_ap, src_ap)

            gx_sb = postp.tile([IPC, D], F32)
            nc.vector.tensor_copy(gx_sb, gx_ps)
            gxT = postp.tile([128, NCH, IPC], F16)
            for c in range(NCH):
                ptg = pss.tile([128, IPC], F32, tag="ps_small")
                nc.tensor.transpose(ptg, gx_sb[:, c * 128:(c + 1) * 128],
                                    id_sb[0:IPC, 0:IPC])
                nc.vector.tensor_copy(gxT[:, c, :], ptg)

            nc.tensor.matmul(msg_ps, u_sb, WpT_sb, start=True, stop=False)
            for c in range(NCH):
                nc.tensor.matmul(msg_ps, gxT[:, c, :], WoT_sb[:, c, :],
                                 start=False, stop=False)
            nc.tensor.matmul(msg_ps, s_row, bop_row, start=False, stop=True)

            # ---- residual + LN1 ----
            def layer_norm(v):
                stats = smallp.tile([IPC, 6], F32)
                nc.vector.bn_stats(out=stats, in_=v)
                mv = smallp.tile([IPC, 2], F32)
                nc.vector.bn_aggr(out=mv, in_=stats)
                std = smallp.tile([IPC, 1], F32)
                nc.scalar.activation(std, mv[:, 1:2], AF.Sqrt, bias=eps_col)
                rstd = smallp.tile([IPC, 1], F32)
                nc.vector.reciprocal(rstd, std)
                cen = postp.tile([IPC, H], F32)
                nc.vector.tensor_scalar(cen, v, mv[:, 0:1], rstd,
                                        OP.subtract, OP.mult)
                o = postp.tile([IPC, H], F32)
                nc.vector.tensor_mul(o, cen, gb_sb)
                nc.vector.tensor_add(o, o, bb_sb)
                return o

            h_sb = postp.tile([IPC, H], F32)
            nc.vector.tensor_add(h_sb, xi_sb, msg_ps)
            out1 = layer_norm(h_sb)

            # ---- FFN ----
            def transpose_rows(v):
                vT = postp.tile([128, NCH, IPC], F16, tag="vT")
                for c in range(NCH):
                    ptt = pss.tile([128, IPC], F32, tag="ps_small")
                    nc.tensor.transpose(ptt, v[:, c * 128:(c + 1) * 128],
                                        id_sb[0:IPC, 0:IPC])
                    nc.vector.tensor_copy(vT[:, c, :], ptt)
                return vT

            o1T = transpose_rows(out1)
            f1_ps = psflex.tile([IPC, H], F32, tag="flex")
            for c in range(NCH):
                nc.tensor.matmul(f1_ps, o1T[:, c, :], W1T_sb[:, c, :],
                                 start=(c == 0), stop=(c == NCH - 1))
            f1 = postp.tile([IPC, H], F32)
            nc.vector.tensor_add(f1, f1_ps, b1_sb)
            nc.vector.tensor_scalar_max(f1, f1, 0.0)

            f1T = transpose_rows(f1)
            f2_ps = psflex.tile([IPC, H], F32, tag="flex")
            for c in range(NCH):
                nc.tensor.matmul(f2_ps, f1T[:, c, :], W2T_sb[:, c, :],
                                 start=(c == 0), stop=(c == NCH - 1))
            h2 = postp.tile([IPC, H], F32)
            nc.vector.tensor_add(h2, f2_ps, b2_sb)
            nc.vector.tensor_add(h2, h2, out1)
            out2 = layer_norm(h2)

            nc.sync.dma_start(out=out_d, in_=out2)

    return nc


def _poison() -> np.ndarray:
    p = np.zeros((1, 128), np.float32)
    p[0, 127] = -1e9
    return p


def _cmat() -> np.ndarray:
    c = np.zeros((128, 4, 128), np.float32)
    c[:, 0, :] = np.eye(128)
    c[:, 1, :] = 1.0
    c[:, 2, :] = np.eye(128, k=-1)     # shift1[q, p] = (q == p+1)
    c[0, 3, 127] = 1.0                  # shift2[q, p] = (q==0)&(p==127)
    return c


def prep_in_maps(inputs) -> list[dict]:
    x = np.asarray(inputs["x"], np.float32)
    pf = np.asarray(inputs["pair_feats"], np.float32)
    W_att = np.asarray(inputs["W_att"], np.float32)
    b_att = np.asarray(inputs["b_att"], np.float32)
    W_obj = np.asarray(inputs["W_obj"], np.float32)
    b_obj = np.asarray(inputs["b_obj"], np.float32)
    W_pair = np.asarray(inputs["W_pair"], np.float32)
    b_pair = np.asarray(inputs["b_pair"], np.float32)
    ln_g = np.asarray(inputs["ln_g"], np.float32)
    ln_b = np.asarray(inputs["ln_b"], np.float32)
    W1 = np.asarray(inputs["W1"], np.float32)
    b1 = np.asarray(inputs["b1"], np.float32)
    W2 = np.asarray(inputs["W2"], np.float32)
    b2 = np.asarray(inputs["b2"], np.float32)

    wa, wb, wc = W_att[0, :D], W_att[0, D:2 * D], W_att[0, 2 * D:]
    xpad = np.concatenate([x, np.zeros((1, D), np.float32)], axis=0)

    # fold wc into pf columns; recover U via pre-divided W_pair.T rows.
    colscale = np.sign(wc) * np.maximum(np.abs(wc), 6e-5)
    colscale[colscale == 0] = 6e-5
    # 1/511 (the mean over neighbors) is folded into the three weight paths
    # that consume raw alpha: U@WpT, (A@x)@WoT, and s_alpha*bop.
    WpT2 = (W_pair.T / colscale[:, None] / T).astype(np.float16)
    WoT2 = (W_obj.T / T).astype(np.float16)
    dxf = np.diff(xpad[:K + 1], axis=0)

    base = dict(
        xf=xpad.astype(np.float16),
        dxf=dxf.astype(np.float16),
        cmat=_cmat(),
        ones16=np.ones((128, 8), np.float16),
        poison=_poison(),
        wab=np.stack([wa, wb]).astype(np.float16),
        b_att=b_att.astype(np.float32),
        bias5=np.stack([ln_g, ln_b, b1, b2,
                        (b_obj + b_pair) / T]).astype(np.float32),
        WpT=np.ascontiguousarray(WpT2),
        WoT=np.ascontiguousarray(WoT2),
        W1T=np.ascontiguousarray(W1.T).astype(np.float16),
        W2T=np.ascontiguousarray(W2.T).astype(np.float16),
    )

    pfr = pf.reshape(K, T, PD)
    tgrid = np.arange(128)[:, None] + 128 * np.arange(NCH)[None, :]   # [128, NCH]

    in_maps = []
    for core in range(NCORES):
        ig = np.arange(core * IPC, (core + 1) * IPC)
        mlt = (tgrid[:, :, None] < ig[None, None, :]).astype(np.float32)
        mge = ((tgrid[:, :, None] >= ig[None, None, :])
               & (tgrid[:, :, None] <= T - 1)).astype(np.float16)
        # [chunk, t, i, pd] layout -> each tile DMA is one contiguous burst
        shard = np.zeros((NCH * 128, IPC, PD), np.float16)
        shard[:T] = (pfr[ig] * colscale[None, None, :]).transpose(1, 0, 2)
        xi = x[ig]
        m = dict(base)
        m.update(
            pf=shard.reshape(NCH, 128, IPC, PD),
            xi=xi.astype(np.float32),
            xi16=xi.astype(np.float16),
            mask_lt=mlt,
            mask_ge=mge,
        )
        in_maps.append(m)
    return in_maps


_COMPILED = None


def _get_program() -> bacc.Bacc:
    global _COMPILED
    if _COMPILED is None:
        nc = build_program()
        nc.compile()
        _COMPILED = nc
    return _COMPILED


TRACE = False
LAST_RESULT = None


def _install_axon_ntff_hook():
    """The container's antenv lacks axon_hooks; recreate it from trn_boot's
    ctypes implementation so trace=True can capture NTFF profiles."""
    import sys
    import types
    try:
        from antenv.axon_hooks import get_axon_ntff_profile_hook  # noqa: F401
        return
    except ImportError:
        pass
    from trn_agent_boot.trn_boot import _ntff_profile_via_ctypes
    hook = _ntff_profile_via_ctypes("/opt/axon/libaxon_pjrt.so")
    m = types.ModuleType("antenv.axon_hooks")
    m.get_axon_ntff_profile_hook = lambda: hook
    sys.modules["antenv.axon_hooks"] = m


def kernel(**inputs) -> np.ndarray:
    import concourse.bass_utils as bu
    from concourse.bass_utils import run_bass_kernel_spmd
    global LAST_RESULT
    if TRACE:
        _install_axon_ntff_hook()
        bu.upload_artifacts = lambda tmpdir: str(tmpdir)  # no bucket here
    nc = _get_program()
    in_maps = prep_in_maps(inputs)
    res = run_bass_kernel_spmd(nc, in_maps, list(range(NCORES)), trace=TRACE)
    LAST_RESULT = res
    outs = [res.results[c]["out"] for c in range(NCORES)]
    return np.concatenate(outs, axis=0).astype(np.float32)


# revision 42
# speedup vs baseline: 1.4801x; 1.0734x over previous
"""Trainium2 Bass kernel for nn_ARTLayer (gnn_message_passing).

Math (reference):
    j(i,t) = t + (t>=i)                                    # [K, K-1] neighbor index
    alpha  = sigmoid(x@wa [i] + x@wb [j] + pf@wc + b_att)  # [K, K-1]
    msgs   = mean_t alpha * ((x@WobjT + b_obj)[j] + pf@WpairT + b_pair)
    out    = LN(x + msgs); out = LN(out + FFN(out))

Key algebraic rewrite (removes the 34-GFLOP [P,PD]x[PD,H] einsum):
    sum_t a*(pf@WpT)  = (sum_t a*pf) @ WpT               -> U[i,:] @ WpT
    sum_t a*oj[j]     = (A @ x) @ WobjT                  -> Gx[i,:] @ WoT
      with A[i,j] decomposed via lo/hi shifted views of x and a t>=i mask
    sum_t a*(b_obj+b_pair) = s_alpha[i] * bop

Sharding: rows i split across 8 cores (64 each); small tensors replicated;
host concatenates the per-core [64, 512] outputs.

Implementation notes (driven by NTFF profiles):
  - wc is folded into pf on the host (column scales, floored at fp16
    min-normal); sc becomes a pure fp16 halving-tree add-reduce and U is
    recovered exactly via W_pairT rows pre-divided by the scales.
  - 1/511 (the neighbor mean) is folded into WpT/WoT/bop on the host, and
    the t=511 pad slot is poisoned with -1e9 pre-sigmoid, so raw sigmoid
    output is used directly with no mask/scale multiplies.
  - U accumulation runs as M=4 quad matmuls (alpha quad stationary, four pf
    blocks streaming at N=512); the wanted rows sit on the block diagonal
    and are gathered by a stride-640 DRAM access pattern after a bank dump.
  - pf is re-laid-out on the host to [chunk, t, i, pd] so each tile DMA is
    one fully-contiguous 8KB-per-partition burst, and all DMAs are spread
    round-robin over the three DMA-capable engines (sync/scalar HWDGE,
    gpsimd SWDGE) instead of serializing on one queue.
"""
import numpy as np

import concourse.bass as bass
import concourse.tile as tile
from concourse import bacc, mybir

F32, F16 = mybir.dt.float32, mybir.dt.float16
AX = mybir.AxisListType
OP = mybir.AluOpType
AF = mybir.ActivationFunctionType

K, D, H, PD = 512, 512, 512, 128
T = K - 1                      # 511 neighbors per row
NCORES, IPC = 8, 64            # rows per core
NCH = 4                        # t-chunks of 128 (last chunk row 127 is t=511 pad)
IB, NIB = 64, 1                # i-block within a core
EPS = 1e-5


def build_program() -> bacc.Bacc:
    nc = bacc.Bacc("TRN2", target_bir_lowering=False, debug=False)

    def inp(name, shape, dt):
        return nc.dram_tensor(name, shape, dt, kind="ExternalInput").ap()

    pf = inp("pf", [NCH, 128, IPC, PD], F16)    # [chunk, t-in-chunk, i, pd]
    # single-DMA packed critical constants (128 contiguous rows each):
    # b32: [cmat 0:512 | poison 512:640 | mask_lt 640:896 | b_att 896:904]
    # b16: [wa_t 0:512 | wb_t 512:1024 | mask_ge 1024:1280 | ones 1280:1288
    #       | xi16 1288:1800]
    b32 = inp("b32", [128, 904], F32)
    b16 = inp("b16", [128, 1800], F16)
    xlo_ch = inp("xlo_ch", [128, NCH, D], F16)  # x rows chunked [t%128, t//128]
    dxf = inp("dxf", [K, D], F16)               # x[t+1] - x[t], host computed
    xi = inp("xi", [IPC, D], F32)               # this core's rows of x
    bias5 = inp("bias5", [5, H], F32)           # [ln_g; ln_b; b1; b2; bop/511]
    WpT = inp("WpT", [PD, H], F16)              # W_pair.T / colscale / 511
    WoT = inp("WoT", [D, H], F16)               # W_obj.T / 511
    W1T = inp("W1T", [H, H], F16)
    W2T = inp("W2T", [H, H], F16)

    out_d = nc.dram_tensor("out", [IPC, H], F32, kind="ExternalOutput").ap()

    with tile.TileContext(nc) as tc:
        with (
            tc.tile_pool(name="const", bufs=1) as cpool,
            tc.tile_pool(name="pfp", bufs=4) as pfp,
            tc.tile_pool(name="scrp", bufs=3) as scrp,
            tc.tile_pool(name="smallp", bufs=4) as smallp,
            tc.tile_pool(name="postp", bufs=2) as postp,
            tc.tile_pool(name="pss", bufs=2, space="PSUM") as pss,
            tc.tile_pool(name="psflex", bufs=4, space="PSUM") as psflex,
            tc.tile_pool(name="psmp", bufs=1, space="PSUM") as psmp,
            tc.tile_pool(name="psgp", bufs=1, space="PSUM") as psgp,
        ):
            # DMA policy: one HWDGE queue (sync) carries the critical path
            # in program order (a single queue reaches ~313 GB/s here and
            # multi-queue round-robin measures WORSE); bulky tail-only
            # weights stream on the gpsimd SWDGE queue in parallel.
            def dma(out, in_):
                nc.sync.dma_start(out=out, in_=in_)

            def dma_late(out, in_):
                nc.scalar.dma_start(out=out, in_=in_)

            # ---- constants & weights to SBUF (3 packed critical DMAs) ----
            hp = tc.high_priority()
            hp.__enter__()
            b32_sb = cpool.tile([128, 904], F32)
            dma(b32_sb, b32)
            b16_sb = cpool.tile([128, 1800], F16)
            dma(b16_sb, b16)
            xlo = cpool.tile([128, NCH, D], F16)
            dma(xlo, xlo_ch)
            hp.__exit__(None, None, None)
            tc.no_sync_barrier()   # keep pf DMAs behind the critical consts
            id_sb = b32_sb[:, 0:128]
            ones_sb = b32_sb[:, 128:256]
            sh1_sb = b32_sb[:, 256:384]
            sh2_sb = b32_sb[:, 384:512]
            poison_sb = b32_sb[0:1, 512:640]
            mlt_sb = b32_sb[:, 640:896].rearrange("p (c i) -> p c i", c=NCH)
            b_att_col = b32_sb[0:IPC, 896:897]
            wa_b = b16_sb[0:IPC, 0:512]
            wb_b = b16_sb[:, 512:1024]
            mge_sb = b16_sb[:, 1024:1280].rearrange("p (c i) -> p c i", c=NCH)
            ones16_sb = b16_sb[:, 1280:1288]
            xi16_sb = b16_sb[0:IPC, 1288:1800]
            bias_sb = cpool.tile([IPC, 5, H], F32)
            dma_late(bias_sb, bias5[None, :, :].to_broadcast([IPC, 5, H]))
            gb_sb = bias_sb[:, 0, :]
            bb_sb = bias_sb[:, 1, :]
            b1_sb = bias_sb[:, 2, :]
            b2_sb = bias_sb[:, 3, :]
            bop_row = bias_sb[0:1, 4, :]
            # tail-only loads on the slow queue, in rough use order
            dx = cpool.tile([128, NCH, D], F16)
            dma_late(dx, dxf.rearrange("(c p) d -> p c d", p=128))
            WpT_sb = cpool.tile([128, H], F16)
            dma_late(WpT_sb, WpT)
            WoT_sb = cpool.tile([128, NCH, H], F16)
            dma_late(WoT_sb, WoT.rearrange("(c p) h -> p c h", p=128))
            xi_sb = cpool.tile([IPC, D], F32)
            dma_late(xi_sb, xi)
            W1T_sb = cpool.tile([128, NCH, H], F16)
            dma_late(W1T_sb, W1T.rearrange("(c p) h -> p c h", p=128))
            W2T_sb = cpool.tile([128, NCH, H], F16)
            dma_late(W2T_sb, W2T.rearrange("(c p) h -> p c h", p=128))

            eps_col = cpool.tile([IPC, 1], F32)
            nc.vector.memset(eps_col, EPS)

            # ---- sa (this core's rows) and sb (all rows) ----
            scr_sa = smallp.tile([IPC, D], F16)
            nc.vector.tensor_mul(scr_sa, xi16_sb, wa_b)
            sa_col = smallp.tile([IPC, 1], F32)
            nc.vector.tensor_reduce(sa_col, scr_sa, axis=AX.X, op=OP.add)
            nc.vector.tensor_add(sa_col, sa_col, b_att_col)
            sa_diag = smallp.tile([IPC, IPC], F32)
            nc.vector.tensor_mul(sa_diag, id_sb[0:IPC, 0:IPC],
                                 sa_col.to_broadcast([IPC, IPC]))

            sb_cols = smallp.tile([128, NCH], F32)
            for c in range(NCH):
                scr_sb = smallp.tile([128, D], F16)
                nc.vector.tensor_mul(scr_sb, xlo[:, c, :], wb_b)
                nc.vector.tensor_reduce(
                    sb_cols[:, c:c + 1], scr_sb, axis=AX.X, op=OP.add)

            # sb_hi[p, c] = sb[c*128+p+1] via shift matmuls; slot 511 stays 0
            sbhi_ps = pss.tile([128, NCH], F32, tag="ps_small")
            nc.tensor.matmul(sbhi_ps, sh1_sb, sb_cols, start=True, stop=False)
            nc.tensor.matmul(sbhi_ps[:, 0:NCH - 1], sh2_sb, sb_cols[:, 1:NCH],
                             start=False, stop=True)
            sbhi_cols = smallp.tile([128, NCH], F32)
            nc.vector.tensor_copy(sbhi_cols, sbhi_ps)

            # ---- SBJ[t, i] = sa[i] + b_att + sb_hi[t] + mask_lt*(sb_lo-sb_hi),
            #      with -1e9 poison at the t=511 pad slot ----
            sbj = cpool.tile([128, NCH, IPC], F32)
            for c in range(NCH):
                diffc = smallp.tile([128, 1], F32)
                nc.vector.tensor_tensor(
                    diffc, sb_cols[:, c:c + 1], sbhi_cols[:, c:c + 1], OP.subtract)
                diagc = smallp.tile([128, 128], F32)
                nc.vector.tensor_mul(diagc, id_sb, diffc.to_broadcast([128, 128]))
                diagb = smallp.tile([128, 128], F32)
                nc.vector.tensor_mul(
                    diagb, id_sb, sbhi_cols[:, c:c + 1].to_broadcast([128, 128]))
                ps_sbj = pss.tile([128, IPC], F32, tag="ps_small")
                nc.tensor.matmul(ps_sbj, ones_sb[0:IPC, :], sa_diag,
                                 start=True, stop=False)
                nc.tensor.matmul(ps_sbj, diagb, ones_sb[:, 0:IPC],
                                 start=False, stop=False)
                if c == NCH - 1:
                    # poison: sigmoid(-1e9) = 0 exactly, pad row drops out
                    nc.tensor.matmul(ps_sbj, poison_sb, ones_sb[0:1, 0:IPC],
                                     start=False, stop=False)
                nc.tensor.matmul(ps_sbj, diagc, mlt_sb[:, c, :],
                                 start=False, stop=True)
                nc.vector.tensor_copy(sbj[:, c, :], ps_sbj)

            # ---- main edge pass ----
            alpha_full = cpool.tile([128, NCH, IPC], F16)   # raw sigmoid out
            age_full = cpool.tile([128, NCH, IPC], F16)     # masked (t>=i) alpha
            gx_ps = psgp.tile([IPC, D], F32)                # sum_t a*x[j]
            msg_ps = psmp.tile([IPC, H], F32)
            s_ps = pss.tile([1, IPC], F32, tag="ps_small")
            # U quad rows: bank b, partition slot 32s..32s+3 holds i=16b+4s+j
            u_ps = [psflex.tile([128, 512], F32, tag="flex", name=f"u_ps{b}")
                    for b in range(4)]
            for b in range(4):
                nc.vector.memset(u_ps[b], 0.0)

            for c in range(NCH):
                pf_t = pfp.tile([128, IB, PD], F16)
                dma(pf_t, pf[c, :, :, :])
                # sc = sum_pd pf_sent (wc pre-folded): fp16 halving tree
                scr = scrp.tile([128, IB, 64], F16)
                nc.vector.tensor_add(scr, pf_t[:, :, 0:64], pf_t[:, :, 64:128])
                w = 32
                while w >= 2:
                    nc.vector.tensor_add(
                        scr[:, :, 0:w], scr[:, :, 0:w], scr[:, :, w:2 * w])
                    w //= 2
                sc_t = smallp.tile([128, IB], F32)
                nc.vector.tensor_add(sc_t, scr[:, :, 0], scr[:, :, 1])
                aarg = smallp.tile([128, IB], F32)
                nc.vector.tensor_add(aarg, sc_t, sbj[:, c, :])
                nc.scalar.activation(alpha_full[:, c, :], aarg, AF.Sigmoid)
                nc.vector.tensor_mul(age_full[:, c, :], alpha_full[:, c, :],
                                     mge_sb[:, c, :])
                # U quads: lhsT = 4 alpha columns, rhs = 4 pf blocks; the
                # wanted rows sit on the diagonal (gathered via DRAM AP)
                for q in range(IB // 4):
                    b, sp = divmod(q, 4)
                    nc.tensor.matmul(
                        u_ps[b][32 * sp:32 * sp + 4, :],
                        alpha_full[:, c, 4 * q:4 * q + 4],
                        pf_t[:, 4 * q:4 * q + 4, :],
                        start=(c == 0), stop=(c == NCH - 1),
                        tile_position=(0, 32 * sp))
                nc.tensor.matmul(gx_ps, alpha_full[:, c, :], xlo[:, c, :],
                                 start=(c == 0), stop=False)
                nc.tensor.matmul(s_ps, ones16_sb[:, 0:1], alpha_full[:, c, :],
                                 start=(c == 0), stop=(c == NCH - 1))

            # scheduler fence: keep every tail instruction after the loop in
            # each engine stream (strict-FIFO engines head-of-line block if
            # e.g. an LN Sqrt lands between loop sigmoids in the ACT queue)
            tc.no_sync_barrier()

            # G2 (shifted-x correction) after the loop: dx arrives on the slow
            # queue and age_full persists, so this overlaps the loop tail
            for c in range(NCH):
                nc.tensor.matmul(gx_ps, age_full[:, c, :], dx[:, c, :],
                                 start=False, stop=(c == NCH - 1))

            # ---- messages = U@WpT + Gx@WoT + s_alpha x bop ----
            s_row = smallp.tile([1, IPC], F32)
            nc.vector.tensor_copy(s_row, s_ps)

            # U reassembly on-chip: bank copy -> PE transpose of each
            # 128-col block (diagonal quad becomes free-strided columns) ->
            # tiny strided copies assemble UT directly; no DRAM bounce.
            u_sb = postp.tile([128, IPC], F16)
            for b in range(4):
                u_cp = postp.tile([128, 512], F32, tag="u_cp")
                nc.vector.tensor_copy(u_cp, u_ps[b])
                for j in range(4):
                    ptu = pss.tile([128, 128], F32, tag="ps_small")
                    nc.tensor.transpose(ptu, u_cp[:, j * 128:(j + 1) * 128],
                                        id_sb)
                    # cols {j, 32+j, 64+j, 96+j} hold U rows i=16b+4s+j
                    src_ap = ptu.rearrange("p (s q) -> p s q", q=32)[:, :, j]
                    dst_ap = u_sb.rearrange("p (r s f) -> p r s f", r=4, s=4)[
                        :, b, :, j]
                    nc.vector.tensor_copy(dst_ap, src_ap)

            gx_sb = postp.tile([IPC, D], F32)
            nc.vector.tensor_copy(gx_sb, gx_ps)
            gxT = postp.tile([128, NCH, IPC], F16)
            for c in range(NCH):
                ptg = pss.tile([128, IPC], F32, tag="ps_small")
                nc.tensor.transpose(ptg, gx_sb[:, c * 128:(c + 1) * 128],
                                    id_sb[0:IPC, 0:IPC])
                nc.vector.tensor_copy(gxT[:, c, :], ptg)

            nc.tensor.matmul(msg_ps, u_sb, WpT_sb, start=True, stop=False)
            for c in range(NCH):
                nc.tensor.matmul(msg_ps, gxT[:, c, :], WoT_sb[:, c, :],
                                 start=False, stop=False)
            nc.tensor.matmul(msg_ps, s_row, bop_row, start=False, stop=True)

            # ---- residual + LN1 ----
            def layer_norm(v):
                stats = smallp.tile([IPC, 6], F32)
                nc.vector.bn_stats(out=stats, in_=v)
                mv = smallp.tile([IPC, 2], F32)
                nc.vector.bn_aggr(out=mv, in_=stats)
                std = smallp.tile([IPC, 1], F32)
                nc.scalar.activation(std, mv[:, 1:2], AF.Sqrt, bias=eps_col)
                rstd = smallp.tile([IPC, 1], F32)
                nc.vector.reciprocal(rstd, std)
                cen = postp.tile([IPC, H], F32)
                nc.vector.tensor_scalar(cen, v, mv[:, 0:1], rstd,
                                        OP.subtract, OP.mult)
                o = postp.tile([IPC, H], F32)
                nc.vector.tensor_mul(o, cen, gb_sb)
                nc.vector.tensor_add(o, o, bb_sb)
                return o

            h_sb = postp.tile([IPC, H], F32)
            nc.vector.tensor_add(h_sb, xi_sb, msg_ps)
            out1 = layer_norm(h_sb)

            # ---- FFN ----
            def transpose_rows(v):
                vT = postp.tile([128, NCH, IPC], F16, tag="vT")
                for c in range(NCH):
                    ptt = pss.tile([128, IPC], F32, tag="ps_small")
                    nc.tensor.transpose(ptt, v[:, c * 128:(c + 1) * 128],
                                        id_sb[0:IPC, 0:IPC])
                    nc.vector.tensor_copy(vT[:, c, :], ptt)
                return vT

            o1T = transpose_rows(out1)
            f1_ps = psflex.tile([IPC, H], F32, tag="flex")
            for c in range(NCH):
                nc.tensor.matmul(f1_ps, o1T[:, c, :], W1T_sb[:, c, :],
                                 start=(c == 0), stop=(c == NCH - 1))
            f1 = postp.tile([IPC, H], F32)
            nc.vector.tensor_add(f1, f1_ps, b1_sb)
            nc.vector.tensor_scalar_max(f1, f1, 0.0)

            f1T = transpose_rows(f1)
            f2_ps = psflex.tile([IPC, H], F32, tag="flex")
            for c in range(NCH):
                nc.tensor.matmul(f2_ps, f1T[:, c, :], W2T_sb[:, c, :],
                                 start=(c == 0), stop=(c == NCH - 1))
            h2 = postp.tile([IPC, H], F32)
            nc.vector.tensor_add(h2, f2_ps, b2_sb)
            nc.vector.tensor_add(h2, h2, out1)
            out2 = layer_norm(h2)

            nc.sync.dma_start(out=out_d, in_=out2)

    return nc


def _poison() -> np.ndarray:
    p = np.zeros((1, 128), np.float32)
    p[0, 127] = -1e9
    return p


def _cmat() -> np.ndarray:
    c = np.zeros((128, 4, 128), np.float32)
    c[:, 0, :] = np.eye(128)
    c[:, 1, :] = 1.0
    c[:, 2, :] = np.eye(128, k=-1)     # shift1[q, p] = (q == p+1)
    c[0, 3, 127] = 1.0                  # shift2[q, p] = (q==0)&(p==127)
    return c


def prep_in_maps(inputs) -> list[dict]:
    x = np.asarray(inputs["x"], np.float32)
    pf = np.asarray(inputs["pair_feats"], np.float32)
    W_att = np.asarray(inputs["W_att"], np.float32)
    b_att = np.asarray(inputs["b_att"], np.float32)
    W_obj = np.asarray(inputs["W_obj"], np.float32)
    b_obj = np.asarray(inputs["b_obj"], np.float32)
    W_pair = np.asarray(inputs["W_pair"], np.float32)
    b_pair = np.asarray(inputs["b_pair"], np.float32)
    ln_g = np.asarray(inputs["ln_g"], np.float32)
    ln_b = np.asarray(inputs["ln_b"], np.float32)
    W1 = np.asarray(inputs["W1"], np.float32)
    b1 = np.asarray(inputs["b1"], np.float32)
    W2 = np.asarray(inputs["W2"], np.float32)
    b2 = np.asarray(inputs["b2"], np.float32)

    wa, wb, wc = W_att[0, :D], W_att[0, D:2 * D], W_att[0, 2 * D:]
    xpad = np.concatenate([x, np.zeros((1, D), np.float32)], axis=0)

    # fold wc into pf columns; recover U via pre-divided W_pair.T rows.
    colscale = np.sign(wc) * np.maximum(np.abs(wc), 6e-5)
    colscale[colscale == 0] = 6e-5
    # 1/511 (the mean over neighbors) is folded into the three weight paths
    # that consume raw alpha: U@WpT, (A@x)@WoT, and s_alpha*bop.
    WpT2 = (W_pair.T / colscale[:, None] / T).astype(np.float16)
    WoT2 = (W_obj.T / T).astype(np.float16)
    dxf = np.diff(xpad[:K + 1], axis=0)

    b32a = np.zeros((128, 904), np.float32)
    b32a[:, 0:512] = _cmat().reshape(128, 512)
    b32a[0, 512 + 127] = -1e9
    b32a[:, 896] = b_att[0]
    b16a = np.zeros((128, 1800), np.float16)
    b16a[:, 0:512] = wa[None, :]
    b16a[:, 512:1024] = wb[None, :]
    b16a[:, 1280:1288] = 1.0
    xlo_np = np.ascontiguousarray(
        x.reshape(NCH, 128, D).transpose(1, 0, 2)).astype(np.float16)

    base = dict(
        xlo_ch=xlo_np,
        dxf=dxf.astype(np.float16),
        bias5=np.stack([ln_g, ln_b, b1, b2,
                        (b_obj + b_pair) / T]).astype(np.float32),
        WpT=np.ascontiguousarray(WpT2),
        WoT=np.ascontiguousarray(WoT2),
        W1T=np.ascontiguousarray(W1.T).astype(np.float16),
        W2T=np.ascontiguousarray(W2.T).astype(np.float16),
    )

    pfr = pf.reshape(K, T, PD)
    tgrid = np.arange(128)[:, None] + 128 * np.arange(NCH)[None, :]   # [128, NCH]

    in_maps = []
    for core in range(NCORES):
        ig = np.arange(core * IPC, (core + 1) * IPC)
        mlt = (tgrid[:, :, None] < ig[None, None, :]).astype(np.float32)
        mge = ((tgrid[:, :, None] >= ig[None, None, :])
               & (tgrid[:, :, None] <= T - 1)).astype(np.float16)
        # [chunk, t, i, pd] layout -> each tile DMA is one contiguous burst
        shard = np.zeros((NCH * 128, IPC, PD), np.float16)
        shard[:T] = (pfr[ig] * colscale[None, None, :]).transpose(1, 0, 2)
        xi = x[ig]
        cb32 = b32a.copy()
        cb32[:, 640:896] = mlt.reshape(128, NCH * IPC)
        cb16 = b16a.copy()
        cb16[:, 1024:1280] = mge.reshape(128, NCH * IPC)
        cb16[0:IPC, 1288:1800] = xi.astype(np.float16)
        m = dict(base)
        m.update(
            pf=shard.reshape(NCH, 128, IPC, PD),
            xi=xi.astype(np.float32),
            b32=cb32,
            b16=cb16,
        )
        in_maps.append(m)
    return in_maps


_COMPILED = None


def _get_program() -> bacc.Bacc:
    global _COMPILED
    if _COMPILED is None:
        nc = build_program()
        nc.compile()
        _COMPILED = nc
    return _COMPILED


TRACE = False
LAST_RESULT = None


def _install_axon_ntff_hook():
    """The container's antenv lacks axon_hooks; recreate it from trn_boot's
    ctypes implementation so trace=True can capture NTFF profiles."""
    import sys
    import types
    try:
        from antenv.axon_hooks import get_axon_ntff_profile_hook  # noqa: F401
        return
    except ImportError:
        pass
    from trn_agent_boot.trn_boot import _ntff_profile_via_ctypes
    hook = _ntff_profile_via_ctypes("/opt/axon/libaxon_pjrt.so")
    m = types.ModuleType("antenv.axon_hooks")
    m.get_axon_ntff_profile_hook = lambda: hook
    sys.modules["antenv.axon_hooks"] = m


def kernel(**inputs) -> np.ndarray:
    import concourse.bass_utils as bu
    from concourse.bass_utils import run_bass_kernel_spmd
    global LAST_RESULT
    if TRACE:
        _install_axon_ntff_hook()
        bu.upload_artifacts = lambda tmpdir: str(tmpdir)  # no bucket here
    nc = _get_program()
    in_maps = prep_in_maps(inputs)
    res = run_bass_kernel_spmd(nc, in_maps, list(range(NCORES)), trace=TRACE)
    LAST_RESULT = res
    outs = [res.results[c]["out"] for c in range(NCORES)]
    return np.concatenate(outs, axis=0).astype(np.float32)


# revision 43
# speedup vs baseline: 1.5004x; 1.0137x over previous
"""Trainium2 Bass kernel for nn_ARTLayer (gnn_message_passing).

Math (reference):
    j(i,t) = t + (t>=i)                                    # [K, K-1] neighbor index
    alpha  = sigmoid(x@wa [i] + x@wb [j] + pf@wc + b_att)  # [K, K-1]
    msgs   = mean_t alpha * ((x@WobjT + b_obj)[j] + pf@WpairT + b_pair)
    out    = LN(x + msgs); out = LN(out + FFN(out))

Key algebraic rewrite (removes the 34-GFLOP [P,PD]x[PD,H] einsum):
    sum_t a*(pf@WpT)  = (sum_t a*pf) @ WpT               -> U[i,:] @ WpT
    sum_t a*oj[j]     = (A @ x) @ WobjT                  -> Gx[i,:] @ WoT
      with A[i,j] decomposed via lo/hi shifted views of x and a t>=i mask
    sum_t a*(b_obj+b_pair) = s_alpha[i] * bop

Sharding: rows i split across 8 cores (64 each); small tensors replicated;
host concatenates the per-core [64, 512] outputs.

Implementation notes (driven by NTFF profiles):
  - wc is folded into pf on the host (column scales, floored at fp16
    min-normal); sc becomes a pure fp16 halving-tree add-reduce and U is
    recovered exactly via W_pairT rows pre-divided by the scales.
  - 1/511 (the neighbor mean) is folded into WpT/WoT/bop on the host, and
    the t=511 pad slot is poisoned with -1e9 pre-sigmoid, so raw sigmoid
    output is used directly with no mask/scale multiplies.
  - U accumulation runs as M=4 quad matmuls (alpha quad stationary, four pf
    blocks streaming at N=512); the wanted rows sit on the block diagonal
    and are gathered by a stride-640 DRAM access pattern after a bank dump.
  - pf is re-laid-out on the host to [chunk, t, i, pd] so each tile DMA is
    one fully-contiguous 8KB-per-partition burst, and all DMAs are spread
    round-robin over the three DMA-capable engines (sync/scalar HWDGE,
    gpsimd SWDGE) instead of serializing on one queue.
"""
import numpy as np

import concourse.bass as bass
import concourse.tile as tile
from concourse import bacc, mybir

F32, F16 = mybir.dt.float32, mybir.dt.float16
AX = mybir.AxisListType
OP = mybir.AluOpType
AF = mybir.ActivationFunctionType

K, D, H, PD = 512, 512, 512, 128
T = K - 1                      # 511 neighbors per row
NCORES, IPC = 8, 64            # rows per core
NCH = 4                        # t-chunks of 128 (last chunk row 127 is t=511 pad)
IB, NIB = 64, 1                # i-block within a core
EPS = 1e-5


def build_program() -> bacc.Bacc:
    nc = bacc.Bacc("TRN2", target_bir_lowering=False, debug=False)

    def inp(name, shape, dt):
        return nc.dram_tensor(name, shape, dt, kind="ExternalInput").ap()

    pf = inp("pf", [NCH, 128, IPC, PD], F16)    # [chunk, t-in-chunk, i, pd]
    # single-DMA packed critical constants (128 contiguous rows each):
    # b32: [cmat 0:512 | poison 512:640 | mask_lt 640:896 | b_att 896:904]
    # b16: [wa_t 0:512 | wb_t 512:1024 | mask_ge 1024:1280 | ones 1280:1288
    #       | xi16 1288:1800]
    b32 = inp("b32", [128, 904], F32)
    b16 = inp("b16", [128, 1800], F16)
    xlo_ch = inp("xlo_ch", [128, NCH, D], F16)  # x rows chunked [t%128, t//128]
    dxf = inp("dxf", [K, D], F16)               # x[t+1] - x[t], host computed
    xi = inp("xi", [IPC, D], F32)               # this core's rows of x
    bias5 = inp("bias5", [5, H], F32)           # [ln_g; ln_b; b1; b2; bop/511]
    WpT = inp("WpT", [PD, H], F16)              # W_pair.T / colscale / 511
    WoT = inp("WoT", [D, H], F16)               # W_obj.T / 511
    W1T = inp("W1T", [H, H], F16)
    W2T = inp("W2T", [H, H], F16)

    out_d = nc.dram_tensor("out", [IPC, H], F32, kind="ExternalOutput").ap()

    with tile.TileContext(nc) as tc:
        with (
            tc.tile_pool(name="const", bufs=1) as cpool,
            tc.tile_pool(name="pfp", bufs=4) as pfp,
            tc.tile_pool(name="scrp", bufs=3) as scrp,
            tc.tile_pool(name="smallp", bufs=4) as smallp,
            tc.tile_pool(name="postp", bufs=3) as postp,
            tc.tile_pool(name="pss", bufs=2, space="PSUM") as pss,
            tc.tile_pool(name="psflex", bufs=4, space="PSUM") as psflex,
            tc.tile_pool(name="psmp", bufs=1, space="PSUM") as psmp,
            tc.tile_pool(name="psgp", bufs=1, space="PSUM") as psgp,
        ):
            # DMA policy: one HWDGE queue (sync) carries the critical path
            # in program order (a single queue reaches ~313 GB/s here and
            # multi-queue round-robin measures WORSE); bulky tail-only
            # weights stream on the gpsimd SWDGE queue in parallel.
            def dma(out, in_):
                nc.sync.dma_start(out=out, in_=in_)

            def dma_late(out, in_):
                nc.scalar.dma_start(out=out, in_=in_)

            # ---- constants & weights to SBUF (3 packed critical DMAs) ----
            hp = tc.high_priority()
            hp.__enter__()
            b32_sb = cpool.tile([128, 904], F32)
            dma(b32_sb, b32)
            b16_sb = cpool.tile([128, 1800], F16)
            dma(b16_sb, b16)
            xlo = cpool.tile([128, NCH, D], F16)
            dma(xlo, xlo_ch)
            hp.__exit__(None, None, None)
            tc.no_sync_barrier()   # keep pf DMAs behind the critical consts
            id_sb = b32_sb[:, 0:128]
            ones_sb = b32_sb[:, 128:256]
            sh1_sb = b32_sb[:, 256:384]
            sh2_sb = b32_sb[:, 384:512]
            poison_sb = b32_sb[0:1, 512:640]
            mlt_sb = b32_sb[:, 640:896].rearrange("p (c i) -> p c i", c=NCH)
            b_att_col = b32_sb[0:IPC, 896:897]
            wa_b = b16_sb[0:IPC, 0:512]
            wb_b = b16_sb[:, 512:1024]
            mge_sb = b16_sb[:, 1024:1280].rearrange("p (c i) -> p c i", c=NCH)
            ones16_sb = b16_sb[:, 1280:1288]
            xi16_sb = b16_sb[0:IPC, 1288:1800]
            bias_sb = cpool.tile([IPC, 5, H], F32)
            dma_late(bias_sb, bias5[None, :, :].to_broadcast([IPC, 5, H]))
            gb_sb = bias_sb[:, 0, :]
            bb_sb = bias_sb[:, 1, :]
            b1_sb = bias_sb[:, 2, :]
            b2_sb = bias_sb[:, 3, :]
            bop_row = bias_sb[0:1, 4, :]
            # tail-only loads on the slow queue, in rough use order
            dx = cpool.tile([128, NCH, D], F16)
            dma_late(dx, dxf.rearrange("(c p) d -> p c d", p=128))
            WpT_sb = cpool.tile([128, H], F16)
            dma_late(WpT_sb, WpT)
            WoT_sb = cpool.tile([128, NCH, H], F16)
            dma_late(WoT_sb, WoT.rearrange("(c p) h -> p c h", p=128))
            xi_sb = cpool.tile([IPC, D], F32)
            dma_late(xi_sb, xi)
            W1T_sb = cpool.tile([128, NCH, H], F16)
            dma_late(W1T_sb, W1T.rearrange("(c p) h -> p c h", p=128))
            W2T_sb = cpool.tile([128, NCH, H], F16)
            dma_late(W2T_sb, W2T.rearrange("(c p) h -> p c h", p=128))

            eps_col = cpool.tile([IPC, 1], F32)
            nc.vector.memset(eps_col, EPS)

            # ---- sa (this core's rows) and sb (all rows) ----
            scr_sa = smallp.tile([IPC, D], F16)
            nc.vector.tensor_mul(scr_sa, xi16_sb, wa_b)
            sa_col = smallp.tile([IPC, 1], F32)
            nc.vector.tensor_reduce(sa_col, scr_sa, axis=AX.X, op=OP.add)
            nc.vector.tensor_add(sa_col, sa_col, b_att_col)
            sa_diag = smallp.tile([IPC, IPC], F32)
            nc.vector.tensor_mul(sa_diag, id_sb[0:IPC, 0:IPC],
                                 sa_col.to_broadcast([IPC, IPC]))

            sb_cols = smallp.tile([128, NCH], F32)
            for c in range(NCH):
                scr_sb = smallp.tile([128, D], F16)
                nc.vector.tensor_mul(scr_sb, xlo[:, c, :], wb_b)
                nc.vector.tensor_reduce(
                    sb_cols[:, c:c + 1], scr_sb, axis=AX.X, op=OP.add)

            # sb_hi[p, c] = sb[c*128+p+1] via shift matmuls; slot 511 stays 0
            sbhi_ps = pss.tile([128, NCH], F32, tag="ps_small")
            nc.tensor.matmul(sbhi_ps, sh1_sb, sb_cols, start=True, stop=False)
            nc.tensor.matmul(sbhi_ps[:, 0:NCH - 1], sh2_sb, sb_cols[:, 1:NCH],
                             start=False, stop=True)
            sbhi_cols = smallp.tile([128, NCH], F32)
            nc.vector.tensor_copy(sbhi_cols, sbhi_ps)

            # ---- SBJ[t, i] = sa[i] + b_att + sb_hi[t] + mask_lt*(sb_lo-sb_hi),
            #      with -1e9 poison at the t=511 pad slot ----
            sbj = cpool.tile([128, NCH, IPC], F32)
            for c in range(NCH):
                diffc = smallp.tile([128, 1], F32)
                nc.vector.tensor_tensor(
                    diffc, sb_cols[:, c:c + 1], sbhi_cols[:, c:c + 1], OP.subtract)
                diagc = smallp.tile([128, 128], F32)
                nc.vector.tensor_mul(diagc, id_sb, diffc.to_broadcast([128, 128]))
                diagb = smallp.tile([128, 128], F32)
                nc.vector.tensor_mul(
                    diagb, id_sb, sbhi_cols[:, c:c + 1].to_broadcast([128, 128]))
                ps_sbj = pss.tile([128, IPC], F32, tag="ps_small")
                nc.tensor.matmul(ps_sbj, ones_sb[0:IPC, :], sa_diag,
                                 start=True, stop=False)
                nc.tensor.matmul(ps_sbj, diagb, ones_sb[:, 0:IPC],
                                 start=False, stop=False)
                if c == NCH - 1:
                    # poison: sigmoid(-1e9) = 0 exactly, pad row drops out
                    nc.tensor.matmul(ps_sbj, poison_sb, ones_sb[0:1, 0:IPC],
                                     start=False, stop=False)
                nc.tensor.matmul(ps_sbj, diagc, mlt_sb[:, c, :],
                                 start=False, stop=True)
                nc.vector.tensor_copy(sbj[:, c, :], ps_sbj)

            # ---- main edge pass ----
            alpha_full = cpool.tile([128, NCH, IPC], F16)   # raw sigmoid out
            age_full = cpool.tile([128, NCH, IPC], F16)     # masked (t>=i) alpha
            gx_ps = psgp.tile([IPC, D], F32)                # sum_t a*x[j]
            msg_ps = psmp.tile([IPC, H], F32)
            s_ps = pss.tile([1, IPC], F32, tag="ps_small")
            # U quad rows: bank b, partition slot 32s..32s+3 holds i=16b+4s+j
            u_ps = [psflex.tile([128, 512], F32, tag="flex", name=f"u_ps{b}")
                    for b in range(4)]
            for b in range(4):
                nc.vector.memset(u_ps[b], 0.0)

            for c in range(NCH):
                pf_t = pfp.tile([128, IB, PD], F16)
                dma(pf_t, pf[c, :, :, :])
                # sc = sum_pd pf_sent (wc pre-folded): fp16 halving tree
                scr = scrp.tile([128, IB, 64], F16)
                nc.vector.tensor_add(scr, pf_t[:, :, 0:64], pf_t[:, :, 64:128])
                w = 32
                while w >= 2:
                    nc.vector.tensor_add(
                        scr[:, :, 0:w], scr[:, :, 0:w], scr[:, :, w:2 * w])
                    w //= 2
                sc_t = smallp.tile([128, IB], F32)
                nc.vector.tensor_add(sc_t, scr[:, :, 0], scr[:, :, 1])
                aarg = smallp.tile([128, IB], F32)
                nc.vector.tensor_add(aarg, sc_t, sbj[:, c, :])
                nc.scalar.activation(alpha_full[:, c, :], aarg, AF.Sigmoid)
                nc.vector.tensor_mul(age_full[:, c, :], alpha_full[:, c, :],
                                     mge_sb[:, c, :])
                # U quads: lhsT = 4 alpha columns, rhs = 4 pf blocks; the
                # wanted rows sit on the diagonal (gathered via DRAM AP)
                for q in range(IB // 4):
                    b, sp = divmod(q, 4)
                    nc.tensor.matmul(
                        u_ps[b][32 * sp:32 * sp + 4, :],
                        alpha_full[:, c, 4 * q:4 * q + 4],
                        pf_t[:, 4 * q:4 * q + 4, :],
                        start=(c == 0), stop=(c == NCH - 1),
                        tile_position=(0, 32 * sp))
                nc.tensor.matmul(gx_ps, alpha_full[:, c, :], xlo[:, c, :],
                                 start=(c == 0), stop=False)
                nc.tensor.matmul(s_ps, ones16_sb[:, 0:1], alpha_full[:, c, :],
                                 start=(c == 0), stop=(c == NCH - 1))

            # scheduler fence: keep every tail instruction after the loop in
            # each engine stream (strict-FIFO engines head-of-line block if
            # e.g. an LN Sqrt lands between loop sigmoids in the ACT queue)
            tc.no_sync_barrier()

            # G2 (shifted-x correction) after the loop: dx arrives on the slow
            # queue and age_full persists, so this overlaps the loop tail
            for c in range(NCH):
                nc.tensor.matmul(gx_ps, age_full[:, c, :], dx[:, c, :],
                                 start=False, stop=(c == NCH - 1))

            # ---- messages = U@WpT + Gx@WoT + s_alpha x bop ----
            s_row = smallp.tile([1, IPC], F32)
            nc.vector.tensor_copy(s_row, s_ps)

            # U reassembly on-chip: bank copy -> PE transpose of each
            # 128-col block (diagonal quad becomes free-strided columns) ->
            # tiny strided copies assemble UT directly; no DRAM bounce.
            u_sb = postp.tile([128, IPC], F16)
            for b in range(4):
                u_cp = postp.tile([128, 512], F32, tag="u_cp")
                nc.vector.tensor_copy(u_cp, u_ps[b])
                for j in range(4):
                    ptu = pss.tile([128, 128], F32, tag="ps_small")
                    nc.tensor.transpose(ptu, u_cp[:, j * 128:(j + 1) * 128],
                                        id_sb)
                    # cols {j, 32+j, 64+j, 96+j} hold U rows i=16b+4s+j
                    src_ap = ptu.rearrange("p (s q) -> p s q", q=32)[:, :, j]
                    dst_ap = u_sb.rearrange("p (r s f) -> p r s f", r=4, s=4)[
                        :, b, :, j]
                    nc.vector.tensor_copy(dst_ap, src_ap)

            gx_sb = postp.tile([IPC, D], F32)
            nc.vector.tensor_copy(gx_sb, gx_ps)
            gxT = postp.tile([128, NCH, IPC], F16)
            for c in range(NCH):
                ptg = pss.tile([128, IPC], F32, tag="ps_small")
                nc.tensor.transpose(ptg, gx_sb[:, c * 128:(c + 1) * 128],
                                    id_sb[0:IPC, 0:IPC])
                nc.vector.tensor_copy(gxT[:, c, :], ptg)

            nc.tensor.matmul(msg_ps, u_sb, WpT_sb, start=True, stop=False)
            for c in range(NCH):
                nc.tensor.matmul(msg_ps, gxT[:, c, :], WoT_sb[:, c, :],
                                 start=False, stop=False)
            nc.tensor.matmul(msg_ps, s_row, bop_row, start=False, stop=True)

            # ---- residual + LN1 ----
            def layer_norm(v):
                stats = smallp.tile([IPC, 6], F32)
                nc.vector.bn_stats(out=stats, in_=v)
                mv = smallp.tile([IPC, 2], F32)
                nc.vector.bn_aggr(out=mv, in_=stats)
                std = smallp.tile([IPC, 1], F32)
                nc.scalar.activation(std, mv[:, 1:2], AF.Sqrt, bias=eps_col)
                rstd = smallp.tile([IPC, 1], F32)
                nc.vector.reciprocal(rstd, std)
                cen = postp.tile([IPC, H], F32)
                nc.vector.tensor_scalar(cen, v, mv[:, 0:1], rstd,
                                        OP.subtract, OP.mult)
                o = postp.tile([IPC, H], F32)
                nc.vector.tensor_mul(o, cen, gb_sb)
                nc.vector.tensor_add(o, o, bb_sb)
                return o

            h_sb = postp.tile([IPC, H], F32)
            nc.vector.tensor_add(h_sb, xi_sb, msg_ps)
            out1 = layer_norm(h_sb)

            # ---- FFN ----
            def transpose_rows(v):
                vT = postp.tile([128, NCH, IPC], F16, tag="vT")
                for c in range(NCH):
                    ptt = pss.tile([128, IPC], F32, tag="ps_small")
                    nc.tensor.transpose(ptt, v[:, c * 128:(c + 1) * 128],
                                        id_sb[0:IPC, 0:IPC])
                    nc.vector.tensor_copy(vT[:, c, :], ptt)
                return vT

            o1T = transpose_rows(out1)
            o1b = postp.tile([IPC, H], F32)
            nc.vector.tensor_add(o1b, out1, b2_sb)
            f1_ps = psflex.tile([IPC, H], F32, tag="flex")
            for c in range(NCH):
                nc.tensor.matmul(f1_ps, o1T[:, c, :], W1T_sb[:, c, :],
                                 start=(c == 0), stop=(c == NCH - 1))
            f1 = postp.tile([IPC, H], F32)
            nc.vector.tensor_add(f1, f1_ps, b1_sb)
            nc.vector.tensor_scalar_max(f1, f1, 0.0)

            f1T = transpose_rows(f1)
            f2_ps = psflex.tile([IPC, H], F32, tag="flex")
            for c in range(NCH):
                nc.tensor.matmul(f2_ps, f1T[:, c, :], W2T_sb[:, c, :],
                                 start=(c == 0), stop=(c == NCH - 1))
            h2 = postp.tile([IPC, H], F32)
            nc.vector.tensor_add(h2, f2_ps, o1b)
            out2 = layer_norm(h2)

            nc.sync.dma_start(out=out_d, in_=out2)

    return nc


def _poison() -> np.ndarray:
    p = np.zeros((1, 128), np.float32)
    p[0, 127] = -1e9
    return p


def _cmat() -> np.ndarray:
    c = np.zeros((128, 4, 128), np.float32)
    c[:, 0, :] = np.eye(128)
    c[:, 1, :] = 1.0
    c[:, 2, :] = np.eye(128, k=-1)     # shift1[q, p] = (q == p+1)
    c[0, 3, 127] = 1.0                  # shift2[q, p] = (q==0)&(p==127)
    return c


def prep_in_maps(inputs) -> list[dict]:
    x = np.asarray(inputs["x"], np.float32)
    pf = np.asarray(inputs["pair_feats"], np.float32)
    W_att = np.asarray(inputs["W_att"], np.float32)
    b_att = np.asarray(inputs["b_att"], np.float32)
    W_obj = np.asarray(inputs["W_obj"], np.float32)
    b_obj = np.asarray(inputs["b_obj"], np.float32)
    W_pair = np.asarray(inputs["W_pair"], np.float32)
    b_pair = np.asarray(inputs["b_pair"], np.float32)
    ln_g = np.asarray(inputs["ln_g"], np.float32)
    ln_b = np.asarray(inputs["ln_b"], np.float32)
    W1 = np.asarray(inputs["W1"], np.float32)
    b1 = np.asarray(inputs["b1"], np.float32)
    W2 = np.asarray(inputs["W2"], np.float32)
    b2 = np.asarray(inputs["b2"], np.float32)

    wa, wb, wc = W_att[0, :D], W_att[0, D:2 * D], W_att[0, 2 * D:]
    xpad = np.concatenate([x, np.zeros((1, D), np.float32)], axis=0)

    # fold wc into pf columns; recover U via pre-divided W_pair.T rows.
    colscale = np.sign(wc) * np.maximum(np.abs(wc), 6e-5)
    colscale[colscale == 0] = 6e-5
    # 1/511 (the mean over neighbors) is folded into the three weight paths
    # that consume raw alpha: U@WpT, (A@x)@WoT, and s_alpha*bop.
    WpT2 = (W_pair.T / colscale[:, None] / T).astype(np.float16)
    WoT2 = (W_obj.T / T).astype(np.float16)
    dxf = np.diff(xpad[:K + 1], axis=0)

    b32a = np.zeros((128, 904), np.float32)
    b32a[:, 0:512] = _cmat().reshape(128, 512)
    b32a[0, 512 + 127] = -1e9
    b32a[:, 896] = b_att[0]
    b16a = np.zeros((128, 1800), np.float16)
    b16a[:, 0:512] = wa[None, :]
    b16a[:, 512:1024] = wb[None, :]
    b16a[:, 1280:1288] = 1.0
    xlo_np = np.ascontiguousarray(
        x.reshape(NCH, 128, D).transpose(1, 0, 2)).astype(np.float16)

    base = dict(
        xlo_ch=xlo_np,
        dxf=dxf.astype(np.float16),
        bias5=np.stack([ln_g, ln_b, b1, b2,
                        (b_obj + b_pair) / T]).astype(np.float32),
        WpT=np.ascontiguousarray(WpT2),
        WoT=np.ascontiguousarray(WoT2),
        W1T=np.ascontiguousarray(W1.T).astype(np.float16),
        W2T=np.ascontiguousarray(W2.T).astype(np.float16),
    )

    pfr = pf.reshape(K, T, PD)
    tgrid = np.arange(128)[:, None] + 128 * np.arange(NCH)[None, :]   # [128, NCH]

    in_maps = []
    for core in range(NCORES):
        ig = np.arange(core * IPC, (core + 1) * IPC)
        mlt = (tgrid[:, :, None] < ig[None, None, :]).astype(np.float32)
        mge = ((tgrid[:, :, None] >= ig[None, None, :])
               & (tgrid[:, :, None] <= T - 1)).astype(np.float16)
        # [chunk, t, i, pd] layout -> each tile DMA is one contiguous burst
        shard = np.zeros((NCH * 128, IPC, PD), np.float16)
        shard[:T] = (pfr[ig] * colscale[None, None, :]).transpose(1, 0, 2)
        xi = x[ig]
        cb32 = b32a.copy()
        cb32[:, 640:896] = mlt.reshape(128, NCH * IPC)
        cb16 = b16a.copy()
        cb16[:, 1024:1280] = mge.reshape(128, NCH * IPC)
        cb16[0:IPC, 1288:1800] = xi.astype(np.float16)
        m = dict(base)
        m.update(
            pf=shard.reshape(NCH, 128, IPC, PD),
            xi=xi.astype(np.float32),
            b32=cb32,
            b16=cb16,
        )
        in_maps.append(m)
    return in_maps


_COMPILED = None


def _get_program() -> bacc.Bacc:
    global _COMPILED
    if _COMPILED is None:
        nc = build_program()
        nc.compile()
        _COMPILED = nc
    return _COMPILED


TRACE = False
LAST_RESULT = None


def _install_axon_ntff_hook():
    """The container's antenv lacks axon_hooks; recreate it from trn_boot's
    ctypes implementation so trace=True can capture NTFF profiles."""
    import sys
    import types
    try:
        from antenv.axon_hooks import get_axon_ntff_profile_hook  # noqa: F401
        return
    except ImportError:
        pass
    from trn_agent_boot.trn_boot import _ntff_profile_via_ctypes
    hook = _ntff_profile_via_ctypes("/opt/axon/libaxon_pjrt.so")
    m = types.ModuleType("antenv.axon_hooks")
    m.get_axon_ntff_profile_hook = lambda: hook
    sys.modules["antenv.axon_hooks"] = m


def kernel(**inputs) -> np.ndarray:
    import concourse.bass_utils as bu
    from concourse.bass_utils import run_bass_kernel_spmd
    global LAST_RESULT
    if TRACE:
        _install_axon_ntff_hook()
        bu.upload_artifacts = lambda tmpdir: str(tmpdir)  # no bucket here
    nc = _get_program()
    in_maps = prep_in_maps(inputs)
    res = run_bass_kernel_spmd(nc, in_maps, list(range(NCORES)), trace=TRACE)
    LAST_RESULT = res
    outs = [res.results[c]["out"] for c in range(NCORES)]
    return np.concatenate(outs, axis=0).astype(np.float32)


# revision 44
# speedup vs baseline: 1.5121x; 1.0078x over previous
"""Trainium2 Bass kernel for nn_ARTLayer (gnn_message_passing).

Math (reference):
    j(i,t) = t + (t>=i)                                    # [K, K-1] neighbor index
    alpha  = sigmoid(x@wa [i] + x@wb [j] + pf@wc + b_att)  # [K, K-1]
    msgs   = mean_t alpha * ((x@WobjT + b_obj)[j] + pf@WpairT + b_pair)
    out    = LN(x + msgs); out = LN(out + FFN(out))

Key algebraic rewrite (removes the 34-GFLOP [P,PD]x[PD,H] einsum):
    sum_t a*(pf@WpT)  = (sum_t a*pf) @ WpT               -> U[i,:] @ WpT
    sum_t a*oj[j]     = (A @ x) @ WobjT                  -> Gx[i,:] @ WoT
      with A[i,j] decomposed via lo/hi shifted views of x and a t>=i mask
    sum_t a*(b_obj+b_pair) = s_alpha[i] * bop

Sharding: rows i split across 8 cores (64 each); small tensors replicated;
host concatenates the per-core [64, 512] outputs.

Implementation notes (driven by NTFF profiles):
  - wc is folded into pf on the host (column scales, floored at fp16
    min-normal); sc becomes a pure fp16 halving-tree add-reduce and U is
    recovered exactly via W_pairT rows pre-divided by the scales.
  - 1/511 (the neighbor mean) is folded into WpT/WoT/bop on the host, and
    the t=511 pad slot is poisoned with -1e9 pre-sigmoid, so raw sigmoid
    output is used directly with no mask/scale multiplies.
  - U accumulation runs as M=4 quad matmuls (alpha quad stationary, four pf
    blocks streaming at N=512); the wanted rows sit on the block diagonal
    and are gathered by a stride-640 DRAM access pattern after a bank dump.
  - pf is re-laid-out on the host to [chunk, t, i, pd] so each tile DMA is
    one fully-contiguous 16KB-per-partition burst. One HWDGE queue (sync)
    carries the critical path in order (packed const blobs, then pf chunks);
    tail-only weights stream on the second HWDGE queue (scalar) in parallel.
  - U rows are extracted on-chip (bank copy -> per-block PE transpose ->
    free-strided gather copies); no DRAM bounce.
"""
import numpy as np

import concourse.bass as bass
import concourse.tile as tile
from concourse import bacc, mybir

F32, F16 = mybir.dt.float32, mybir.dt.float16
AX = mybir.AxisListType
OP = mybir.AluOpType
AF = mybir.ActivationFunctionType

K, D, H, PD = 512, 512, 512, 128
T = K - 1                      # 511 neighbors per row
NCORES, IPC = 8, 64            # rows per core
NCH = 4                        # t-chunks of 128 (last chunk row 127 is t=511 pad)
IB, NIB = 64, 1                # i-block within a core
EPS = 1e-5


def build_program() -> bacc.Bacc:
    nc = bacc.Bacc("TRN2", target_bir_lowering=False, debug=False)

    def inp(name, shape, dt):
        return nc.dram_tensor(name, shape, dt, kind="ExternalInput").ap()

    pf = inp("pf", [NCH, 128, IPC, PD], F16)    # [chunk, t-in-chunk, i, pd]
    # single-DMA packed critical constants (128 contiguous rows each):
    # b32: [cmat 0:512 | poison 512:640 | mask_lt 640:896 | b_att 896:904]
    # b16: [wa_t 0:512 | wb_t 512:1024 | mask_ge 1024:1280 | ones 1280:1288
    #       | xi16 1288:1800]
    b32 = inp("b32", [128, 904], F32)
    b16 = inp("b16", [128, 1800], F16)
    xlo_ch = inp("xlo_ch", [128, NCH, D], F16)  # x rows chunked [t%128, t//128]
    dxf = inp("dxf", [K, D], F16)               # x[t+1] - x[t], host computed
    xi = inp("xi", [IPC, D], F32)               # this core's rows of x
    bias5 = inp("bias5", [5, H], F32)           # [ln_g; ln_b; b1; b2; bop/511]
    WpT = inp("WpT", [PD, H], F16)              # W_pair.T / colscale / 511
    WoT = inp("WoT", [D, H], F16)               # W_obj.T / 511
    W1T = inp("W1T", [H, H], F16)
    W2T = inp("W2T", [H, H], F16)

    out_d = nc.dram_tensor("out", [IPC, H], F32, kind="ExternalOutput").ap()

    with tile.TileContext(nc) as tc:
        with (
            tc.tile_pool(name="const", bufs=1) as cpool,
            tc.tile_pool(name="pfp", bufs=4) as pfp,
            tc.tile_pool(name="scrp", bufs=3) as scrp,
            tc.tile_pool(name="smallp", bufs=4) as smallp,
            tc.tile_pool(name="postp", bufs=3) as postp,
            tc.tile_pool(name="pss", bufs=2, space="PSUM") as pss,
            tc.tile_pool(name="psflex", bufs=4, space="PSUM") as psflex,
            tc.tile_pool(name="psmp", bufs=1, space="PSUM") as psmp,
            tc.tile_pool(name="psgp", bufs=1, space="PSUM") as psgp,
        ):
            # DMA policy: one HWDGE queue (sync) carries the critical path
            # in program order (a single queue reaches ~313 GB/s here and
            # multi-queue round-robin measures WORSE); bulky tail-only
            # weights stream on the gpsimd SWDGE queue in parallel.
            def dma(out, in_):
                nc.sync.dma_start(out=out, in_=in_)

            def dma_late(out, in_):
                nc.scalar.dma_start(out=out, in_=in_)

            # ---- constants & weights to SBUF (3 packed critical DMAs) ----
            hp = tc.high_priority()
            hp.__enter__()
            b32_sb = cpool.tile([128, 904], F32)
            dma(b32_sb, b32)
            b16_sb = cpool.tile([128, 1800], F16)
            dma(b16_sb, b16)
            xlo = cpool.tile([128, NCH, D], F16)
            dma(xlo, xlo_ch)
            hp.__exit__(None, None, None)
            tc.no_sync_barrier()   # keep pf DMAs behind the critical consts
            id_sb = b32_sb[:, 0:128]
            ones_sb = b32_sb[:, 128:256]
            sh1_sb = b32_sb[:, 256:384]
            sh2_sb = b32_sb[:, 384:512]
            poison_sb = b32_sb[0:1, 512:640]
            mlt_sb = b32_sb[:, 640:896].rearrange("p (c i) -> p c i", c=NCH)
            b_att_col = b32_sb[0:IPC, 896:897]
            wa_b = b16_sb[0:IPC, 0:512]
            wb_b = b16_sb[:, 512:1024]
            mge_sb = b16_sb[:, 1024:1280].rearrange("p (c i) -> p c i", c=NCH)
            ones16_sb = b16_sb[:, 1280:1288]
            xi16_sb = b16_sb[0:IPC, 1288:1800]
            bias_sb = cpool.tile([IPC, 5, H], F32)
            dma_late(bias_sb, bias5[None, :, :].to_broadcast([IPC, 5, H]))
            gb_sb = bias_sb[:, 0, :]
            bb_sb = bias_sb[:, 1, :]
            b1_sb = bias_sb[:, 2, :]
            b2_sb = bias_sb[:, 3, :]
            bop_row = bias_sb[0:1, 4, :]
            # tail-only loads on the slow queue, in rough use order
            dx = cpool.tile([128, NCH, D], F16)
            dma_late(dx, dxf.rearrange("(c p) d -> p c d", p=128))
            WpT_sb = cpool.tile([128, H], F16)
            dma_late(WpT_sb, WpT)
            WoT_sb = cpool.tile([128, NCH, H], F16)
            dma_late(WoT_sb, WoT.rearrange("(c p) h -> p c h", p=128))
            xi_sb = cpool.tile([IPC, D], F32)
            dma_late(xi_sb, xi)
            W1T_sb = cpool.tile([128, NCH, H], F16)
            dma_late(W1T_sb, W1T.rearrange("(c p) h -> p c h", p=128))
            W2T_sb = cpool.tile([128, NCH, H], F16)
            dma_late(W2T_sb, W2T.rearrange("(c p) h -> p c h", p=128))

            eps_col = cpool.tile([IPC, 1], F32)
            nc.vector.memset(eps_col, EPS)

            # ---- sa (this core's rows) and sb (all rows) ----
            scr_sa = smallp.tile([IPC, D], F16)
            nc.vector.tensor_mul(scr_sa, xi16_sb, wa_b)
            sa_col = smallp.tile([IPC, 1], F32)
            nc.vector.tensor_reduce(sa_col, scr_sa, axis=AX.X, op=OP.add)
            nc.vector.tensor_add(sa_col, sa_col, b_att_col)
            sa_diag = smallp.tile([IPC, IPC], F32)
            nc.vector.tensor_mul(sa_diag, id_sb[0:IPC, 0:IPC],
                                 sa_col.to_broadcast([IPC, IPC]))

            sb_cols = smallp.tile([128, NCH], F32)
            for c in range(NCH):
                scr_sb = smallp.tile([128, D], F16)
                nc.vector.tensor_mul(scr_sb, xlo[:, c, :], wb_b)
                nc.vector.tensor_reduce(
                    sb_cols[:, c:c + 1], scr_sb, axis=AX.X, op=OP.add)

            # sb_hi[p, c] = sb[c*128+p+1] via shift matmuls; slot 511 stays 0
            sbhi_ps = pss.tile([128, NCH], F32, tag="ps_small")
            nc.tensor.matmul(sbhi_ps, sh1_sb, sb_cols, start=True, stop=False)
            nc.tensor.matmul(sbhi_ps[:, 0:NCH - 1], sh2_sb, sb_cols[:, 1:NCH],
                             start=False, stop=True)
            sbhi_cols = smallp.tile([128, NCH], F32)
            nc.vector.tensor_copy(sbhi_cols, sbhi_ps)

            # ---- SBJ[t, i] = sa[i] + b_att + sb_hi[t] + mask_lt*(sb_lo-sb_hi),
            #      with -1e9 poison at the t=511 pad slot ----
            sbj = cpool.tile([128, NCH, IPC], F32)
            for c in range(NCH):
                diffc = smallp.tile([128, 1], F32)
                nc.vector.tensor_tensor(
                    diffc, sb_cols[:, c:c + 1], sbhi_cols[:, c:c + 1], OP.subtract)
                diagc = smallp.tile([128, 128], F32)
                nc.vector.tensor_mul(diagc, id_sb, diffc.to_broadcast([128, 128]))
                diagb = smallp.tile([128, 128], F32)
                nc.vector.tensor_mul(
                    diagb, id_sb, sbhi_cols[:, c:c + 1].to_broadcast([128, 128]))
                ps_sbj = pss.tile([128, IPC], F32, tag="ps_small")
                nc.tensor.matmul(ps_sbj, ones_sb[0:IPC, :], sa_diag,
                                 start=True, stop=False)
                nc.tensor.matmul(ps_sbj, diagb, ones_sb[:, 0:IPC],
                                 start=False, stop=False)
                if c == NCH - 1:
                    # poison: sigmoid(-1e9) = 0 exactly, pad row drops out
                    nc.tensor.matmul(ps_sbj, poison_sb, ones_sb[0:1, 0:IPC],
                                     start=False, stop=False)
                nc.tensor.matmul(ps_sbj, diagc, mlt_sb[:, c, :],
                                 start=False, stop=True)
                nc.vector.tensor_copy(sbj[:, c, :], ps_sbj)

            # ---- main edge pass ----
            alpha_full = cpool.tile([128, NCH, IPC], F16)   # raw sigmoid out
            age_full = cpool.tile([128, NCH, IPC], F16)     # masked (t>=i) alpha
            gx_ps = psgp.tile([IPC, D], F32)                # sum_t a*x[j]
            msg_ps = psmp.tile([IPC, H], F32)
            s_ps = pss.tile([1, IPC], F32, tag="ps_small")
            # U quad rows: bank b, partition slot 32s..32s+3 holds i=16b+4s+j
            u_ps = [psflex.tile([128, 512], F32, tag="flex", name=f"u_ps{b}")
                    for b in range(4)]
            for b in range(4):
                nc.vector.memset(u_ps[b], 0.0)

            for c in range(NCH):
                pf_t = pfp.tile([128, IB, PD], F16)
                dma(pf_t, pf[c, :, :, :])
                # sc = sum_pd pf_sent (wc pre-folded): fp16 halving tree
                scr = scrp.tile([128, IB, 64], F16)
                nc.vector.tensor_add(scr, pf_t[:, :, 0:64], pf_t[:, :, 64:128])
                w = 32
                while w >= 2:
                    nc.vector.tensor_add(
                        scr[:, :, 0:w], scr[:, :, 0:w], scr[:, :, w:2 * w])
                    w //= 2
                sc_t = smallp.tile([128, IB], F32)
                nc.vector.tensor_add(sc_t, scr[:, :, 0], scr[:, :, 1])
                aarg = smallp.tile([128, IB], F32)
                nc.vector.tensor_add(aarg, sc_t, sbj[:, c, :])
                nc.scalar.activation(alpha_full[:, c, :], aarg, AF.Sigmoid)
                nc.vector.tensor_mul(age_full[:, c, :], alpha_full[:, c, :],
                                     mge_sb[:, c, :])
                # U quads: lhsT = 4 alpha columns, rhs = 4 pf blocks; the
                # wanted rows sit on the diagonal (gathered via DRAM AP)
                for q in range(IB // 4):
                    b, sp = divmod(q, 4)
                    nc.tensor.matmul(
                        u_ps[b][32 * sp:32 * sp + 4, :],
                        alpha_full[:, c, 4 * q:4 * q + 4],
                        pf_t[:, 4 * q:4 * q + 4, :],
                        start=(c == 0), stop=(c == NCH - 1),
                        tile_position=(0, 32 * sp))
                nc.tensor.matmul(gx_ps, alpha_full[:, c, :], xlo[:, c, :],
                                 start=(c == 0), stop=False)
                nc.tensor.matmul(s_ps, ones16_sb[:, 0:1], alpha_full[:, c, :],
                                 start=(c == 0), stop=(c == NCH - 1))

            # scheduler fence: keep every tail instruction after the loop in
            # each engine stream (strict-FIFO engines head-of-line block if
            # e.g. an LN Sqrt lands between loop sigmoids in the ACT queue)
            tc.no_sync_barrier()

            # G2 (shifted-x correction) after the loop: dx arrives on the slow
            # queue and age_full persists, so this overlaps the loop tail
            for c in range(NCH):
                nc.tensor.matmul(gx_ps, age_full[:, c, :], dx[:, c, :],
                                 start=False, stop=(c == NCH - 1))

            # ---- messages = U@WpT + Gx@WoT + s_alpha x bop ----
            s_row = smallp.tile([1, IPC], F32)
            nc.vector.tensor_copy(s_row, s_ps)

            # U reassembly on-chip: bank copy -> PE transpose of each
            # 128-col block (diagonal quad becomes free-strided columns) ->
            # tiny strided copies assemble UT directly; no DRAM bounce.
            u_sb = postp.tile([128, IPC], F16)
            for b in range(4):
                u_cp = postp.tile([128, 512], F32, tag="u_cp")
                nc.vector.tensor_copy(u_cp, u_ps[b])
                for j in range(4):
                    ptu = pss.tile([128, 128], F32, tag="ps_small")
                    nc.tensor.transpose(ptu, u_cp[:, j * 128:(j + 1) * 128],
                                        id_sb)
                    # cols {j, 32+j, 64+j, 96+j} hold U rows i=16b+4s+j
                    src_ap = ptu.rearrange("p (s q) -> p s q", q=32)[:, :, j]
                    dst_ap = u_sb.rearrange("p (r s f) -> p r s f", r=4, s=4)[
                        :, b, :, j]
                    nc.vector.tensor_copy(dst_ap, src_ap)

            gx_sb = postp.tile([IPC, D], F32)
            nc.vector.tensor_copy(gx_sb, gx_ps)
            gxT = postp.tile([128, NCH, IPC], F16)
            for c in range(NCH):
                ptg = pss.tile([128, IPC], F32, tag="ps_small")
                nc.tensor.transpose(ptg, gx_sb[:, c * 128:(c + 1) * 128],
                                    id_sb[0:IPC, 0:IPC])
                nc.vector.tensor_copy(gxT[:, c, :], ptg)

            nc.tensor.matmul(msg_ps, u_sb, WpT_sb, start=True, stop=False)
            for c in range(NCH):
                nc.tensor.matmul(msg_ps, gxT[:, c, :], WoT_sb[:, c, :],
                                 start=False, stop=False)
            nc.tensor.matmul(msg_ps, s_row, bop_row, start=False, stop=True)

            # ---- residual + LN1 ----
            def layer_norm(v):
                stats = smallp.tile([IPC, 6], F32)
                nc.vector.bn_stats(out=stats, in_=v)
                mv = smallp.tile([IPC, 2], F32)
                nc.vector.bn_aggr(out=mv, in_=stats)
                std = smallp.tile([IPC, 1], F32)
                nc.scalar.activation(std, mv[:, 1:2], AF.Sqrt, bias=eps_col)
                rstd = smallp.tile([IPC, 1], F32)
                nc.vector.reciprocal(rstd, std)
                cen = postp.tile([IPC, H], F32)
                nc.vector.tensor_scalar(cen, v, mv[:, 0:1], rstd,
                                        OP.subtract, OP.mult)
                o = postp.tile([IPC, H], F32)
                nc.vector.tensor_mul(o, cen, gb_sb)
                nc.vector.tensor_add(o, o, bb_sb)
                return o

            h_sb = postp.tile([IPC, H], F32)
            nc.vector.tensor_add(h_sb, xi_sb, msg_ps)
            out1 = layer_norm(h_sb)

            # ---- FFN ----
            def transpose_rows(v):
                vT = postp.tile([128, NCH, IPC], F16, tag="vT")
                for c in range(NCH):
                    ptt = pss.tile([128, IPC], F32, tag="ps_small")
                    nc.tensor.transpose(ptt, v[:, c * 128:(c + 1) * 128],
                                        id_sb[0:IPC, 0:IPC])
                    nc.vector.tensor_copy(vT[:, c, :], ptt)
                return vT

            o1T = transpose_rows(out1)
            o1b = postp.tile([IPC, H], F32)
            nc.vector.tensor_add(o1b, out1, b2_sb)
            f1_ps = psflex.tile([IPC, H], F32, tag="flex")
            for c in range(NCH):
                nc.tensor.matmul(f1_ps, o1T[:, c, :], W1T_sb[:, c, :],
                                 start=(c == 0), stop=(c == NCH - 1))
            f1 = postp.tile([IPC, H], F32)
            nc.vector.tensor_add(f1, f1_ps, b1_sb)
            nc.vector.tensor_scalar_max(f1, f1, 0.0)

            f1T = transpose_rows(f1)
            f2_ps = psflex.tile([IPC, H], F32, tag="flex")
            for c in range(NCH):
                nc.tensor.matmul(f2_ps, f1T[:, c, :], W2T_sb[:, c, :],
                                 start=(c == 0), stop=(c == NCH - 1))
            h2 = postp.tile([IPC, H], F32)
            nc.vector.tensor_add(h2, f2_ps, o1b)
            out2 = layer_norm(h2)

            nc.sync.dma_start(out=out_d, in_=out2)

    return nc


def _poison() -> np.ndarray:
    p = np.zeros((1, 128), np.float32)
    p[0, 127] = -1e9
    return p


def _cmat() -> np.ndarray:
    c = np.zeros((128, 4, 128), np.float32)
    c[:, 0, :] = np.eye(128)
    c[:, 1, :] = 1.0
    c[:, 2, :] = np.eye(128, k=-1)     # shift1[q, p] = (q == p+1)
    c[0, 3, 127] = 1.0                  # shift2[q, p] = (q==0)&(p==127)
    return c


def prep_in_maps(inputs) -> list[dict]:
    x = np.asarray(inputs["x"], np.float32)
    pf = np.asarray(inputs["pair_feats"], np.float32)
    W_att = np.asarray(inputs["W_att"], np.float32)
    b_att = np.asarray(inputs["b_att"], np.float32)
    W_obj = np.asarray(inputs["W_obj"], np.float32)
    b_obj = np.asarray(inputs["b_obj"], np.float32)
    W_pair = np.asarray(inputs["W_pair"], np.float32)
    b_pair = np.asarray(inputs["b_pair"], np.float32)
    ln_g = np.asarray(inputs["ln_g"], np.float32)
    ln_b = np.asarray(inputs["ln_b"], np.float32)
    W1 = np.asarray(inputs["W1"], np.float32)
    b1 = np.asarray(inputs["b1"], np.float32)
    W2 = np.asarray(inputs["W2"], np.float32)
    b2 = np.asarray(inputs["b2"], np.float32)

    wa, wb, wc = W_att[0, :D], W_att[0, D:2 * D], W_att[0, 2 * D:]
    xpad = np.concatenate([x, np.zeros((1, D), np.float32)], axis=0)

    # fold wc into pf columns; recover U via pre-divided W_pair.T rows.
    colscale = np.sign(wc) * np.maximum(np.abs(wc), 6e-5)
    colscale[colscale == 0] = 6e-5
    # 1/511 (the mean over neighbors) is folded into the three weight paths
    # that consume raw alpha: U@WpT, (A@x)@WoT, and s_alpha*bop.
    WpT2 = (W_pair.T / colscale[:, None] / T).astype(np.float16)
    WoT2 = (W_obj.T / T).astype(np.float16)
    dxf = np.diff(xpad[:K + 1], axis=0)

    b32a = np.zeros((128, 904), np.float32)
    b32a[:, 0:512] = _cmat().reshape(128, 512)
    b32a[0, 512 + 127] = -1e9
    b32a[:, 896] = b_att[0]
    b16a = np.zeros((128, 1800), np.float16)
    b16a[:, 0:512] = wa[None, :]
    b16a[:, 512:1024] = wb[None, :]
    b16a[:, 1280:1288] = 1.0
    xlo_np = np.ascontiguousarray(
        x.reshape(NCH, 128, D).transpose(1, 0, 2)).astype(np.float16)

    base = dict(
        xlo_ch=xlo_np,
        dxf=dxf.astype(np.float16),
        bias5=np.stack([ln_g, ln_b, b1, b2,
                        (b_obj + b_pair) / T]).astype(np.float32),
        WpT=np.ascontiguousarray(WpT2),
        WoT=np.ascontiguousarray(WoT2),
        W1T=np.ascontiguousarray(W1.T).astype(np.float16),
        W2T=np.ascontiguousarray(W2.T).astype(np.float16),
    )

    pfr = pf.reshape(K, T, PD)
    tgrid = np.arange(128)[:, None] + 128 * np.arange(NCH)[None, :]   # [128, NCH]

    in_maps = []
    for core in range(NCORES):
        ig = np.arange(core * IPC, (core + 1) * IPC)
        mlt = (tgrid[:, :, None] < ig[None, None, :]).astype(np.float32)
        mge = ((tgrid[:, :, None] >= ig[None, None, :])
               & (tgrid[:, :, None] <= T - 1)).astype(np.float16)
        # [chunk, t, i, pd] layout -> each tile DMA is one contiguous burst
        shard = np.zeros((NCH * 128, IPC, PD), np.float16)
        shard[:T] = (pfr[ig] * colscale[None, None, :]).transpose(1, 0, 2)
        xi = x[ig]
        cb32 = b32a.copy()
        cb32[:, 640:896] = mlt.reshape(128, NCH * IPC)
        cb16 = b16a.copy()
        cb16[:, 1024:1280] = mge.reshape(128, NCH * IPC)
        cb16[0:IPC, 1288:1800] = xi.astype(np.float16)
        m = dict(base)
        m.update(
            pf=shard.reshape(NCH, 128, IPC, PD),
            xi=xi.astype(np.float32),
            b32=cb32,
            b16=cb16,
        )
        in_maps.append(m)
    return in_maps


_COMPILED = None


def _get_program() -> bacc.Bacc:
    global _COMPILED
    if _COMPILED is None:
        nc = build_program()
        nc.compile()
        _COMPILED = nc
    return _COMPILED


TRACE = False
LAST_RESULT = None


def _install_axon_ntff_hook():
    """The container's antenv lacks axon_hooks; recreate it from trn_boot's
    ctypes implementation so trace=True can capture NTFF profiles."""
    import sys
    import types
    try:
        from antenv.axon_hooks import get_axon_ntff_profile_hook  # noqa: F401
        return
    except ImportError:
        pass
    from trn_agent_boot.trn_boot import _ntff_profile_via_ctypes
    hook = _ntff_profile_via_ctypes("/opt/axon/libaxon_pjrt.so")
    m = types.ModuleType("antenv.axon_hooks")
    m.get_axon_ntff_profile_hook = lambda: hook
    sys.modules["antenv.axon_hooks"] = m


def kernel(**inputs) -> np.ndarray:
    import concourse.bass_utils as bu
    from concourse.bass_utils import run_bass_kernel_spmd
    global LAST_RESULT
    if TRACE:
        _install_axon_ntff_hook()
        bu.upload_artifacts = lambda tmpdir: str(tmpdir)  # no bucket here
    nc = _get_program()
    in_maps = prep_in_maps(inputs)
    res = run_bass_kernel_spmd(nc, in_maps, list(range(NCORES)), trace=TRACE)
    LAST_RESULT = res
    outs = [res.results[c]["out"] for c in range(NCORES)]
    return np.concatenate(outs, axis=0).astype(np.float32)
